# revision 25
# baseline (speedup 1.0000x reference)
"""Trainium2 Bass kernel for nn_IsgnBeatMeasEncoder (gnn_message_passing).

Sharding: destination-node sharding for the gated-graph message passing
(128 dest-nodes/core; per-core adjacency slice resident in SBUF, fp16);
fp16 AllGather of the updated secondary state per graph iteration; nb
(graph-between) computed replicated from the gathered state (no second
collective); static message terms for g1 computed via beat/measure
aggregated adjacency; attention pooling via host-built one-hot matmuls;
BiLSTMs replicated with fw/bw batched and gate inputs pinned in PSUM.
"""
import numpy as np

import concourse.bass as bass
import concourse.mybir as mybir
from concourse import bacc
from concourse.tile import TileContext
from concourse import bass_utils

F32 = mybir.dt.float32
F16 = mybir.dt.float16

N = 1024
E = 10
IN = 78
NOTE = 128
BEAT = 64
MEAS = 32
S = 320
SEC = 128
HEADS = 8
NB = 256
NM = 64
SEQ_ITER = 2
GRAPH_ITER = 2
NCORES = 8
LOC = N // NCORES

FCS = [(0, 128), (128, 64), (192, 128)]  # (start, width); 0,1 static; 2 dyn

_CACHE = {}


def _input_specs():
    sp = dict(
        nodes_T16=((IN, N), F16),
        nodes_T_loc=((IN, LOC), F32),
        note_fc_w16=((IN, NOTE), F16),
        note_fc_b16=((1, NOTE), F16),
        note_fc_w32=((IN, NOTE), F32),
        note_fc_b32=((1, NOTE), F32),
        adj_sl=((E, 128, N), F16),
        adjB=((E, 128, 2 * LOC), F16),
        adjM=((E, NM, LOC), F16),
        gb_w16=((128, 3 * S), F16),
        gb_b16=((1, S), F16),
        gb_w32=((128, 3 * S), F32),
        gb_b32=((1, S), F32),
        batt_w=((2, 128, 2 * NOTE), F32),
        batt_b=((128, 2), F32),
        matt_w=((128, 2 * BEAT), F32),
        matt_b=((128, 1), F32),
        Cb=((2, 128, HEADS), F32),
        Cm=((128, HEADS), F32),
        Bfree_b=((HEADS, 2 * NOTE), F32),
        Bfree_m=((HEADS, 2 * BEAT), F32),
        Ppool=((8, 128, 32), F32),
        Ppoolm=((128, 2 * 32), F32),
        S_bs=((8, 128, 2 * 128), F16),   # half-chunked: [k][p,(half,c)]
        S_ms=((8, NM, 128), F16),
        S_bs_loc=((128, 2 * 128), F16),
        S_ms_loc=((NM, 128), F16),
        ident=((128, 128), F32),
        ident16=((128, 128), F16),
        bwhp=((2, 2, BEAT, 2 * BEAT), F16),   # [d][pair][H,(gA,gB)]
        bwip=((2, 2, 128, 4 * BEAT), F16),    # [d][kc][feat,(p0,p1)]
        bbp=((2, 1, 4 * BEAT), F16),
        mwhp=((2, 2, MEAS, 2 * MEAS), F16),
        mwip=((2, 1, 128, 4 * MEAS), F16),
        mbp=((2, 1, 4 * MEAS), F16),
        idmv64=((128, 64), F16),
        idmv32=((64, 32), F16),
        J128_16=((128, 128), F16),
        J64_16=((64, 64), F16),
        J32=((32, 32), F32),
    )
    for g in ("g1", "g2"):
        for gate in ("z", "r", "h"):
            sp[f"{g}_w{gate}_dhi"] = ((128, E * SEC), F16)
            sp[f"{g}_w{gate}_dlo"] = ((128, E * SEC), F16)
            sp[f"{g}_w{gate}_st"] = ((128, E * 2 * SEC), F16)
            sp[f"{g}_u{gate}16"] = ((SEC, SEC), F16)
            sp[f"{g}_b{gate}16"] = ((1, SEC), F16)
    return sp


def _build_program():
    nc = bacc.Bacc("TRN2", target_bir_lowering=False, debug=False,
                   num_devices=NCORES)
    io = {}
    for name, (shape, dt) in _input_specs().items():
        io[name] = nc.dram_tensor(name, list(shape), dt,
                                  kind="ExternalInput").ap()
    out_dram = nc.dram_tensor("out", [1, N, S + SEC], F32,
                              kind="ExternalOutput").ap()
    dbg_dram = nc.dram_tensor("dbg", [16, 128, 512], F32,
                              kind="ExternalOutput").ap()
    _CACHE["dbg_dram"] = dbg_dram
    ag = {}
    for i in range(8):
        ag[f"sec_in{i}"] = nc.dram_tensor(f"sec_in{i}", [LOC, SEC], F16).ap()
        ag[f"sec_out{i}"] = nc.dram_tensor(f"sec_out{i}", [N, SEC], F16,
                                           addr_space="Shared").ap()
    with TileContext(nc) as tc:
        _emit(nc, tc, io, out_dram, ag, dbg_dram)
    nc.compile()
    return nc


def _emit(nc, tc, io, out_dram, ag, dbg_dram):
    import contextlib
    RG = [list(range(NCORES))]
    AF = mybir.ActivationFunctionType
    OP = mybir.AluOpType
    MM = nc.tensor.matmul

    stack = contextlib.ExitStack()
    const = stack.enter_context(tc.tile_pool(name="const", bufs=1))
    pers = stack.enter_context(tc.tile_pool(name="pers", bufs=1))
    acts = stack.enter_context(tc.tile_pool(name="acts", bufs=2))
    dynp = stack.enter_context(tc.tile_pool(name="dynp", bufs=2))
    lsp = stack.enter_context(tc.tile_pool(name="lsp", bufs=6))
    ps_t = stack.enter_context(tc.tile_pool(name="ps_t", bufs=1, space="PSUM"))
    ps_m = stack.enter_context(tc.tile_pool(name="ps_m", bufs=2, space="PSUM"))

    def dma(dst, src):
        nc.sync.dma_start(out=dst, in_=src)

    def dump(idx, src_ap, rows, cols):
        st = acts.tile([128, 512], F32, tag="dbgst", name="dbgst")
        nc.vector.tensor_copy(st[0:rows, 0:cols], src_ap)
        dma(dbg_dram[idx, 0:rows, 0:cols], st[0:rows, 0:cols])

    cst = {}

    def load(name, dt=None, src=None, tag=None):
        src = io[name] if src is None else src
        if dt is None:
            dt = src.dtype
        t = const.tile([src.shape[-2], src.shape[-1]], dt, tag=tag or name)
        dma(t[:, :], src)
        cst[tag or name] = t
        return t

    for nm in ("nodes_T16", "nodes_T_loc", "note_fc_w16", "note_fc_b16",
               "note_fc_w32", "note_fc_b32", "gb_w16", "gb_b16", "gb_w32",
               "gb_b32", "batt_b", "matt_w", "matt_b", "Cm", "Bfree_b",
               "Bfree_m", "Ppoolm", "S_bs_loc", "S_ms_loc", "ident",
               "ident16", "idmv64", "idmv32", "J128_16", "J64_16", "J32"):
        load(nm)
    for kc in range(2):
        load("batt_w", src=io["batt_w"][kc], tag=f"battw{kc}")
        load("Cb", src=io["Cb"][kc], tag=f"Cb{kc}")
    for k in range(8):
        load("Ppool", src=io["Ppool"][k], tag=f"Ppool{k}")
        load("S_bs", src=io["S_bs"][k], tag=f"S_bs{k}")
        load("S_ms", src=io["S_ms"][k], tag=f"S_ms{k}")
    for e in range(E):
        load("adj_sl", src=io["adj_sl"][e], tag=f"adj{e}")
        load("adjB", src=io["adjB"][e], tag=f"adjB{e}")
        load("adjM", src=io["adjM"][e], tag=f"adjM{e}")
    for g in ("g1", "g2"):
        for gate in ("z", "r", "h"):
            load(f"{g}_w{gate}_dhi")
            load(f"{g}_w{gate}_dlo")
            load(f"{g}_w{gate}_st")
            load(f"{g}_u{gate}16")
            load(f"{g}_b{gate}16")
    for d in range(2):
        for p in range(2):
            load("bwhp", src=io["bwhp"][d, p], tag=f"bwhp{d}{p}")
            load("mwhp", src=io["mwhp"][d, p], tag=f"mwhp{d}{p}")
        for kc in range(2):
            load("bwip", src=io["bwip"][d, kc], tag=f"bwip{d}{kc}")
        load("mwip", src=io["mwip"][d, 0], tag=f"mwip{d}0")
        load("bbp", src=io["bbp"][d], tag=f"bbp{d}")
        load("mbp", src=io["mbp"][d], tag=f"mbp{d}")

    ones1 = const.tile([1, 512], F32, tag="ones1", name="ones1")
    nc.gpsimd.memset(ones1[:, :], 1.0)
    ones16 = const.tile([1, 512], F16, tag="ones16", name="ones16")
    nc.gpsimd.memset(ones16[:, :], 1.0)
    zsmall = const.tile([128, 2], F32, tag="zsmall", name="zsmall")
    nc.gpsimd.memset(zsmall[:, :], 0.0)
    z16 = const.tile([128, 2], F16, tag="z16", name="z16")
    nc.gpsimd.memset(z16[:, :], 0.0)
    ident = cst["ident"]
    ident16 = cst["ident16"]

    xb = [pers.tile([128, S], F16, tag=f"xb{k}", name=f"xb{k}")
          for k in range(8)]
    xl = pers.tile([128, S], F32, tag="xl", name="xl")
    nh16 = [pers.tile([128, SEC], F16, tag=f"nh16_{k}", name=f"nh16_{k}")
            for k in range(8)]
    nh216 = [pers.tile([128, SEC], F16, tag=f"nh216_{k}", name=f"nh216_{k}")
             for k in range(8)]
    bnT = [pers.tile([128, NB], F16, tag=f"bnT{h}", name=f"bnT{h}")
           for h in range(2)]
    bnTr = [pers.tile([128, NB], F16, tag=f"bnTr{h}", name=f"bnTr{h}")
            for h in range(2)]
    Hfb = pers.tile([BEAT, NB], F16, tag="Hfb", name="Hfb")
    Hbb = pers.tile([BEAT, NB], F16, tag="Hbb", name="Hbb")
    Hfm = pers.tile([MEAS, NM], F16, tag="Hfm", name="Hfm")
    Hbm = pers.tile([MEAS, NM], F16, tag="Hbm", name="Hbm")
    bh0 = pers.tile([128, 128], F32, tag="bh0", name="bh0")
    bh1 = pers.tile([128, 128], F32, tag="bh1", name="bh1")
    bh16 = [pers.tile([128, 128], F16, tag=f"bh16_{h}", name=f"bh16_{h}")
            for h in range(2)]
    bhT = pers.tile([128, NB], F32, tag="bhT", name="bhT")
    mh = pers.tile([NM, 2 * MEAS], F32, tag="mh", name="mh")
    mh16 = pers.tile([NM, 2 * MEAS], F16, tag="mh16", name="mh16")
    mnT = pers.tile([2 * BEAT, NM], F16, tag="mnT", name="mnT")
    mnTr = pers.tile([2 * BEAT, NM], F16, tag="mnTr", name="mnTr")
    mstat = pers.tile([128, 3 * SEC], F32, tag="mstat", name="mstat")
    nsl = pers.tile([128, SEC], F32, tag="nsl", name="nsl")

    def transpose_to(dst_ap, src_ap, rows):
        cols = src_ap.shape[-1]
        pt = ps_t.tile([128, 128], F32, tag="pt", name="pt")
        nc.tensor.transpose(pt[0:cols, 0:rows], src_ap,
                            ident[0:rows, 0:rows])
        nc.vector.tensor_copy(dst_ap, pt[0:cols, 0:rows])

    def transpose_new(src_ap, rows, tag="tr"):
        cols = src_ap.shape[-1]
        sb = acts.tile([cols, rows], F32, tag=tag, name=tag)
        transpose_to(sb[0:cols, 0:rows], src_ap, rows)
        return sb

    def transpose16(pool, src_ap, rows, tag):
        # fp16 src -> fp16 transposed SBUF tile
        cols = src_ap.shape[-1]
        pt = pool.tile([128, 128], F16, tag="pt16", name="pt16")
        nc.tensor.transpose(pt[0:cols, 0:rows], src_ap,
                            ident16[0:rows, 0:rows])
        sb = dynp.tile([128, 128], F16, tag=tag, name=tag)
        nc.vector.tensor_copy(sb[0:cols, 0:rows], pt[0:cols, 0:rows])
        return sb

    # ---------------- initial x ----------------
    for k in range(8):
        nc.gpsimd.memset(xb[k][:, 0:192], 0.0)
        pm = ps_m.tile([128, 512], F32, tag="pm", name="pm")
        MM(pm[:, 0:NOTE], cst["nodes_T16"][:, k * 128:(k + 1) * 128],
           cst["note_fc_w16"][:, :], start=True, stop=False)
        MM(pm[:, 0:NOTE], ones16[0:1, 0:128], cst["note_fc_b16"][:, :],
           start=False, stop=True)
        nc.scalar.activation(xb[k][:, 192:S], pm[:, 0:NOTE], AF.Tanh)
    nc.gpsimd.memset(xl[:, 0:192], 0.0)
    pm = ps_m.tile([128, 512], F32, tag="pm", name="pm")
    MM(pm[:, 0:NOTE], cst["nodes_T_loc"][:, :], cst["note_fc_w32"][:, :],
       start=True, stop=False)
    MM(pm[:, 0:NOTE], ones1[0:1, 0:LOC], cst["note_fc_b32"][:, :],
       start=False, stop=True)
    nc.scalar.activation(xl[:, 192:S], pm[:, 0:NOTE], AF.Tanh)
    dump(0, xb[0][:, 192:S], 128, 128)

    # ---------------- gated graph ----------------
    def gated_graph(g, static_mode, agins, agouts, nh_tiles, save_local):
        wdh = [cst[f"{g}_w{gt}_dhi"] for gt in "zrh"]
        wdl = [cst[f"{g}_w{gt}_dlo"] for gt in "zrh"]
        wst = [cst[f"{g}_w{gt}_st"] for gt in "zrh"]
        us = [cst[f"{g}_u{gt}16"] for gt in "zrh"]
        bs = [cst[f"{g}_b{gt}16"] for gt in "zrh"]
        with tc.tile_pool(name=f"ga{g}", bufs=1, space="PSUM") as gacc, \
                tc.tile_pool(name=f"gd{g}", bufs=2, space="PSUM") as gact:
            mt = [gacc.tile([128, SEC], F32, tag=f"m{gi}", name=f"m{gi}")
                  for gi in range(3)]
            # ---- static messages (constant across graph iters) ----
            if static_mode != "none":
                for e in range(E):
                    pa = gact.tile([128, 128], F32, tag="pact", name="pact")
                    if static_mode == "bh":
                        for ch in range(2):
                            MM(pa[:, :], bh16[ch][:, :],
                               cst[f"adjB{e}"][:, ch * 128:(ch + 1) * 128],
                               start=(ch == 0), stop=(ch == 1))
                    else:
                        for k in range(8):
                            MM(pa[:, :], xb[k][:, 0:128],
                               cst[f"adj{e}"][:, k * 128:(k + 1) * 128],
                               start=(k == 0), stop=(k == 7))
                    a0 = dynp.tile([128, 128], F16, tag="a0", name="a0")
                    nc.vector.tensor_copy(a0[:, :], pa[:, :])
                    pa1 = gact.tile([128, 128], F32, tag="pact", name="pact")
                    if static_mode == "bh":
                        MM(pa1[0:NM, :], mh16[:, :], cst[f"adjM{e}"][:, :],
                           start=True, stop=True)
                    else:
                        for k in range(8):
                            MM(pa1[0:64, :], xb[k][:, 128:192],
                               cst[f"adj{e}"][:, k * 128:(k + 1) * 128],
                               start=(k == 0), stop=(k == 7))
                    a1 = dynp.tile([128, 128], F16, tag="a1", name="a1")
                    nc.vector.tensor_copy(a1[0:64, :], pa1[0:64, :])
                    for gi in range(3):
                        MM(mt[gi][:, :], a0[:, :],
                           wst[gi][:, e * 256:e * 256 + 128],
                           start=(e == 0), stop=False)
                        MM(mt[gi][:, :], a1[0:64, :],
                           wst[gi][0:64, e * 256 + 128:e * 256 + 256],
                           start=False, stop=(e == E - 1))
                for gi in range(3):
                    nc.vector.tensor_copy(mstat[:, gi * SEC:(gi + 1) * SEC],
                                          mt[gi][:, :])
            # ---- graph iterations ----
            for it in range(GRAPH_ITER):
                last = it == GRAPH_ITER - 1
                for e in range(E):
                    pa = gact.tile([128, 128], F32, tag="pact", name="pact")
                    for k in range(8):
                        MM(pa[:, :], xb[k][:, 192:S],
                           cst[f"adj{e}"][:, k * 128:(k + 1) * 128],
                           start=(k == 0), stop=(k == 7))
                    ad = dynp.tile([128, 128], F16, tag="ad", name="ad")
                    nc.vector.tensor_copy(ad[:, :], pa[:, :])
                    for gi in range(3):
                        MM(mt[gi][:, :], ad[:, :],
                           wdh[gi][:, e * SEC:(e + 1) * SEC],
                           start=(e == 0), stop=False)
                        MM(mt[gi][:, :], ad[:, :],
                           wdl[gi][:, e * SEC:(e + 1) * SEC],
                           start=False, stop=False)
                xs = xl[:, 192:S]
                pt = ps_t.tile([128, 128], F32, tag="pt", name="pt")
                nc.tensor.transpose(pt[:, :], xs, ident[:, :])
                xsT = dynp.tile([128, 128], F16, tag="xsT", name="xsT")
                nc.vector.tensor_copy(xsT[:, :], pt[:, :])
                for gi in range(3):
                    MM(mt[gi][:, :], ones16[0:1, 0:128],
                       bs[gi][:, :], start=False, stop=False)
                for gi in range(2):
                    MM(mt[gi][:, :], xsT[:, :], us[gi][:, :],
                       start=False, stop=True)

                def gate_act(gi, func, dst):
                    reg = mt[gi][:, :]
                    if static_mode != "none":
                        tz = acts.tile([128, SEC], F32, tag="tz", name="tz")
                        nc.vector.tensor_tensor(
                            tz[:, :], reg, mstat[:, gi * SEC:(gi + 1) * SEC],
                            op=OP.add)
                        nc.scalar.activation(dst, tz[:, :], func)
                    else:
                        nc.scalar.activation(dst, reg, func)

                zt = acts.tile([128, SEC], F32, tag="zt", name="zt")
                rt = acts.tile([128, SEC], F32, tag="rt", name="rt")
                gate_act(0, AF.Sigmoid, zt[:, :])
                gate_act(1, AF.Sigmoid, rt[:, :])
                rx = acts.tile([128, SEC], F32, tag="rx", name="rx")
                nc.vector.tensor_tensor(rx[:, :], rt[:, :], xs, op=OP.mult)
                pt2 = ps_t.tile([128, 128], F32, tag="pt", name="pt")
                nc.tensor.transpose(pt2[:, :], rx[:, :], ident[:, :])
                rxT = dynp.tile([128, 128], F16, tag="rxT", name="rxT")
                nc.vector.tensor_copy(rxT[:, :], pt2[:, :])
                MM(mt[2][:, :], rxT[:, :], us[2][:, :],
                   start=False, stop=True)
                ht = acts.tile([128, SEC], F32, tag="ht", name="ht")
                gate_act(2, AF.Tanh, ht[:, :])
                t1 = acts.tile([128, SEC], F32, tag="t1", name="t1")
                nc.vector.tensor_tensor(t1[:, :], zt[:, :], xs, op=OP.mult)
                t2 = acts.tile([128, SEC], F32, tag="t2", name="t2")
                nc.vector.tensor_tensor(t2[:, :], rt[:, :], ht[:, :],
                                        op=OP.mult)
                ns = acts.tile([128, SEC], F32, tag="ns", name="ns")
                nc.vector.tensor_tensor(ns[:, :], xs, t1[:, :],
                                        op=OP.subtract)
                nc.vector.tensor_tensor(ns[:, :], ns[:, :], t2[:, :],
                                        op=OP.add)
                ns16 = acts.tile([128, SEC], F16, tag="ns16", name="ns16")
                nc.vector.tensor_copy(ns16[:, :], ns[:, :])
                if g == "g1" and it == 0 and dbgflag[0]:
                    dump(13, ns[:, :], 128, 128)
                a_in, a_out = agins[it], agouts[it]
                dma(a_in, ns16[:, :])
                nc.gpsimd.collective_compute(
                    "AllGather", OP.bypass, replica_groups=RG,
                    ins=[a_in], outs=[a_out])
                if not last:
                    for k in range(8):
                        dma(xb[k][:, 192:S], a_out[k * 128:(k + 1) * 128, :])
                else:
                    for k in range(8):
                        dma(nh_tiles[k][:, :],
                            a_out[k * 128:(k + 1) * 128, :])
                nc.vector.tensor_copy(xl[:, 192:S], ns[:, :])
                if last and save_local is not None:
                    nc.vector.tensor_copy(save_local[:, :], ns[:, :])

    # ---------------- nb (graph-between), replicated ----------------
    def nb_phase(s):
        fcs = [2] if s == 0 else [0, 1, 2]
        with tc.tile_pool(name="nbp", bufs=2, space="PSUM") as nbp:
          for k in range(8):
            pnb = ps_m.tile([128, 512], F32, tag="pm", name="pm")
            for fc in fcs:
                st, wd = FCS[fc]
                src = nh16[k][:, :] if fc == 2 else xb[k][:, st:st + wd]
                tT = transpose16(nbp, src, 128, tag="nbT")
                MM(pnb[:, 0:S], tT[0:wd, 0:128],
                   cst["gb_w16"][0:wd, fc * S:(fc + 1) * S],
                   start=(fc == fcs[0]), stop=False)
            MM(pnb[:, 0:S], ones16[0:1, 0:128], cst["gb_b16"][:, :],
               start=False, stop=True)
            nc.scalar.activation(xb[k][:, 0:S], pnb[:, 0:S], AF.Relu)
        # local f32 copy (xl holds nh locally: primary + nsl sec)
        pnl = ps_m.tile([128, 512], F32, tag="pm", name="pm")
        for fc in fcs:
            st, wd = FCS[fc]
            tT = transpose_new(xl[:, st:st + wd], 128, tag="nbT32")
            MM(pnl[:, 0:S], tT[0:wd, 0:128],
               cst["gb_w32"][0:wd, fc * S:(fc + 1) * S],
               start=(fc == fcs[0]), stop=False)
        MM(pnl[:, 0:S], ones1[0:1, 0:128], cst["gb_b32"][:, :],
           start=False, stop=True)
        nc.scalar.activation(xl[:, 0:S], pnl[:, 0:S], AF.Relu)

    # ---------------- beat attention ----------------
    def beat_attention():
        for k in range(8):
            cat_nm = acts.tile([128, 2 * NOTE], F32, tag="cat_nm",
                               name="cat_nm")
            nc.vector.tensor_copy(cat_nm[:, 0:NOTE], nh16[k][:, :])
            nc.vector.tensor_copy(cat_nm[:, NOTE:2 * NOTE], nh216[k][:, :])
            ct = [transpose_new(cat_nm[:, kc * 128:(kc + 1) * 128], 128,
                                tag=f"ct{kc}") for kc in range(2)]
            aT = []
            for mc in range(2):
                pa = ps_m.tile([128, 512], F32, tag="pm", name="pm")
                for kc in range(2):
                    MM(pa[:, 0:128],
                       cst[f"battw{kc}"][:, mc * 128:(mc + 1) * 128],
                       ct[kc][:, :], start=(kc == 0), stop=(kc == 1))
                sb = acts.tile([128, 128], F32, tag=f"aT{mc}", name=f"aT{mc}")
                nc.scalar.activation(sb[:, :], pa[:, 0:128], AF.Tanh,
                                     bias=cst["batt_b"][:, mc:mc + 1])
                aT.append(sb)
            psim = ps_t.tile([128, 128], F32, tag="pt", name="pt")
            for kc in range(2):
                MM(psim[0:HEADS, :], cst[f"Cb{kc}"][:, :], aT[kc][:, :],
                   start=(kc == 0), stop=(kc == 1))
            pp = acts.tile([HEADS, 128], F32, tag="pp", name="pp")
            nc.scalar.activation(pp[:, :], psim[0:HEADS, :], AF.Sigmoid)
            qq = acts.tile([HEADS, 128], F32, tag="qq", name="qq")
            nc.scalar.activation(qq[:, :], psim[0:HEADS, :], AF.Sigmoid,
                                 scale=-1.0)
            rq = acts.tile([HEADS, 128], F32, tag="rq", name="rq")
            nc.vector.reciprocal(rq[:, :], qq[:, :])
            wt = acts.tile([HEADS, 128], F32, tag="wt", name="wt")
            nc.vector.tensor_tensor(wt[:, :], pp[:, :], rq[:, :], op=OP.mult)
            pwe = ps_m.tile([128, 512], F32, tag="pm", name="pm")
            wexp = acts.tile([128, 2 * NOTE], F32, tag="wexp", name="wexp")
            MM(pwe[:, 0:256], wt[:, :], cst["Bfree_b"][:, :],
               start=True, stop=True)
            nc.vector.tensor_copy(wexp[:, :], pwe[:, 0:256])
            tt = acts.tile([128, 2 * NOTE], F32, tag="tt", name="tt")
            nc.vector.tensor_tensor(tt[:, :], cat_nm[:, :], wexp[:, :],
                                    op=OP.mult)
            pool = ps_m.tile([128, 512], F32, tag="pm", name="pm")
            MM(pool[0:32, 0:256], cst[f"Ppool{k}"][:, :], tt[:, :],
               start=True, stop=True)
            MM(pool[0:32, 256:512], cst[f"Ppool{k}"][:, :], wexp[:, :],
               start=True, stop=True)
            rd = acts.tile([32, 256], F32, tag="rd", name="rd")
            nc.vector.reciprocal(rd[:, :], pool[0:32, 256:512])
            bnk = acts.tile([32, 256], F32, tag="bnk", name="bnk")
            nc.vector.tensor_tensor(bnk[:, :], pool[0:32, 0:256], rd[:, :],
                                    op=OP.mult)
            for h in range(2):
                transpose_to(bnT[h][:, k * 32:(k + 1) * 32],
                             bnk[0:32, h * 128:(h + 1) * 128], 32)
                ptr = ps_t.tile([128, 128], F32, tag="pt", name="pt")
                nc.tensor.transpose(ptr[0:128, 0:32],
                                    bnk[0:32, h * 128:(h + 1) * 128],
                                    cst["J32"][:, :])
                nc.vector.tensor_copy(bnTr[h][:, (7 - k) * 32:(8 - k) * 32],
                                      ptr[0:128, 0:32])

    # ---------------- measure attention ----------------
    def measure_attention():
        paT = ps_m.tile([128, 512], F32, tag="pm", name="pm")
        MM(paT[:, 0:NB], cst["matt_w"][:, :], bhT[:, :],
           start=True, stop=True)
        amT = acts.tile([128, NB], F32, tag="amT", name="amT")
        nc.scalar.activation(amT[:, :], paT[:, 0:NB], AF.Tanh,
                             bias=cst["matt_b"][:, 0:1])
        psim = ps_t.tile([128, 128], F32, tag="pt", name="pt")
        pp = acts.tile([HEADS, NB], F32, tag="ppm", name="ppm")
        qq = acts.tile([HEADS, NB], F32, tag="qqm", name="qqm")
        for hc in range(2):
            MM(psim[0:HEADS, 0:128], cst["Cm"][:, :],
               amT[:, hc * 128:(hc + 1) * 128], start=True, stop=True)
            nc.scalar.activation(pp[:, hc * 128:(hc + 1) * 128],
                                 psim[0:HEADS, 0:128], AF.Sigmoid)
            nc.scalar.activation(qq[:, hc * 128:(hc + 1) * 128],
                                 psim[0:HEADS, 0:128], AF.Sigmoid,
                                 scale=-1.0)
        rq = acts.tile([HEADS, NB], F32, tag="rqm", name="rqm")
        nc.vector.reciprocal(rq[:, :], qq[:, :])
        wt = acts.tile([HEADS, NB], F32, tag="wtm", name="wtm")
        nc.vector.tensor_tensor(wt[:, :], pp[:, :], rq[:, :], op=OP.mult)
        for h in range(2):
            bh_h = bh0 if h == 0 else bh1
            pwe = ps_m.tile([128, 512], F32, tag="pm", name="pm")
            MM(pwe[:, 0:2 * BEAT], wt[:, h * 128:(h + 1) * 128],
               cst["Bfree_m"][:, :], start=True, stop=True)
            wexp = acts.tile([128, 2 * BEAT], F32, tag="wexpm", name="wexpm")
            nc.vector.tensor_copy(wexp[:, :], pwe[:, 0:2 * BEAT])
            tt = acts.tile([128, 2 * BEAT], F32, tag="ttm", name="ttm")
            nc.vector.tensor_tensor(tt[:, :], bh_h[:, :], wexp[:, :],
                                    op=OP.mult)
            pool = ps_m.tile([128, 512], F32, tag="pm", name="pm")
            MM(pool[0:32, 0:128], cst["Ppoolm"][:, h * 32:(h + 1) * 32],
               tt[:, :], start=True, stop=True)
            MM(pool[0:32, 128:256], cst["Ppoolm"][:, h * 32:(h + 1) * 32],
               wexp[:, :], start=True, stop=True)
            rd = acts.tile([32, 128], F32, tag="rdm", name="rdm")
            nc.vector.reciprocal(rd[:, :], pool[0:32, 128:256])
            mnk = acts.tile([32, 128], F32, tag="mnk", name="mnk")
            nc.vector.tensor_tensor(mnk[:, :], pool[0:32, 0:128], rd[:, :],
                                    op=OP.mult)
            transpose_to(mnT[:, h * 32:(h + 1) * 32], mnk[0:32, :], 32)
            ptr = ps_t.tile([128, 128], F32, tag="pt", name="pt")
            nc.tensor.transpose(ptr[0:128, 0:32], mnk[0:32, :],
                                cst["J32"][:, :])
            nc.vector.tensor_copy(mnTr[:, (1 - h) * 32:(2 - h) * 32],
                                  ptr[0:128, 0:32])

    # ---------------- LSTM ----------------
    def run_lstm2(H, T, inT, inTr, whp, wip, bp, nkc, Hf_t, Hb_t):
        """Decoupled fw/bw scan. Per dir PSUM U [2H, 2T]:
        pair0 cols 0:T = (f @ rows 0:H ; i @ rows H:2H),
        pair1 cols T:2T = (o ; 2*g~). All gates sigmoid (tanh folded as
        2*sigma(2x)-1 with weights/bias pre-scaled). c and h live at rows
        0:H; the i*g~ product is moved from rows H:2H via a PE
        identity-matmul."""
        Hs = [Hf_t, Hb_t]
        with tc.tile_pool(name=f"lu{H}", bufs=1, space="PSUM") as up, \
                tc.tile_pool(name=f"lm{H}", bufs=2, space="PSUM") as mp:
            U = [up.tile([2 * H, 2 * T], F32, tag=f"U{d}", name=f"U{H}{d}")
                 for d in range(2)]
            for d in range(2):
                srcs = inT if d == 0 else inTr
                for p in range(2):
                    reg = U[d][:, p * T:(p + 1) * T]
                    for kc in range(nkc):
                        MM(reg,
                           cst[f"{wip}{d}{kc}"][:, p * 2 * H:(p + 1) * 2 * H],
                           srcs[kc], start=(kc == 0 and p == 0), stop=False)
                    MM(reg, cst[f"{bp}{d}"][0:1, p * 2 * H:(p + 1) * 2 * H],
                       ones16[0:1, 0:T], start=False, stop=False)
            idm = cst["idmv64" if H == 64 else "idmv32"]
            cs = [None, None]
            for t in range(T):
                for d in range(2):
                    rhs = z16[0:H, 0:1] if t == 0 else Hs[d][0:H, t - 1:t]
                    MM(U[d][:, t:t + 1], cst[f"{whp}{d}0"][:, :], rhs,
                       start=False, stop=True)
                    MM(U[d][:, T + t:T + t + 1], cst[f"{whp}{d}1"][:, :],
                       rhs, start=False, stop=True)
                sio = []
                for d in range(2):
                    sv = lsp.tile([2 * H, 2], F32, tag=f"sio{H}{d}",
                                  name=f"sio{H}{d}")
                    Uv = U[d].rearrange("p (pr t) -> p pr t", pr=2, t=T)
                    nc.scalar.activation(
                        sv[0:2 * H, :].rearrange("p (a b) -> p a b",
                                                 a=2, b=1),
                        Uv[:, :, t:t + 1], AF.Sigmoid)
                    sio.append(sv)
                if H == 64 and t == 0 and dbgflag[0]:
                    dump(10, sio[0][:, :], 128, 2)
                    dump(11, U[0][:, 0:8], 128, 8)
                    dump(12, U[0][:, T:T + 8], 128, 8)
                for d in range(2):
                    sv = sio[d]
                    t1 = lsp.tile([2 * H, 2], F32, tag=f"t1{H}{d}",
                                  name=f"t1{H}{d}")
                    nc.vector.tensor_tensor(t1[H:2 * H, 0:1],
                                            sv[H:2 * H, 0:1],
                                            sv[H:2 * H, 1:2], op=OP.mult)
                    u16 = lsp.tile([2 * H, 2], F16, tag=f"u{H}{d}",
                                   name=f"u{H}{d}")
                    nc.vector.scalar_tensor_tensor(
                        u16[H:2 * H, 0:1], t1[H:2 * H, 0:1], 2.0,
                        sv[H:2 * H, 0:1], op0=OP.mult, op1=OP.subtract)
                    mv = mp.tile([128, 8], F32, tag=f"mv{H}",
                                 name=f"mv{H}")
                    MM(mv[0:H, 0:1], idm[H:2 * H, 0:H], u16[H:2 * H, 0:1],
                       start=True, stop=True)
                    v = lsp.tile([2 * H, 2], F32, tag=f"v{H}{d}",
                                 name=f"v{H}{d}")
                    cprev = zsmall[0:H, 0:1] if t == 0 else cs[d][0:H, 0:1]
                    nc.vector.tensor_tensor(v[0:H, 0:1], sv[0:H, 0:1],
                                            cprev, op=OP.mult)
                    cn = lsp.tile([2 * H, 2], F32, tag=f"cn{H}{d}",
                                  name=f"cn{H}{d}")
                    nc.vector.tensor_tensor(cn[0:H, 0:1], v[0:H, 0:1],
                                            mv[0:H, 0:1], op=OP.add)
                    cs[d] = cn
                    tcx = lsp.tile([2 * H, 2], F32, tag=f"tc{H}{d}",
                                   name=f"tc{H}{d}")
                    nc.scalar.activation(tcx[0:H, 0:1], cn[0:H, 0:1],
                                         AF.Sigmoid, scale=2.0)
                    t2 = lsp.tile([2 * H, 2], F32, tag=f"t2{H}{d}",
                                  name=f"t2{H}{d}")
                    nc.vector.tensor_tensor(t2[0:H, 0:1], sv[0:H, 1:2],
                                            tcx[0:H, 0:1], op=OP.mult)
                    nc.vector.scalar_tensor_tensor(
                        Hs[d][0:H, t:t + 1], t2[0:H, 0:1], 2.0,
                        sv[0:H, 1:2], op0=OP.mult, op1=OP.subtract)

    def build_beat():
        with tc.tile_pool(name="bhp", bufs=2, space="PSUM") as bp2:
            for half in range(2):
                bh_h = bh0 if half == 0 else bh1
                ptA = bp2.tile([128, 128], F16, tag="ptA", name="ptA")
                nc.tensor.transpose(ptA[0:128, 0:BEAT],
                                    Hfb[:, half * 128:(half + 1) * 128],
                                    ident16[0:BEAT, 0:BEAT])
                nc.vector.tensor_copy(bh_h[:, 0:BEAT], ptA[0:128, 0:BEAT])
                ptB = bp2.tile([128, 128], F16, tag="ptA", name="ptA")
                nc.tensor.transpose(
                    ptB[0:128, 0:BEAT],
                    Hbb[:, (1 - half) * 128:(2 - half) * 128],
                    ident16[0:BEAT, 0:BEAT])
                C1 = dynp.tile([128, 128], F16, tag="C1", name="C1")
                nc.vector.tensor_copy(C1[0:128, 0:BEAT],
                                      ptB[0:128, 0:BEAT])
                pf = bp2.tile([128, 128], F32, tag="ptF", name="ptF")
                MM(pf[0:128, 0:BEAT], cst["J128_16"][:, :],
                   C1[0:128, 0:BEAT], start=True, stop=True)
                nc.vector.tensor_copy(bh_h[:, BEAT:2 * BEAT],
                                      pf[0:128, 0:BEAT])
                nc.vector.tensor_copy(bh16[half][:, :], bh_h[:, :])
                transpose_to(bhT[:, half * 128:(half + 1) * 128],
                             bh_h[:, :], 128)

    def build_meas():
        with tc.tile_pool(name="mhp", bufs=2, space="PSUM") as mp2:
            ptA = mp2.tile([64, 64], F16, tag="ptA", name="ptA")
            nc.tensor.transpose(ptA[0:NM, 0:MEAS], Hfm[:, :],
                                ident16[0:MEAS, 0:MEAS])
            nc.vector.tensor_copy(mh[:, 0:MEAS], ptA[0:NM, 0:MEAS])
            ptB = mp2.tile([64, 64], F16, tag="ptA", name="ptA")
            nc.tensor.transpose(ptB[0:NM, 0:MEAS], Hbm[:, :],
                                ident16[0:MEAS, 0:MEAS])
            C1 = dynp.tile([128, 128], F16, tag="C1", name="C1")
            nc.vector.tensor_copy(C1[0:NM, 0:MEAS], ptB[0:NM, 0:MEAS])
            pf = mp2.tile([64, 64], F32, tag="ptF", name="ptF")
            MM(pf[0:NM, 0:MEAS], cst["J64_16"][:, :], C1[0:NM, 0:MEAS],
               start=True, stop=True)
            nc.vector.tensor_copy(mh[:, MEAS:2 * MEAS], pf[0:NM, 0:MEAS])
            nc.vector.tensor_copy(mh16[:, :], mh[:, :])

    # ---------------- main sequence ----------------
    dbgflag = [True]
    for s in range(SEQ_ITER):
        dbgflag[0] = s == 0
        with nc.named_scope(f"g1_{s}"):
            gated_graph("g1", "none" if s == 0 else "bh",
                        [ag[f"sec_in{s * 4 + i}"] for i in range(2)],
                        [ag[f"sec_out{s * 4 + i}"] for i in range(2)],
                        nh16, nsl)
        dump(1 if s == 0 else 8, nh16[0][:, :], 128, 128)
        if s == 1:
            dump(9, mstat[:, :], 128, 384)
        with nc.named_scope(f"nb_{s}"):
            nb_phase(s)
        if s == 0:
            dump(2, xb[0][:, 0:S], 128, S)
        with nc.named_scope(f"g2_{s}"):
            gated_graph("g2", "generic",
                        [ag[f"sec_in{s * 4 + 2 + i}"] for i in range(2)],
                        [ag[f"sec_out{s * 4 + 2 + i}"] for i in range(2)],
                        nh216, None)
        if s == 0:
            dump(3, nh216[0][:, :], 128, 128)
        with nc.named_scope(f"batt_{s}"):
            beat_attention()
        if s == 0:
            dump(4, bnT[0][:, :], 128, 256)
        with nc.named_scope(f"blstm_{s}"):
            run_lstm2(BEAT, NB, [bnT[0][:, :], bnT[1][:, :]],
                      [bnTr[0][:, :], bnTr[1][:, :]], "bwhp", "bwip",
                      "bbp", 2, Hfb, Hbb)
            build_beat()
            if s == 0:
                dump(5, bh0[:, :], 128, 128)
        with nc.named_scope(f"matt_{s}"):
            measure_attention()
        with nc.named_scope(f"mlstm_{s}"):
            run_lstm2(MEAS, NM, [mnT[:, :]], [mnTr[:, :]], "mwhp",
                      "mwip", "mbp", 1, Hfm, Hbm)
            build_meas()
            if s == 0:
                dump(6, mh[:, :], NM, 2 * MEAS)
        # rebuild x tiles for next iteration / final output
        with nc.named_scope(f"rebuild_{s}"):
            for k in range(8):
                pbs = ps_m.tile([128, 512], F32, tag="pm", name="pm")
                for half in range(2):
                    MM(pbs[:, 0:128],
                       cst[f"S_bs{k}"][:, half * 128:(half + 1) * 128],
                       bh16[half][:, :], start=(half == 0), stop=(half == 1))
                MM(pbs[:, 128:192], cst[f"S_ms{k}"][:, :], mh16[:, :],
                   start=True, stop=True)
                if s + 1 < SEQ_ITER:
                    nc.vector.tensor_copy(xb[k][:, 0:192], pbs[:, 0:192])
                    nc.vector.tensor_copy(xb[k][:, 192:S], nh16[k][:, :])
                else:
                    outst = acts.tile([128, S + SEC], F32, tag="outst",
                                      name="outst")
                    nc.vector.tensor_copy(outst[:, 0:192], pbs[:, 0:192])
                    nc.vector.tensor_copy(outst[:, 192:S], nh16[k][:, :])
                    nc.vector.tensor_copy(outst[:, S:S + SEC], nh216[k][:, :])
                    dma(out_dram[0, k * 128:(k + 1) * 128, :], outst[:, :])
            if s + 1 < SEQ_ITER:
                pbs = ps_m.tile([128, 512], F32, tag="pm", name="pm")
                for half in range(2):
                    MM(pbs[:, 0:128],
                       cst["S_bs_loc"][:, half * 128:(half + 1) * 128],
                       bh16[half][:, :], start=(half == 0), stop=(half == 1))
                MM(pbs[:, 128:192], cst["S_ms_loc"][:, :], mh16[:, :],
                   start=True, stop=True)
                nc.vector.tensor_copy(xl[:, 0:192], pbs[:, 0:192])
                nc.vector.tensor_copy(xl[:, 192:S], nsl[:, :])
                dump(7, xb[0][:, 0:S], 128, S)
    stack.close()


# ================= host side =================

def _host_inputs(inputs):
    f32, f16 = np.float32, np.float16
    nodes = np.asarray(inputs["nodes"], f32)[0]
    adjacency = np.asarray(inputs["adjacency"], f32)
    beat = np.asarray(inputs["beat_numbers"], np.int64)
    meas = np.asarray(inputs["measure_numbers"], np.int64)
    rep = {}
    rep["nodes_T16"] = nodes.T.astype(f16)
    fw = np.asarray(inputs["note_fc_w"], f32)
    fb = np.asarray(inputs["note_fc_b"], f32)[None, :]
    rep["note_fc_w16"] = fw.astype(f16)
    rep["note_fc_b16"] = fb.astype(f16)
    rep["note_fc_w32"] = fw
    rep["note_fc_b32"] = fb
    gbw = np.zeros((128, 3, S), f32)
    gw = np.asarray(inputs["gb_w"], f32)
    for fc, (st, w) in enumerate(FCS):
        gbw[0:w, fc, :] = gw[st:st + w, :]
    rep["gb_w32"] = gbw.reshape(128, 3 * S)
    rep["gb_b32"] = np.asarray(inputs["gb_b"], f32)[None, :]
    rep["gb_w16"] = rep["gb_w32"].astype(f16)
    rep["gb_b16"] = rep["gb_b32"].astype(f16)
    rep["batt_w"] = np.asarray(inputs["batt_w"], f32).reshape(2, 128,
                                                             2 * NOTE)
    rep["batt_b"] = np.asarray(inputs["batt_b"],
                               f32).reshape(2, 128).T.copy()
    rep["matt_w"] = np.asarray(inputs["matt_w"], f32)
    rep["matt_b"] = np.asarray(inputs["matt_b"], f32)[:, None]
    bc = np.asarray(inputs["batt_c"], f32)
    Cb = np.zeros((2 * NOTE, HEADS), f32)
    for h in range(HEADS):
        Cb[h * 32:(h + 1) * 32, h] = bc[h]
    rep["Cb"] = Cb.reshape(2, 128, HEADS)
    mcc = np.asarray(inputs["matt_c"], f32)
    Cm = np.zeros((2 * BEAT, HEADS), f32)
    for h in range(HEADS):
        Cm[h * 16:(h + 1) * 16, h] = mcc[h]
    rep["Cm"] = Cm
    Bf = np.zeros((HEADS, 2 * NOTE), f32)
    for h in range(HEADS):
        Bf[h, h * 32:(h + 1) * 32] = 1.0
    rep["Bfree_b"] = Bf
    Bm = np.zeros((HEADS, 2 * BEAT), f32)
    for h in range(HEADS):
        Bm[h, h * 16:(h + 1) * 16] = 1.0
    rep["Bfree_m"] = Bm
    Ppool = np.zeros((8, 128, 32), f32)
    for k in range(8):
        for p in range(128):
            b = beat[k * 128 + p] - 32 * k
            assert 0 <= b < 32, "beats not aligned to 128-node chunks"
            Ppool[k, p, b] = 1.0
    rep["Ppool"] = Ppool
    b2m = np.full(NB, 1 << 40, np.int64)
    np.minimum.at(b2m, beat, meas)
    Ppoolm = np.zeros((2, 128, 32), f32)
    for half in range(2):
        for p in range(128):
            m_ = b2m[half * 128 + p] - 32 * half
            assert 0 <= m_ < 32
            Ppoolm[half, p, m_] = 1.0
    rep["Ppoolm"] = np.concatenate([Ppoolm[0], Ppoolm[1]], axis=1)
    S_bs = np.zeros((8, NB, 128), f32)
    S_ms = np.zeros((8, NM, 128), f32)
    for k in range(8):
        for p in range(128):
            S_bs[k, beat[k * 128 + p], p] = 1.0
            S_ms[k, meas[k * 128 + p], p] = 1.0
    S_bs_hc = np.zeros((8, 128, 2, 128), f32)
    for k in range(8):
        S_bs_hc[k, :, 0, :] = S_bs[k, 0:128, :]
        S_bs_hc[k, :, 1, :] = S_bs[k, 128:256, :]
    rep["S_bs"] = S_bs_hc.reshape(8, 128, 256).astype(f16)
    rep["S_ms"] = S_ms.astype(f16)
    rep["ident"] = np.eye(128, dtype=f32)
    rep["ident16"] = np.eye(128, dtype=f32).astype(f16)

    def lstm_pack2(wi_f, wh_f, b_f, wi_b, wh_b, b_b, H):
        # pair0 = (f; i), pair1 = (o; 2*g); reference gate order i,f,g,o
        KIN = wi_f.shape[1]
        nkc = max(1, KIN // 128)
        whp = np.zeros((2, 2, H, 2 * H), f32)
        wip = np.zeros((2, nkc, 128, 4 * H), f32)
        bp = np.zeros((2, 1, 4 * H), f32)
        for d, (wi, wh, b) in enumerate(((wi_f, wh_f, b_f),
                                         (wi_b, wh_b, b_b))):
            blk = [wh[i * H:(i + 1) * H] for i in range(4)]  # i,f,g,o
            whp[d, 0, :, 0:H] = blk[1].T
            whp[d, 0, :, H:2 * H] = blk[0].T
            whp[d, 1, :, 0:H] = blk[3].T
            whp[d, 1, :, H:2 * H] = 2.0 * blk[2].T
            wt = wi.T  # (KIN, 4H) col blocks i,f,g,o
            for kc in range(nkc):
                w = wt[kc * 128:(kc + 1) * 128]
                r = w.shape[0]
                wip[d, kc, 0:r, 0:H] = w[:, H:2 * H]
                wip[d, kc, 0:r, H:2 * H] = w[:, 0:H]
                wip[d, kc, 0:r, 2 * H:3 * H] = w[:, 3 * H:4 * H]
                wip[d, kc, 0:r, 3 * H:4 * H] = 2.0 * w[:, 2 * H:3 * H]
            bp[d, 0, 0:H] = b[H:2 * H]
            bp[d, 0, H:2 * H] = b[0:H]
            bp[d, 0, 2 * H:3 * H] = b[3 * H:4 * H]
            bp[d, 0, 3 * H:4 * H] = 2.0 * b[2 * H:3 * H]
        return whp.astype(f16), wip.astype(f16), bp.astype(f16)

    g = lambda n: np.asarray(inputs[n], f32)
    rep["bwhp"], rep["bwip"], rep["bbp"] = lstm_pack2(
        g("blstm_wi_f"), g("blstm_wh_f"), g("blstm_b_f"),
        g("blstm_wi_b"), g("blstm_wh_b"), g("blstm_b_b"), BEAT)
    rep["mwhp"], rep["mwip"], rep["mbp"] = lstm_pack2(
        g("mlstm_wi_f"), g("mlstm_wh_f"), g("mlstm_b_f"),
        g("mlstm_wi_b"), g("mlstm_wh_b"), g("mlstm_b_b"), MEAS)
    idmv64 = np.zeros((128, 64), f32)
    idmv64[64:128] = np.eye(64)
    rep["idmv64"] = idmv64.astype(f16)
    idmv32 = np.zeros((64, 32), f32)
    idmv32[32:64] = np.eye(32)
    rep["idmv32"] = idmv32.astype(f16)
    rep["J128_16"] = np.eye(128, dtype=f32)[::-1].astype(f16)
    rep["J64_16"] = np.eye(64, dtype=f32)[::-1].astype(f16)
    rep["J32"] = np.eye(32, dtype=f32)[::-1].copy()
    for gg in ("g1", "g2"):
        for gate in ("z", "r", "h"):
            w = np.asarray(inputs[f"{gg}_w{gate}"], f32)  # (E, S, SEC)
            dyn = w[:, 192:320, :]                        # (E, 128, SEC)
            dhi = dyn.astype(f16)
            dlo = (dyn - dhi.astype(f32)).astype(f16)
            rep[f"{gg}_w{gate}_dhi"] = \
                dhi.transpose(1, 0, 2).reshape(128, E * SEC).copy()
            rep[f"{gg}_w{gate}_dlo"] = \
                dlo.transpose(1, 0, 2).reshape(128, E * SEC).copy()
            ws = np.zeros((128, E, 2 * SEC), f16)
            ws[0:128, :, 0:SEC] = w[:, 0:128, :].transpose(1, 0, 2)
            ws[0:64, :, SEC:2 * SEC] = w[:, 128:192, :].transpose(1, 0, 2)
            rep[f"{gg}_w{gate}_st"] = ws.reshape(128, E * 2 * SEC)
            rep[f"{gg}_u{gate}16"] = \
                np.asarray(inputs[f"{gg}_u{gate}"], f32).astype(f16)
            rep[f"{gg}_b{gate}16"] = \
                np.asarray(inputs[f"{gg}_b{gate}"], f32)[None, :].astype(f16)

    # beat/measure aggregated adjacency (static message terms for g1)
    if np.array_equal(beat, np.arange(N) // 4):
        adjB_full = adjacency.reshape(E, NB, 4, N).sum(2)
    else:
        Sb = np.zeros((NB, N), f32)
        Sb[beat, np.arange(N)] = 1.0
        adjB_full = np.einsum("bm,emn->ebn", Sb, adjacency)
    if np.array_equal(b2m, np.arange(NB) // 4):
        adjM_full = adjB_full.reshape(E, NM, 4, N).sum(2)
    else:
        Sm = np.zeros((NM, NB), f32)
        Sm[b2m, np.arange(NB)] = 1.0
        adjM_full = np.einsum("mb,ebn->emn", Sm, adjB_full)

    specs = _input_specs()
    # convert replicated entries once
    for kk in list(rep.keys()):
        shape, dt = specs[kk]
        npdt = np.float16 if dt == F16 else np.float32
        rep[kk] = np.ascontiguousarray(
            np.asarray(rep[kk]).reshape(shape).astype(npdt))
    in_maps = []
    for c in range(NCORES):
        sl = slice(c * LOC, (c + 1) * LOC)
        m = dict(rep)
        m["nodes_T_loc"] = np.ascontiguousarray(nodes[sl].T)
        adjc = adjacency[:, :, sl]
        m["adj_sl"] = np.ascontiguousarray(
            adjc.reshape(E, 8, 128, LOC).transpose(0, 2, 1, 3)
            .reshape(E, 128, N).astype(f16))
        m["adjB"] = np.ascontiguousarray(
            adjB_full[:, :, sl].reshape(E, 2, 128, LOC)
            .transpose(0, 2, 1, 3).reshape(E, 128, 2 * LOC).astype(f16))
        m["adjM"] = np.ascontiguousarray(adjM_full[:, :, sl].astype(f16))
        m["S_bs_loc"] = rep["S_bs"][c]
        m["S_ms_loc"] = rep["S_ms"][c]
        in_maps.append(m)
    return in_maps


def kernel(**inputs):
    if "nc" not in _CACHE:
        _CACHE["nc"] = _build_program()
    nc = _CACHE["nc"]
    in_maps = _host_inputs(inputs)
    res = bass_utils.run_bass_kernel_spmd(nc, in_maps,
                                          core_ids=list(range(NCORES)))
    _CACHE["last_res"] = res
    out = res.results[0]["out"]
    return np.asarray(out, np.float32)


# revision 29
# speedup vs baseline: 1.1079x; 1.1079x over previous
"""Trainium2 Bass kernel for nn_IsgnBeatMeasEncoder (gnn_message_passing).

Sharding: destination-node sharding for the gated-graph message passing
(128 dest-nodes/core; per-core adjacency slice resident in SBUF, fp16);
fp16 AllGather of the updated secondary state per graph iteration; nb
(graph-between) computed replicated from the gathered state (no second
collective); static message terms for g1 computed via beat/measure
aggregated adjacency; attention pooling via host-built one-hot matmuls;
BiLSTMs replicated with fw/bw batched and gate inputs pinned in PSUM.
"""
import numpy as np

import concourse.bass as bass
import concourse.mybir as mybir
from concourse import bacc
from concourse.tile import TileContext
from concourse import bass_utils

F32 = mybir.dt.float32
F16 = mybir.dt.float16

N = 1024
E = 10
IN = 78
NOTE = 128
BEAT = 64
MEAS = 32
S = 320
SEC = 128
HEADS = 8
NB = 256
NM = 64
SEQ_ITER = 2
GRAPH_ITER = 2
NCORES = 8
LOC = N // NCORES

FCS = [(0, 128), (128, 64), (192, 128)]  # (start, width); 0,1 static; 2 dyn

_CACHE = {}


def _input_specs():
    sp = dict(
        nodes_T16=((IN, N), F16),
        nodes_T_loc=((IN, LOC), F32),
        note_fc_w16=((IN, NOTE), F16),
        note_fc_b16=((1, NOTE), F16),
        note_fc_w32=((IN, NOTE), F32),
        note_fc_b32=((1, NOTE), F32),
        adj_sl=((E, 128, N), F16),
        adjB=((E, 128, 2 * LOC), F16),
        adjM=((E, NM, LOC), F16),
        gb_w16=((128, 3 * S), F16),
        gb_b16=((1, S), F16),
        gb_w32=((128, 3 * S), F32),
        gb_b32=((1, S), F32),
        batt_w=((2, 128, 2 * NOTE), F32),
        batt_b=((128, 2), F32),
        matt_w=((128, 2 * BEAT), F32),
        matt_b=((128, 1), F32),
        Cb=((2, 128, HEADS), F32),
        Cm=((128, HEADS), F32),
        Bfree_b=((HEADS, 2 * NOTE), F32),
        Bfree_m=((HEADS, 2 * BEAT), F32),
        Ppool=((8, 128, 32), F32),
        Ppoolm=((128, 2 * 32), F32),
        S_bs=((8, 128, 2 * 128), F16),   # half-chunked: [k][p,(half,c)]
        S_ms=((8, NM, 128), F16),
        S_bs_loc=((128, 2 * 128), F16),
        S_ms_loc=((NM, 128), F16),
        ident=((128, 128), F32),
        ident16=((128, 128), F16),
        bwhp=((2, 2, BEAT, 2 * BEAT), F16),   # [d][pair][H,(gA,gB)]
        bwip=((2, 2, 128, 4 * BEAT), F16),    # [d][kc][feat,(p0,p1)]
        bbp=((2, 1, 4 * BEAT), F16),
        mwhp=((2, 2, MEAS, 2 * MEAS), F16),
        mwip=((2, 1, 128, 4 * MEAS), F16),
        mbp=((2, 1, 4 * MEAS), F16),
        idmv64=((128, 64), F16),
        idmv32=((64, 32), F16),
        ident2_16=((128, 128), F16),
        J128_2=((128, 128), F16),
        J64_2=((64, 64), F16),
        J32=((32, 32), F32),
    )
    for g in ("g1", "g2"):
        for gate in ("z", "r", "h"):
            sp[f"{g}_w{gate}_dhi"] = ((128, E * SEC), F16)
            sp[f"{g}_w{gate}_dlo"] = ((128, E * SEC), F16)
            sp[f"{g}_w{gate}_st"] = ((128, E * 2 * SEC), F16)
            sp[f"{g}_u{gate}16"] = ((SEC, SEC), F16)
            sp[f"{g}_b{gate}16"] = ((1, SEC), F16)
    return sp


def _build_program():
    nc = bacc.Bacc("TRN2", target_bir_lowering=False, debug=False,
                   num_devices=NCORES)
    io = {}
    for name, (shape, dt) in _input_specs().items():
        io[name] = nc.dram_tensor(name, list(shape), dt,
                                  kind="ExternalInput").ap()
    out_dram = nc.dram_tensor("out", [1, N, S + SEC], F32,
                              kind="ExternalOutput").ap()
    dbg_dram = nc.dram_tensor("dbg", [16, 128, 512], F32,
                              kind="ExternalOutput").ap()
    _CACHE["dbg_dram"] = dbg_dram
    ag = {}
    for i in range(8):
        ag[f"sec_in{i}"] = nc.dram_tensor(f"sec_in{i}", [LOC, SEC], F16).ap()
        ag[f"sec_out{i}"] = nc.dram_tensor(f"sec_out{i}", [N, SEC], F16,
                                           addr_space="Shared").ap()
    with TileContext(nc) as tc:
        _emit(nc, tc, io, out_dram, ag, dbg_dram)
    nc.compile()
    return nc


def _emit(nc, tc, io, out_dram, ag, dbg_dram):
    import contextlib
    RG = [list(range(NCORES))]
    AF = mybir.ActivationFunctionType
    OP = mybir.AluOpType
    MM = nc.tensor.matmul

    stack = contextlib.ExitStack()
    const = stack.enter_context(tc.tile_pool(name="const", bufs=1))
    pers = stack.enter_context(tc.tile_pool(name="pers", bufs=1))
    acts = stack.enter_context(tc.tile_pool(name="acts", bufs=2))
    dynp = stack.enter_context(tc.tile_pool(name="dynp", bufs=2))
    lsp = stack.enter_context(tc.tile_pool(name="lsp", bufs=6))
    ps_t = stack.enter_context(tc.tile_pool(name="ps_t", bufs=1, space="PSUM"))
    ps_m = stack.enter_context(tc.tile_pool(name="ps_m", bufs=2, space="PSUM"))

    def dma(dst, src):
        nc.sync.dma_start(out=dst, in_=src)

    def dump(idx, src_ap, rows, cols):
        st = acts.tile([128, 512], F32, tag="dbgst", name="dbgst")
        nc.vector.tensor_copy(st[0:rows, 0:cols], src_ap)
        dma(dbg_dram[idx, 0:rows, 0:cols], st[0:rows, 0:cols])

    cst = {}

    def load(name, dt=None, src=None, tag=None):
        src = io[name] if src is None else src
        if dt is None:
            dt = src.dtype
        t = const.tile([src.shape[-2], src.shape[-1]], dt, tag=tag or name)
        dma(t[:, :], src)
        cst[tag or name] = t
        return t

    for nm in ("nodes_T16", "nodes_T_loc", "note_fc_w16", "note_fc_b16",
               "note_fc_w32", "note_fc_b32", "gb_w16", "gb_b16", "gb_w32",
               "gb_b32", "batt_b", "matt_w", "matt_b", "Cm", "Bfree_b",
               "Bfree_m", "Ppoolm", "S_bs_loc", "S_ms_loc", "ident",
               "ident16", "idmv64", "idmv32", "ident2_16", "J128_2",
               "J64_2", "J32"):
        load(nm)
    for kc in range(2):
        load("batt_w", src=io["batt_w"][kc], tag=f"battw{kc}")
        load("Cb", src=io["Cb"][kc], tag=f"Cb{kc}")
    for k in range(8):
        load("Ppool", src=io["Ppool"][k], tag=f"Ppool{k}")
        load("S_bs", src=io["S_bs"][k], tag=f"S_bs{k}")
        load("S_ms", src=io["S_ms"][k], tag=f"S_ms{k}")
    for e in range(E):
        load("adj_sl", src=io["adj_sl"][e], tag=f"adj{e}")
        load("adjB", src=io["adjB"][e], tag=f"adjB{e}")
        load("adjM", src=io["adjM"][e], tag=f"adjM{e}")
    for g in ("g1", "g2"):
        for gate in ("z", "r", "h"):
            load(f"{g}_w{gate}_dhi")
            load(f"{g}_w{gate}_dlo")
            load(f"{g}_w{gate}_st")
            load(f"{g}_u{gate}16")
            load(f"{g}_b{gate}16")
    for d in range(2):
        for p in range(2):
            load("bwhp", src=io["bwhp"][d, p], tag=f"bwhp{d}{p}")
            load("mwhp", src=io["mwhp"][d, p], tag=f"mwhp{d}{p}")
        for kc in range(2):
            load("bwip", src=io["bwip"][d, kc], tag=f"bwip{d}{kc}")
        load("mwip", src=io["mwip"][d, 0], tag=f"mwip{d}0")
        load("bbp", src=io["bbp"][d], tag=f"bbp{d}")
        load("mbp", src=io["mbp"][d], tag=f"mbp{d}")

    ones1 = const.tile([1, 512], F32, tag="ones1", name="ones1")
    nc.gpsimd.memset(ones1[:, :], 1.0)
    ones16 = const.tile([1, 512], F16, tag="ones16", name="ones16")
    nc.gpsimd.memset(ones16[:, :], 1.0)
    zsmall = const.tile([128, 2], F32, tag="zsmall", name="zsmall")
    nc.gpsimd.memset(zsmall[:, :], 0.0)
    z16 = const.tile([128, 2], F16, tag="z16", name="z16")
    nc.gpsimd.memset(z16[:, :], 0.0)
    ident = cst["ident"]
    ident16 = cst["ident16"]

    xb = [pers.tile([128, S], F16, tag=f"xb{k}", name=f"xb{k}")
          for k in range(8)]
    xl = pers.tile([128, S], F32, tag="xl", name="xl")
    nh16 = [pers.tile([128, SEC], F16, tag=f"nh16_{k}", name=f"nh16_{k}")
            for k in range(8)]
    nh216 = [pers.tile([128, SEC], F16, tag=f"nh216_{k}", name=f"nh216_{k}")
             for k in range(8)]
    bnT = [pers.tile([128, NB], F16, tag=f"bnT{h}", name=f"bnT{h}")
           for h in range(2)]
    bnTr = [pers.tile([128, NB], F16, tag=f"bnTr{h}", name=f"bnTr{h}")
            for h in range(2)]
    Hfb = pers.tile([BEAT, NB], F16, tag="Hfb", name="Hfb")
    Hbb = pers.tile([BEAT, NB], F16, tag="Hbb", name="Hbb")
    Hfm = pers.tile([MEAS, NM], F16, tag="Hfm", name="Hfm")
    Hbm = pers.tile([MEAS, NM], F16, tag="Hbm", name="Hbm")
    bh0 = pers.tile([128, 128], F32, tag="bh0", name="bh0")
    bh1 = pers.tile([128, 128], F32, tag="bh1", name="bh1")
    bh16 = [pers.tile([128, 128], F16, tag=f"bh16_{h}", name=f"bh16_{h}")
            for h in range(2)]
    bhT = pers.tile([128, NB], F32, tag="bhT", name="bhT")
    mh = pers.tile([NM, 2 * MEAS], F32, tag="mh", name="mh")
    mh16 = pers.tile([NM, 2 * MEAS], F16, tag="mh16", name="mh16")
    mnT = pers.tile([2 * BEAT, NM], F16, tag="mnT", name="mnT")
    mnTr = pers.tile([2 * BEAT, NM], F16, tag="mnTr", name="mnTr")
    mstat = pers.tile([128, 3 * SEC], F32, tag="mstat", name="mstat")
    nsl = pers.tile([128, SEC], F32, tag="nsl", name="nsl")

    def transpose_to(dst_ap, src_ap, rows):
        cols = src_ap.shape[-1]
        pt = ps_t.tile([128, 128], F32, tag="pt", name="pt")
        nc.tensor.transpose(pt[0:cols, 0:rows], src_ap,
                            ident[0:rows, 0:rows])
        nc.vector.tensor_copy(dst_ap, pt[0:cols, 0:rows])

    def transpose_new(src_ap, rows, tag="tr"):
        cols = src_ap.shape[-1]
        sb = acts.tile([cols, rows], F32, tag=tag, name=tag)
        transpose_to(sb[0:cols, 0:rows], src_ap, rows)
        return sb

    def transpose16(pool, src_ap, rows, tag):
        # fp16 src -> fp16 transposed SBUF tile
        cols = src_ap.shape[-1]
        pt = pool.tile([128, 128], F16, tag="pt16", name="pt16")
        nc.tensor.transpose(pt[0:cols, 0:rows], src_ap,
                            ident16[0:rows, 0:rows])
        sb = dynp.tile([128, 128], F16, tag=tag, name=tag)
        nc.vector.tensor_copy(sb[0:cols, 0:rows], pt[0:cols, 0:rows])
        return sb

    # ---------------- initial x ----------------
    for k in range(8):
        nc.gpsimd.memset(xb[k][:, 0:192], 0.0)
        pm = ps_m.tile([128, 512], F32, tag="pm", name="pm")
        MM(pm[:, 0:NOTE], cst["nodes_T16"][:, k * 128:(k + 1) * 128],
           cst["note_fc_w16"][:, :], start=True, stop=False)
        MM(pm[:, 0:NOTE], ones16[0:1, 0:128], cst["note_fc_b16"][:, :],
           start=False, stop=True)
        nc.scalar.activation(xb[k][:, 192:S], pm[:, 0:NOTE], AF.Tanh)
    nc.gpsimd.memset(xl[:, 0:192], 0.0)
    pm = ps_m.tile([128, 512], F32, tag="pm", name="pm")
    MM(pm[:, 0:NOTE], cst["nodes_T_loc"][:, :], cst["note_fc_w32"][:, :],
       start=True, stop=False)
    MM(pm[:, 0:NOTE], ones1[0:1, 0:LOC], cst["note_fc_b32"][:, :],
       start=False, stop=True)
    nc.scalar.activation(xl[:, 192:S], pm[:, 0:NOTE], AF.Tanh)
    dump(0, xb[0][:, 192:S], 128, 128)

    # ---------------- gated graph ----------------
    def gated_graph(g, static_mode, agins, agouts, nh_tiles, save_local):
        wdh = [cst[f"{g}_w{gt}_dhi"] for gt in "zrh"]
        wdl = [cst[f"{g}_w{gt}_dlo"] for gt in "zrh"]
        wst = [cst[f"{g}_w{gt}_st"] for gt in "zrh"]
        us = [cst[f"{g}_u{gt}16"] for gt in "zrh"]
        bs = [cst[f"{g}_b{gt}16"] for gt in "zrh"]
        with tc.tile_pool(name=f"ga{g}", bufs=1, space="PSUM") as gacc, \
                tc.tile_pool(name=f"gd{g}", bufs=2, space="PSUM") as gact:
            mt = [gacc.tile([128, SEC], F32, tag=f"m{gi}", name=f"m{gi}")
                  for gi in range(3)]
            # ---- static messages (constant across graph iters) ----
            if static_mode != "none":
                for e in range(E):
                    pa = gact.tile([128, 128], F32, tag="pact", name="pact")
                    if static_mode == "bh":
                        for ch in range(2):
                            MM(pa[:, :], bh16[ch][:, :],
                               cst[f"adjB{e}"][:, ch * 128:(ch + 1) * 128],
                               start=(ch == 0), stop=(ch == 1))
                    else:
                        for k in range(8):
                            MM(pa[:, :], xb[k][:, 0:128],
                               cst[f"adj{e}"][:, k * 128:(k + 1) * 128],
                               start=(k == 0), stop=(k == 7))
                    a0 = dynp.tile([128, 128], F16, tag="a0", name="a0")
                    nc.vector.tensor_copy(a0[:, :], pa[:, :])
                    pa1 = gact.tile([128, 128], F32, tag="pact", name="pact")
                    if static_mode == "bh":
                        MM(pa1[0:NM, :], mh16[:, :], cst[f"adjM{e}"][:, :],
                           start=True, stop=True)
                    else:
                        for k in range(8):
                            MM(pa1[0:64, :], xb[k][:, 128:192],
                               cst[f"adj{e}"][:, k * 128:(k + 1) * 128],
                               start=(k == 0), stop=(k == 7))
                    a1 = dynp.tile([128, 128], F16, tag="a1", name="a1")
                    nc.vector.tensor_copy(a1[0:64, :], pa1[0:64, :])
                    for gi in range(3):
                        MM(mt[gi][:, :], a0[:, :],
                           wst[gi][:, e * 256:e * 256 + 128],
                           start=(e == 0), stop=False)
                        MM(mt[gi][:, :], a1[0:64, :],
                           wst[gi][0:64, e * 256 + 128:e * 256 + 256],
                           start=False, stop=(e == E - 1))
                for gi in range(3):
                    nc.vector.tensor_copy(mstat[:, gi * SEC:(gi + 1) * SEC],
                                          mt[gi][:, :])
            # ---- graph iterations ----
            for it in range(GRAPH_ITER):
                last = it == GRAPH_ITER - 1
                for e in range(E):
                    pa = gact.tile([128, 128], F32, tag="pact", name="pact")
                    for k in range(8):
                        MM(pa[:, :], xb[k][:, 192:S],
                           cst[f"adj{e}"][:, k * 128:(k + 1) * 128],
                           start=(k == 0), stop=(k == 7))
                    ad = dynp.tile([128, 128], F16, tag="ad", name="ad")
                    nc.vector.tensor_copy(ad[:, :], pa[:, :])
                    for gi in range(3):
                        MM(mt[gi][:, :], ad[:, :],
                           wdh[gi][:, e * SEC:(e + 1) * SEC],
                           start=(e == 0), stop=False)
                        MM(mt[gi][:, :], ad[:, :],
                           wdl[gi][:, e * SEC:(e + 1) * SEC],
                           start=False, stop=False)
                xs = xl[:, 192:S]
                pt = ps_t.tile([128, 128], F32, tag="pt", name="pt")
                nc.tensor.transpose(pt[:, :], xs, ident[:, :])
                xsT = dynp.tile([128, 128], F16, tag="xsT", name="xsT")
                nc.vector.tensor_copy(xsT[:, :], pt[:, :])
                for gi in range(3):
                    MM(mt[gi][:, :], ones16[0:1, 0:128],
                       bs[gi][:, :], start=False, stop=False)
                for gi in range(2):
                    MM(mt[gi][:, :], xsT[:, :], us[gi][:, :],
                       start=False, stop=True)

                def gate_act(gi, func, dst):
                    reg = mt[gi][:, :]
                    if static_mode != "none":
                        tz = acts.tile([128, SEC], F32, tag="tz", name="tz")
                        nc.vector.tensor_tensor(
                            tz[:, :], reg, mstat[:, gi * SEC:(gi + 1) * SEC],
                            op=OP.add)
                        nc.scalar.activation(dst, tz[:, :], func)
                    else:
                        nc.scalar.activation(dst, reg, func)

                zt = acts.tile([128, SEC], F32, tag="zt", name="zt")
                rt = acts.tile([128, SEC], F32, tag="rt", name="rt")
                gate_act(0, AF.Sigmoid, zt[:, :])
                gate_act(1, AF.Sigmoid, rt[:, :])
                rx = acts.tile([128, SEC], F32, tag="rx", name="rx")
                nc.vector.tensor_tensor(rx[:, :], rt[:, :], xs, op=OP.mult)
                pt2 = ps_t.tile([128, 128], F32, tag="pt", name="pt")
                nc.tensor.transpose(pt2[:, :], rx[:, :], ident[:, :])
                rxT = dynp.tile([128, 128], F16, tag="rxT", name="rxT")
                nc.vector.tensor_copy(rxT[:, :], pt2[:, :])
                MM(mt[2][:, :], rxT[:, :], us[2][:, :],
                   start=False, stop=True)
                ht = acts.tile([128, SEC], F32, tag="ht", name="ht")
                gate_act(2, AF.Tanh, ht[:, :])
                t1 = acts.tile([128, SEC], F32, tag="t1", name="t1")
                nc.vector.tensor_tensor(t1[:, :], zt[:, :], xs, op=OP.mult)
                t2 = acts.tile([128, SEC], F32, tag="t2", name="t2")
                nc.vector.tensor_tensor(t2[:, :], rt[:, :], ht[:, :],
                                        op=OP.mult)
                ns = acts.tile([128, SEC], F32, tag="ns", name="ns")
                nc.vector.tensor_tensor(ns[:, :], xs, t1[:, :],
                                        op=OP.subtract)
                nc.vector.tensor_tensor(ns[:, :], ns[:, :], t2[:, :],
                                        op=OP.add)
                ns16 = acts.tile([128, SEC], F16, tag="ns16", name="ns16")
                nc.vector.tensor_copy(ns16[:, :], ns[:, :])
                if g == "g1" and it == 0 and dbgflag[0]:
                    dump(13, ns[:, :], 128, 128)
                a_in, a_out = agins[it], agouts[it]
                dma(a_in, ns16[:, :])
                nc.gpsimd.collective_compute(
                    "AllGather", OP.bypass, replica_groups=RG,
                    ins=[a_in], outs=[a_out])
                if not last:
                    for k in range(8):
                        dma(xb[k][:, 192:S], a_out[k * 128:(k + 1) * 128, :])
                else:
                    for k in range(8):
                        dma(nh_tiles[k][:, :],
                            a_out[k * 128:(k + 1) * 128, :])
                nc.vector.tensor_copy(xl[:, 192:S], ns[:, :])
                if last and save_local is not None:
                    nc.vector.tensor_copy(save_local[:, :], ns[:, :])

    # ---------------- nb (graph-between), replicated ----------------
    def nb_phase(s):
        fcs = [2] if s == 0 else [0, 1, 2]
        with tc.tile_pool(name="nbp", bufs=2, space="PSUM") as nbp:
          for k in range(8):
            pnb = ps_m.tile([128, 512], F32, tag="pm", name="pm")
            for fc in fcs:
                st, wd = FCS[fc]
                src = nh16[k][:, :] if fc == 2 else xb[k][:, st:st + wd]
                tT = transpose16(nbp, src, 128, tag="nbT")
                MM(pnb[:, 0:S], tT[0:wd, 0:128],
                   cst["gb_w16"][0:wd, fc * S:(fc + 1) * S],
                   start=(fc == fcs[0]), stop=False)
            MM(pnb[:, 0:S], ones16[0:1, 0:128], cst["gb_b16"][:, :],
               start=False, stop=True)
            nc.scalar.activation(xb[k][:, 0:S], pnb[:, 0:S], AF.Relu)
        # local f32 copy (xl holds nh locally: primary + nsl sec)
        pnl = ps_m.tile([128, 512], F32, tag="pm", name="pm")
        for fc in fcs:
            st, wd = FCS[fc]
            tT = transpose_new(xl[:, st:st + wd], 128, tag="nbT32")
            MM(pnl[:, 0:S], tT[0:wd, 0:128],
               cst["gb_w32"][0:wd, fc * S:(fc + 1) * S],
               start=(fc == fcs[0]), stop=False)
        MM(pnl[:, 0:S], ones1[0:1, 0:128], cst["gb_b32"][:, :],
           start=False, stop=True)
        nc.scalar.activation(xl[:, 0:S], pnl[:, 0:S], AF.Relu)

    # ---------------- beat attention ----------------
    def beat_attention():
        for k in range(8):
            cat_nm = acts.tile([128, 2 * NOTE], F32, tag="cat_nm",
                               name="cat_nm")
            nc.vector.tensor_copy(cat_nm[:, 0:NOTE], nh16[k][:, :])
            nc.vector.tensor_copy(cat_nm[:, NOTE:2 * NOTE], nh216[k][:, :])
            ct = [transpose_new(cat_nm[:, kc * 128:(kc + 1) * 128], 128,
                                tag=f"ct{kc}") for kc in range(2)]
            aT = []
            for mc in range(2):
                pa = ps_m.tile([128, 512], F32, tag="pm", name="pm")
                for kc in range(2):
                    MM(pa[:, 0:128],
                       cst[f"battw{kc}"][:, mc * 128:(mc + 1) * 128],
                       ct[kc][:, :], start=(kc == 0), stop=(kc == 1))
                sb = acts.tile([128, 128], F32, tag=f"aT{mc}", name=f"aT{mc}")
                nc.scalar.activation(sb[:, :], pa[:, 0:128], AF.Tanh,
                                     bias=cst["batt_b"][:, mc:mc + 1])
                aT.append(sb)
            psim = ps_t.tile([128, 128], F32, tag="pt", name="pt")
            for kc in range(2):
                MM(psim[0:HEADS, :], cst[f"Cb{kc}"][:, :], aT[kc][:, :],
                   start=(kc == 0), stop=(kc == 1))
            pp = acts.tile([HEADS, 128], F32, tag="pp", name="pp")
            nc.scalar.activation(pp[:, :], psim[0:HEADS, :], AF.Sigmoid)
            qq = acts.tile([HEADS, 128], F32, tag="qq", name="qq")
            nc.scalar.activation(qq[:, :], psim[0:HEADS, :], AF.Sigmoid,
                                 scale=-1.0)
            rq = acts.tile([HEADS, 128], F32, tag="rq", name="rq")
            nc.vector.reciprocal(rq[:, :], qq[:, :])
            wt = acts.tile([HEADS, 128], F32, tag="wt", name="wt")
            nc.vector.tensor_tensor(wt[:, :], pp[:, :], rq[:, :], op=OP.mult)
            pwe = ps_m.tile([128, 512], F32, tag="pm", name="pm")
            wexp = acts.tile([128, 2 * NOTE], F32, tag="wexp", name="wexp")
            MM(pwe[:, 0:256], wt[:, :], cst["Bfree_b"][:, :],
               start=True, stop=True)
            nc.vector.tensor_copy(wexp[:, :], pwe[:, 0:256])
            tt = acts.tile([128, 2 * NOTE], F32, tag="tt", name="tt")
            nc.vector.tensor_tensor(tt[:, :], cat_nm[:, :], wexp[:, :],
                                    op=OP.mult)
            pool = ps_m.tile([128, 512], F32, tag="pm", name="pm")
            MM(pool[0:32, 0:256], cst[f"Ppool{k}"][:, :], tt[:, :],
               start=True, stop=True)
            MM(pool[0:32, 256:512], cst[f"Ppool{k}"][:, :], wexp[:, :],
               start=True, stop=True)
            rd = acts.tile([32, 256], F32, tag="rd", name="rd")
            nc.vector.reciprocal(rd[:, :], pool[0:32, 256:512])
            bnk = acts.tile([32, 256], F32, tag="bnk", name="bnk")
            nc.vector.tensor_tensor(bnk[:, :], pool[0:32, 0:256], rd[:, :],
                                    op=OP.mult)
            for h in range(2):
                transpose_to(bnT[h][:, k * 32:(k + 1) * 32],
                             bnk[0:32, h * 128:(h + 1) * 128], 32)
                ptr = ps_t.tile([128, 128], F32, tag="pt", name="pt")
                nc.tensor.transpose(ptr[0:128, 0:32],
                                    bnk[0:32, h * 128:(h + 1) * 128],
                                    cst["J32"][:, :])
                nc.vector.tensor_copy(bnTr[h][:, (7 - k) * 32:(8 - k) * 32],
                                      ptr[0:128, 0:32])

    # ---------------- measure attention ----------------
    def measure_attention():
        paT = ps_m.tile([128, 512], F32, tag="pm", name="pm")
        MM(paT[:, 0:NB], cst["matt_w"][:, :], bhT[:, :],
           start=True, stop=True)
        amT = acts.tile([128, NB], F32, tag="amT", name="amT")
        nc.scalar.activation(amT[:, :], paT[:, 0:NB], AF.Tanh,
                             bias=cst["matt_b"][:, 0:1])
        psim = ps_t.tile([128, 128], F32, tag="pt", name="pt")
        pp = acts.tile([HEADS, NB], F32, tag="ppm", name="ppm")
        qq = acts.tile([HEADS, NB], F32, tag="qqm", name="qqm")
        for hc in range(2):
            MM(psim[0:HEADS, 0:128], cst["Cm"][:, :],
               amT[:, hc * 128:(hc + 1) * 128], start=True, stop=True)
            nc.scalar.activation(pp[:, hc * 128:(hc + 1) * 128],
                                 psim[0:HEADS, 0:128], AF.Sigmoid)
            nc.scalar.activation(qq[:, hc * 128:(hc + 1) * 128],
                                 psim[0:HEADS, 0:128], AF.Sigmoid,
                                 scale=-1.0)
        rq = acts.tile([HEADS, NB], F32, tag="rqm", name="rqm")
        nc.vector.reciprocal(rq[:, :], qq[:, :])
        wt = acts.tile([HEADS, NB], F32, tag="wtm", name="wtm")
        nc.vector.tensor_tensor(wt[:, :], pp[:, :], rq[:, :], op=OP.mult)
        for h in range(2):
            bh_h = bh0 if h == 0 else bh1
            pwe = ps_m.tile([128, 512], F32, tag="pm", name="pm")
            MM(pwe[:, 0:2 * BEAT], wt[:, h * 128:(h + 1) * 128],
               cst["Bfree_m"][:, :], start=True, stop=True)
            wexp = acts.tile([128, 2 * BEAT], F32, tag="wexpm", name="wexpm")
            nc.vector.tensor_copy(wexp[:, :], pwe[:, 0:2 * BEAT])
            tt = acts.tile([128, 2 * BEAT], F32, tag="ttm", name="ttm")
            nc.vector.tensor_tensor(tt[:, :], bh_h[:, :], wexp[:, :],
                                    op=OP.mult)
            pool = ps_m.tile([128, 512], F32, tag="pm", name="pm")
            MM(pool[0:32, 0:128], cst["Ppoolm"][:, h * 32:(h + 1) * 32],
               tt[:, :], start=True, stop=True)
            MM(pool[0:32, 128:256], cst["Ppoolm"][:, h * 32:(h + 1) * 32],
               wexp[:, :], start=True, stop=True)
            rd = acts.tile([32, 128], F32, tag="rdm", name="rdm")
            nc.vector.reciprocal(rd[:, :], pool[0:32, 128:256])
            mnk = acts.tile([32, 128], F32, tag="mnk", name="mnk")
            nc.vector.tensor_tensor(mnk[:, :], pool[0:32, 0:128], rd[:, :],
                                    op=OP.mult)
            transpose_to(mnT[:, h * 32:(h + 1) * 32], mnk[0:32, :], 32)
            ptr = ps_t.tile([128, 128], F32, tag="pt", name="pt")
            nc.tensor.transpose(ptr[0:128, 0:32], mnk[0:32, :],
                                cst["J32"][:, :])
            nc.vector.tensor_copy(mnTr[:, (1 - h) * 32:(2 - h) * 32],
                                  ptr[0:128, 0:32])

    # ---------------- LSTM ----------------
    def run_lstm2(H, T, inT, inTr, whp, wip, bp, nkc, Hf_t, Hb_t):
        """Decoupled fw/bw scan. Per dir PSUM U [2H, 2T]:
        pair0 cols 0:T = (f @ rows 0:H ; i @ rows H:2H),
        pair1 cols T:2T = (o ; 2*g~). All gates sigmoid (tanh folded as
        2*sigma(2x)-1 with weights/bias pre-scaled). c and h live at rows
        0:H; the i*g~ product is moved from rows H:2H via a PE
        identity-matmul."""
        Hs = [Hf_t, Hb_t]
        with tc.tile_pool(name=f"lu{H}", bufs=1, space="PSUM") as up, \
                tc.tile_pool(name=f"lm{H}", bufs=2, space="PSUM") as mp:
            U = [up.tile([2 * H, 2 * T], F32, tag=f"U{d}", name=f"U{H}{d}")
                 for d in range(2)]
            for d in range(2):
                srcs = inT if d == 0 else inTr
                for p in range(2):
                    reg = U[d][:, p * T:(p + 1) * T]
                    for kc in range(nkc):
                        MM(reg,
                           cst[f"{wip}{d}{kc}"][:, p * 2 * H:(p + 1) * 2 * H],
                           srcs[kc], start=(kc == 0 and p == 0), stop=False)
                    MM(reg, cst[f"{bp}{d}"][0:1, p * 2 * H:(p + 1) * 2 * H],
                       ones16[0:1, 0:T], start=False, stop=False)
            idm = cst["idmv64" if H == 64 else "idmv32"]
            cs = [None, None]
            for t in range(T):
                for d in range(2):
                    rhs = z16[0:H, 0:1] if t == 0 else Hs[d][0:H, t - 1:t]
                    MM(U[d][:, t:t + 1], cst[f"{whp}{d}0"][:, :], rhs,
                       start=False, stop=True)
                    MM(U[d][:, T + t:T + t + 1], cst[f"{whp}{d}1"][:, :],
                       rhs, start=False, stop=True)
                sio = []
                for d in range(2):
                    sv = lsp.tile([2 * H, 2], F32, tag=f"sio{H}{d}",
                                  name=f"sio{H}{d}")
                    Uv = U[d].rearrange("p (pr t) -> p pr t", pr=2, t=T)
                    nc.scalar.activation(
                        sv[0:2 * H, :].rearrange("p (a b) -> p a b",
                                                 a=2, b=1),
                        Uv[:, :, t:t + 1], AF.Sigmoid)
                    sio.append(sv)
                if H == 64 and t == 0 and dbgflag[0]:
                    dump(10, sio[0][:, :], 128, 2)
                    dump(11, U[0][:, 0:4], 128, 4)
                    dump(12, U[0][:, T:T + 4], 128, 4)
                # w = i*(sig(2g)-0.5) = i*g~/2 at rows H:2H, both dirs in
                # one fp16 tile so a single PE ident-matmul moves both to
                # rows 0:H.
                w16 = lsp.tile([2 * H, 2], F16, tag=f"w{H}", name=f"w{H}")
                for d in range(2):
                    nc.vector.scalar_tensor_tensor(
                        w16[H:2 * H, d:d + 1], sio[d][H:2 * H, 1:2], 0.5,
                        sio[d][H:2 * H, 0:1], op0=OP.subtract, op1=OP.mult)
                vs = []
                for d in range(2):
                    v = lsp.tile([2 * H, 2], F32, tag=f"v{H}{d}",
                                 name=f"v{H}{d}")
                    cprev = zsmall[0:H, 0:1] if t == 0 else cs[d][0:H, 0:1]
                    nc.gpsimd.tensor_tensor(v[0:H, 0:1], sio[d][0:H, 0:1],
                                            cprev, op=OP.mult)
                    vs.append(v)
                mv = mp.tile([128, 8], F32, tag=f"mv{H}", name=f"mv{H}")
                MM(mv[0:H, 0:2], idm[H:2 * H, 0:H], w16[H:2 * H, 0:2],
                   start=True, stop=True)
                cns = []
                for d in range(2):
                    cn = lsp.tile([2 * H, 2], F32, tag=f"cn{H}{d}",
                                  name=f"cn{H}{d}")
                    nc.vector.scalar_tensor_tensor(
                        cn[0:H, 0:1], mv[0:H, d:d + 1], 2.0,
                        vs[d][0:H, 0:1], op0=OP.mult, op1=OP.add)
                    cns.append(cn)
                cs = cns
                tcs = []
                for d in range(2):
                    tcx = lsp.tile([2 * H, 2], F32, tag=f"tc{H}{d}",
                                   name=f"tc{H}{d}")
                    nc.scalar.activation(tcx[0:H, 0:1], cns[d][0:H, 0:1],
                                         AF.Sigmoid, scale=2.0)
                    tcs.append(tcx)
                for d in range(2):
                    # store y = h/2; the *2 is folded into Wh and the
                    # output transposes
                    nc.vector.scalar_tensor_tensor(
                        Hs[d][0:H, t:t + 1], tcs[d][0:H, 0:1], 0.5,
                        sio[d][0:H, 1:2], op0=OP.subtract, op1=OP.mult)

    def build_beat():
        with tc.tile_pool(name="bhp", bufs=2, space="PSUM") as bp2:
            for half in range(2):
                bh_h = bh0 if half == 0 else bh1
                ptA = bp2.tile([128, 128], F16, tag="ptA", name="ptA")
                nc.tensor.transpose(ptA[0:128, 0:BEAT],
                                    Hfb[:, half * 128:(half + 1) * 128],
                                    ident16[0:BEAT, 0:BEAT])
                nc.scalar.activation(bh_h[:, 0:BEAT], ptA[0:128, 0:BEAT],
                                     AF.Copy, scale=2.0)
                ptB = bp2.tile([128, 128], F16, tag="ptA", name="ptA")
                nc.tensor.transpose(
                    ptB[0:128, 0:BEAT],
                    Hbb[:, (1 - half) * 128:(2 - half) * 128],
                    ident16[0:BEAT, 0:BEAT])
                C1 = dynp.tile([128, 128], F16, tag="C1", name="C1")
                nc.vector.tensor_copy(C1[0:128, 0:BEAT],
                                      ptB[0:128, 0:BEAT])
                pf = bp2.tile([128, 128], F32, tag="ptF", name="ptF")
                MM(pf[0:128, 0:BEAT], cst["J128_2"][:, :],
                   C1[0:128, 0:BEAT], start=True, stop=True)
                nc.vector.tensor_copy(bh_h[:, BEAT:2 * BEAT],
                                      pf[0:128, 0:BEAT])
                nc.vector.tensor_copy(bh16[half][:, :], bh_h[:, :])
                transpose_to(bhT[:, half * 128:(half + 1) * 128],
                             bh_h[:, :], 128)

    def build_meas():
        with tc.tile_pool(name="mhp", bufs=2, space="PSUM") as mp2:
            ptA = mp2.tile([64, 64], F16, tag="ptA", name="ptA")
            nc.tensor.transpose(ptA[0:NM, 0:MEAS], Hfm[:, :],
                                ident16[0:MEAS, 0:MEAS])
            nc.scalar.activation(mh[:, 0:MEAS], ptA[0:NM, 0:MEAS],
                                 AF.Copy, scale=2.0)
            ptB = mp2.tile([64, 64], F16, tag="ptA", name="ptA")
            nc.tensor.transpose(ptB[0:NM, 0:MEAS], Hbm[:, :],
                                ident16[0:MEAS, 0:MEAS])
            C1 = dynp.tile([128, 128], F16, tag="C1", name="C1")
            nc.vector.tensor_copy(C1[0:NM, 0:MEAS], ptB[0:NM, 0:MEAS])
            pf = mp2.tile([64, 64], F32, tag="ptF", name="ptF")
            MM(pf[0:NM, 0:MEAS], cst["J64_2"][:, :], C1[0:NM, 0:MEAS],
               start=True, stop=True)
            nc.vector.tensor_copy(mh[:, MEAS:2 * MEAS], pf[0:NM, 0:MEAS])
            nc.vector.tensor_copy(mh16[:, :], mh[:, :])

    # ---------------- main sequence ----------------
    dbgflag = [True]
    for s in range(SEQ_ITER):
        dbgflag[0] = s == 0
        with nc.named_scope(f"g1_{s}"):
            gated_graph("g1", "none" if s == 0 else "bh",
                        [ag[f"sec_in{s * 4 + i}"] for i in range(2)],
                        [ag[f"sec_out{s * 4 + i}"] for i in range(2)],
                        nh16, nsl)
        dump(1 if s == 0 else 8, nh16[0][:, :], 128, 128)
        if s == 1:
            dump(9, mstat[:, :], 128, 384)
        with nc.named_scope(f"nb_{s}"):
            nb_phase(s)
        if s == 0:
            dump(2, xb[0][:, 0:S], 128, S)
        with nc.named_scope(f"g2_{s}"):
            gated_graph("g2", "generic",
                        [ag[f"sec_in{s * 4 + 2 + i}"] for i in range(2)],
                        [ag[f"sec_out{s * 4 + 2 + i}"] for i in range(2)],
                        nh216, None)
        if s == 0:
            dump(3, nh216[0][:, :], 128, 128)
        with nc.named_scope(f"batt_{s}"):
            beat_attention()
        if s == 0:
            dump(4, bnT[0][:, :], 128, 256)
        with nc.named_scope(f"blstm_{s}"):
            run_lstm2(BEAT, NB, [bnT[0][:, :], bnT[1][:, :]],
                      [bnTr[0][:, :], bnTr[1][:, :]], "bwhp", "bwip",
                      "bbp", 2, Hfb, Hbb)
            build_beat()
            if s == 0:
                dump(5, bh0[:, :], 128, 128)
        with nc.named_scope(f"matt_{s}"):
            measure_attention()
        with nc.named_scope(f"mlstm_{s}"):
            run_lstm2(MEAS, NM, [mnT[:, :]], [mnTr[:, :]], "mwhp",
                      "mwip", "mbp", 1, Hfm, Hbm)
            build_meas()
            if s == 0:
                dump(6, mh[:, :], NM, 2 * MEAS)
        # rebuild x tiles for next iteration / final output
        with nc.named_scope(f"rebuild_{s}"):
            for k in range(8):
                pbs = ps_m.tile([128, 512], F32, tag="pm", name="pm")
                for half in range(2):
                    MM(pbs[:, 0:128],
                       cst[f"S_bs{k}"][:, half * 128:(half + 1) * 128],
                       bh16[half][:, :], start=(half == 0), stop=(half == 1))
                MM(pbs[:, 128:192], cst[f"S_ms{k}"][:, :], mh16[:, :],
                   start=True, stop=True)
                if s + 1 < SEQ_ITER:
                    nc.vector.tensor_copy(xb[k][:, 0:192], pbs[:, 0:192])
                    nc.vector.tensor_copy(xb[k][:, 192:S], nh16[k][:, :])
                else:
                    outst = acts.tile([128, S + SEC], F32, tag="outst",
                                      name="outst")
                    nc.vector.tensor_copy(outst[:, 0:192], pbs[:, 0:192])
                    nc.vector.tensor_copy(outst[:, 192:S], nh16[k][:, :])
                    nc.vector.tensor_copy(outst[:, S:S + SEC], nh216[k][:, :])
                    dma(out_dram[0, k * 128:(k + 1) * 128, :], outst[:, :])
            if s + 1 < SEQ_ITER:
                pbs = ps_m.tile([128, 512], F32, tag="pm", name="pm")
                for half in range(2):
                    MM(pbs[:, 0:128],
                       cst["S_bs_loc"][:, half * 128:(half + 1) * 128],
                       bh16[half][:, :], start=(half == 0), stop=(half == 1))
                MM(pbs[:, 128:192], cst["S_ms_loc"][:, :], mh16[:, :],
                   start=True, stop=True)
                nc.vector.tensor_copy(xl[:, 0:192], pbs[:, 0:192])
                nc.vector.tensor_copy(xl[:, 192:S], nsl[:, :])
                dump(7, xb[0][:, 0:S], 128, S)
    stack.close()


# ================= host side =================

def _host_inputs(inputs):
    f32, f16 = np.float32, np.float16
    nodes = np.asarray(inputs["nodes"], f32)[0]
    adjacency = np.asarray(inputs["adjacency"], f32)
    beat = np.asarray(inputs["beat_numbers"], np.int64)
    meas = np.asarray(inputs["measure_numbers"], np.int64)
    rep = {}
    rep["nodes_T16"] = nodes.T.astype(f16)
    fw = np.asarray(inputs["note_fc_w"], f32)
    fb = np.asarray(inputs["note_fc_b"], f32)[None, :]
    rep["note_fc_w16"] = fw.astype(f16)
    rep["note_fc_b16"] = fb.astype(f16)
    rep["note_fc_w32"] = fw
    rep["note_fc_b32"] = fb
    gbw = np.zeros((128, 3, S), f32)
    gw = np.asarray(inputs["gb_w"], f32)
    for fc, (st, w) in enumerate(FCS):
        gbw[0:w, fc, :] = gw[st:st + w, :]
    rep["gb_w32"] = gbw.reshape(128, 3 * S)
    rep["gb_b32"] = np.asarray(inputs["gb_b"], f32)[None, :]
    rep["gb_w16"] = rep["gb_w32"].astype(f16)
    rep["gb_b16"] = rep["gb_b32"].astype(f16)
    rep["batt_w"] = np.asarray(inputs["batt_w"], f32).reshape(2, 128,
                                                             2 * NOTE)
    rep["batt_b"] = np.asarray(inputs["batt_b"],
                               f32).reshape(2, 128).T.copy()
    rep["matt_w"] = np.asarray(inputs["matt_w"], f32)
    rep["matt_b"] = np.asarray(inputs["matt_b"], f32)[:, None]
    bc = np.asarray(inputs["batt_c"], f32)
    Cb = np.zeros((2 * NOTE, HEADS), f32)
    for h in range(HEADS):
        Cb[h * 32:(h + 1) * 32, h] = bc[h]
    rep["Cb"] = Cb.reshape(2, 128, HEADS)
    mcc = np.asarray(inputs["matt_c"], f32)
    Cm = np.zeros((2 * BEAT, HEADS), f32)
    for h in range(HEADS):
        Cm[h * 16:(h + 1) * 16, h] = mcc[h]
    rep["Cm"] = Cm
    Bf = np.zeros((HEADS, 2 * NOTE), f32)
    for h in range(HEADS):
        Bf[h, h * 32:(h + 1) * 32] = 1.0
    rep["Bfree_b"] = Bf
    Bm = np.zeros((HEADS, 2 * BEAT), f32)
    for h in range(HEADS):
        Bm[h, h * 16:(h + 1) * 16] = 1.0
    rep["Bfree_m"] = Bm
    Ppool = np.zeros((8, 128, 32), f32)
    for k in range(8):
        for p in range(128):
            b = beat[k * 128 + p] - 32 * k
            assert 0 <= b < 32, "beats not aligned to 128-node chunks"
            Ppool[k, p, b] = 1.0
    rep["Ppool"] = Ppool
    b2m = np.full(NB, 1 << 40, np.int64)
    np.minimum.at(b2m, beat, meas)
    Ppoolm = np.zeros((2, 128, 32), f32)
    for half in range(2):
        for p in range(128):
            m_ = b2m[half * 128 + p] - 32 * half
            assert 0 <= m_ < 32
            Ppoolm[half, p, m_] = 1.0
    rep["Ppoolm"] = np.concatenate([Ppoolm[0], Ppoolm[1]], axis=1)
    S_bs = np.zeros((8, NB, 128), f32)
    S_ms = np.zeros((8, NM, 128), f32)
    for k in range(8):
        for p in range(128):
            S_bs[k, beat[k * 128 + p], p] = 1.0
            S_ms[k, meas[k * 128 + p], p] = 1.0
    S_bs_hc = np.zeros((8, 128, 2, 128), f32)
    for k in range(8):
        S_bs_hc[k, :, 0, :] = S_bs[k, 0:128, :]
        S_bs_hc[k, :, 1, :] = S_bs[k, 128:256, :]
    rep["S_bs"] = S_bs_hc.reshape(8, 128, 256).astype(f16)
    rep["S_ms"] = S_ms.astype(f16)
    rep["ident"] = np.eye(128, dtype=f32)
    rep["ident16"] = np.eye(128, dtype=f32).astype(f16)

    def lstm_pack2(wi_f, wh_f, b_f, wi_b, wh_b, b_b, H):
        # pair0 = (f; i), pair1 = (o; 2*g); reference gate order i,f,g,o
        KIN = wi_f.shape[1]
        nkc = max(1, KIN // 128)
        whp = np.zeros((2, 2, H, 2 * H), f32)
        wip = np.zeros((2, nkc, 128, 4 * H), f32)
        bp = np.zeros((2, 1, 4 * H), f32)
        for d, (wi, wh, b) in enumerate(((wi_f, wh_f, b_f),
                                         (wi_b, wh_b, b_b))):
            blk = [wh[i * H:(i + 1) * H] for i in range(4)]  # i,f,g,o
            whp[d, 0, :, 0:H] = blk[1].T
            whp[d, 0, :, H:2 * H] = blk[0].T
            whp[d, 1, :, 0:H] = blk[3].T
            whp[d, 1, :, H:2 * H] = 2.0 * blk[2].T
            wt = wi.T  # (KIN, 4H) col blocks i,f,g,o
            for kc in range(nkc):
                w = wt[kc * 128:(kc + 1) * 128]
                r = w.shape[0]
                wip[d, kc, 0:r, 0:H] = w[:, H:2 * H]
                wip[d, kc, 0:r, H:2 * H] = w[:, 0:H]
                wip[d, kc, 0:r, 2 * H:3 * H] = w[:, 3 * H:4 * H]
                wip[d, kc, 0:r, 3 * H:4 * H] = 2.0 * w[:, 2 * H:3 * H]
            bp[d, 0, 0:H] = b[H:2 * H]
            bp[d, 0, H:2 * H] = b[0:H]
            bp[d, 0, 2 * H:3 * H] = b[3 * H:4 * H]
            bp[d, 0, 3 * H:4 * H] = 2.0 * b[2 * H:3 * H]
        return whp.astype(f16), wip.astype(f16), bp.astype(f16)

    g = lambda n: np.asarray(inputs[n], f32)
    rep["bwhp"], rep["bwip"], rep["bbp"] = lstm_pack2(
        g("blstm_wi_f"), g("blstm_wh_f"), g("blstm_b_f"),
        g("blstm_wi_b"), g("blstm_wh_b"), g("blstm_b_b"), BEAT)
    rep["mwhp"], rep["mwip"], rep["mbp"] = lstm_pack2(
        g("mlstm_wi_f"), g("mlstm_wh_f"), g("mlstm_b_f"),
        g("mlstm_wi_b"), g("mlstm_wh_b"), g("mlstm_b_b"), MEAS)
    idmv64 = np.zeros((128, 64), f32)
    idmv64[64:128] = np.eye(64)
    rep["idmv64"] = idmv64.astype(f16)
    idmv32 = np.zeros((64, 32), f32)
    idmv32[32:64] = np.eye(32)
    rep["idmv32"] = idmv32.astype(f16)
    rep["bwhp"] = (rep["bwhp"].astype(f32) * 2.0).astype(f16)
    rep["mwhp"] = (rep["mwhp"].astype(f32) * 2.0).astype(f16)
    rep["ident2_16"] = (np.eye(128, dtype=f32) * 2.0).astype(f16)
    rep["J128_2"] = (np.eye(128, dtype=f32)[::-1] * 2.0).astype(f16)
    rep["J64_2"] = (np.eye(64, dtype=f32)[::-1] * 2.0).astype(f16)
    rep["J32"] = np.eye(32, dtype=f32)[::-1].copy()
    for gg in ("g1", "g2"):
        for gate in ("z", "r", "h"):
            w = np.asarray(inputs[f"{gg}_w{gate}"], f32)  # (E, S, SEC)
            dyn = w[:, 192:320, :]                        # (E, 128, SEC)
            dhi = dyn.astype(f16)
            dlo = (dyn - dhi.astype(f32)).astype(f16)
            rep[f"{gg}_w{gate}_dhi"] = \
                dhi.transpose(1, 0, 2).reshape(128, E * SEC).copy()
            rep[f"{gg}_w{gate}_dlo"] = \
                dlo.transpose(1, 0, 2).reshape(128, E * SEC).copy()
            ws = np.zeros((128, E, 2 * SEC), f16)
            ws[0:128, :, 0:SEC] = w[:, 0:128, :].transpose(1, 0, 2)
            ws[0:64, :, SEC:2 * SEC] = w[:, 128:192, :].transpose(1, 0, 2)
            rep[f"{gg}_w{gate}_st"] = ws.reshape(128, E * 2 * SEC)
            rep[f"{gg}_u{gate}16"] = \
                np.asarray(inputs[f"{gg}_u{gate}"], f32).astype(f16)
            rep[f"{gg}_b{gate}16"] = \
                np.asarray(inputs[f"{gg}_b{gate}"], f32)[None, :].astype(f16)

    # beat/measure aggregated adjacency (static message terms for g1)
    if np.array_equal(beat, np.arange(N) // 4):
        adjB_full = adjacency.reshape(E, NB, 4, N).sum(2)
    else:
        Sb = np.zeros((NB, N), f32)
        Sb[beat, np.arange(N)] = 1.0
        adjB_full = np.einsum("bm,emn->ebn", Sb, adjacency)
    if np.array_equal(b2m, np.arange(NB) // 4):
        adjM_full = adjB_full.reshape(E, NM, 4, N).sum(2)
    else:
        Sm = np.zeros((NM, NB), f32)
        Sm[b2m, np.arange(NB)] = 1.0
        adjM_full = np.einsum("mb,ebn->emn", Sm, adjB_full)

    specs = _input_specs()
    # convert replicated entries once
    for kk in list(rep.keys()):
        shape, dt = specs[kk]
        npdt = np.float16 if dt == F16 else np.float32
        rep[kk] = np.ascontiguousarray(
            np.asarray(rep[kk]).reshape(shape).astype(npdt))
    in_maps = []
    for c in range(NCORES):
        sl = slice(c * LOC, (c + 1) * LOC)
        m = dict(rep)
        m["nodes_T_loc"] = np.ascontiguousarray(nodes[sl].T)
        adjc = adjacency[:, :, sl]
        m["adj_sl"] = np.ascontiguousarray(
            adjc.reshape(E, 8, 128, LOC).transpose(0, 2, 1, 3)
            .reshape(E, 128, N).astype(f16))
        m["adjB"] = np.ascontiguousarray(
            adjB_full[:, :, sl].reshape(E, 2, 128, LOC)
            .transpose(0, 2, 1, 3).reshape(E, 128, 2 * LOC).astype(f16))
        m["adjM"] = np.ascontiguousarray(adjM_full[:, :, sl].astype(f16))
        m["S_bs_loc"] = rep["S_bs"][c]
        m["S_ms_loc"] = rep["S_ms"][c]
        in_maps.append(m)
    return in_maps


def kernel(**inputs):
    if "nc" not in _CACHE:
        _CACHE["nc"] = _build_program()
    nc = _CACHE["nc"]
    in_maps = _host_inputs(inputs)
    res = bass_utils.run_bass_kernel_spmd(nc, in_maps,
                                          core_ids=list(range(NCORES)))
    _CACHE["last_res"] = res
    out = res.results[0]["out"]
    return np.asarray(out, np.float32)


# revision 30
# speedup vs baseline: 1.1299x; 1.0198x over previous
"""Trainium2 Bass kernel for nn_IsgnBeatMeasEncoder (gnn_message_passing).

Sharding: destination-node sharding for the gated-graph message passing
(128 dest-nodes/core; per-core adjacency slice resident in SBUF, fp16);
fp16 AllGather of the updated secondary state per graph iteration; nb
(graph-between) computed replicated from the gathered state (no second
collective); static message terms for g1 computed via beat/measure
aggregated adjacency; attention pooling via host-built one-hot matmuls;
BiLSTMs replicated with fw/bw batched and gate inputs pinned in PSUM.
"""
import numpy as np

import concourse.bass as bass
import concourse.mybir as mybir
from concourse import bacc
from concourse.tile import TileContext
from concourse import bass_utils

F32 = mybir.dt.float32
F16 = mybir.dt.float16

N = 1024
E = 10
IN = 78
NOTE = 128
BEAT = 64
MEAS = 32
S = 320
SEC = 128
HEADS = 8
NB = 256
NM = 64
SEQ_ITER = 2
GRAPH_ITER = 2
NCORES = 8
LOC = N // NCORES

FCS = [(0, 128), (128, 64), (192, 128)]  # (start, width); 0,1 static; 2 dyn

_CACHE = {}


def _input_specs():
    sp = dict(
        nodes_T16=((IN, N), F16),
        nodes_T_loc=((IN, LOC), F32),
        note_fc_w16=((IN, NOTE), F16),
        note_fc_b16=((1, NOTE), F16),
        note_fc_w32=((IN, NOTE), F32),
        note_fc_b32=((1, NOTE), F32),
        adj_sl=((E, 128, N), F16),
        adjB=((E, 128, 2 * LOC), F16),
        adjM=((E, NM, LOC), F16),
        gb_w16=((128, 3 * S), F16),
        gb_b16=((1, S), F16),
        gb_w32=((128, 3 * S), F32),
        gb_b32=((1, S), F32),
        batt_w=((2, 128, 2 * NOTE), F32),
        batt_b=((128, 2), F32),
        matt_w=((128, 2 * BEAT), F32),
        matt_b=((128, 1), F32),
        Cb=((2, 128, HEADS), F32),
        Cm=((128, HEADS), F32),
        Bfree_b=((HEADS, 2 * NOTE), F32),
        Bfree_m=((HEADS, 2 * BEAT), F32),
        Ppool=((8, 128, 32), F32),
        Ppoolm=((128, 2 * 32), F32),
        S_bs=((8, 128, 2 * 128), F16),   # half-chunked: [k][p,(half,c)]
        S_ms=((8, NM, 128), F16),
        S_bs_loc=((128, 2 * 128), F16),
        S_ms_loc=((NM, 128), F16),
        ident=((128, 128), F32),
        ident16=((128, 128), F16),
        bwhp=((2, 2, BEAT, 2 * BEAT), F16),   # [d][pair][H,(gA,gB)]
        bwip=((2, 2, 128, 4 * BEAT), F16),    # [d][kc][feat,(p0,p1)]
        bbp=((2, 1, 4 * BEAT), F16),
        mwhp=((2, 2, MEAS, 2 * MEAS), F16),
        mwip=((2, 1, 128, 4 * MEAS), F16),
        mbp=((2, 1, 4 * MEAS), F16),
        idmv64=((128, 64), F16),
        idmv32=((64, 32), F16),
        ident2_16=((128, 128), F16),
        J128_2=((128, 128), F16),
        J64_2=((64, 64), F16),
        J32=((32, 32), F32),
    )
    for g in ("g1", "g2"):
        for gate in ("z", "r", "h"):
            sp[f"{g}_w{gate}_dhi"] = ((128, E * SEC), F16)
            sp[f"{g}_w{gate}_dlo"] = ((128, E * SEC), F16)
            sp[f"{g}_w{gate}_st"] = ((128, E * 2 * SEC), F16)
            sp[f"{g}_u{gate}16"] = ((SEC, SEC), F16)
            sp[f"{g}_b{gate}16"] = ((1, SEC), F16)
    return sp


def _build_program():
    nc = bacc.Bacc("TRN2", target_bir_lowering=False, debug=False,
                   num_devices=NCORES)
    io = {}
    for name, (shape, dt) in _input_specs().items():
        io[name] = nc.dram_tensor(name, list(shape), dt,
                                  kind="ExternalInput").ap()
    out_dram = nc.dram_tensor("out", [1, N, S + SEC], F32,
                              kind="ExternalOutput").ap()
    dbg_dram = nc.dram_tensor("dbg", [16, 128, 512], F32,
                              kind="ExternalOutput").ap()
    _CACHE["dbg_dram"] = dbg_dram
    ag = {}
    for i in range(8):
        ag[f"sec_in{i}"] = nc.dram_tensor(f"sec_in{i}", [LOC, SEC], F16).ap()
        ag[f"sec_out{i}"] = nc.dram_tensor(f"sec_out{i}", [N, SEC], F16,
                                           addr_space="Shared").ap()
    with TileContext(nc) as tc:
        _emit(nc, tc, io, out_dram, ag, dbg_dram)
    nc.compile()
    return nc


def _emit(nc, tc, io, out_dram, ag, dbg_dram):
    import contextlib
    RG = [list(range(NCORES))]
    AF = mybir.ActivationFunctionType
    OP = mybir.AluOpType
    MM = nc.tensor.matmul

    stack = contextlib.ExitStack()
    const = stack.enter_context(tc.tile_pool(name="const", bufs=1))
    pers = stack.enter_context(tc.tile_pool(name="pers", bufs=1))
    acts = stack.enter_context(tc.tile_pool(name="acts", bufs=2))
    dynp = stack.enter_context(tc.tile_pool(name="dynp", bufs=2))
    lsp = stack.enter_context(tc.tile_pool(name="lsp", bufs=6))
    ps_t = stack.enter_context(tc.tile_pool(name="ps_t", bufs=1, space="PSUM"))
    ps_m = stack.enter_context(tc.tile_pool(name="ps_m", bufs=2, space="PSUM"))

    def dma(dst, src):
        nc.sync.dma_start(out=dst, in_=src)

    def dump(idx, src_ap, rows, cols):
        st = acts.tile([128, 512], F32, tag="dbgst", name="dbgst")
        nc.vector.tensor_copy(st[0:rows, 0:cols], src_ap)
        dma(dbg_dram[idx, 0:rows, 0:cols], st[0:rows, 0:cols])

    cst = {}

    def load(name, dt=None, src=None, tag=None):
        src = io[name] if src is None else src
        if dt is None:
            dt = src.dtype
        t = const.tile([src.shape[-2], src.shape[-1]], dt, tag=tag or name)
        dma(t[:, :], src)
        cst[tag or name] = t
        return t

    for nm in ("nodes_T16", "nodes_T_loc", "note_fc_w16", "note_fc_b16",
               "note_fc_w32", "note_fc_b32", "gb_w16", "gb_b16", "gb_w32",
               "gb_b32", "batt_b", "matt_w", "matt_b", "Cm", "Bfree_b",
               "Bfree_m", "Ppoolm", "S_bs_loc", "S_ms_loc", "ident",
               "ident16", "idmv64", "idmv32", "ident2_16", "J128_2",
               "J64_2", "J32"):
        load(nm)
    for kc in range(2):
        load("batt_w", src=io["batt_w"][kc], tag=f"battw{kc}")
        load("Cb", src=io["Cb"][kc], tag=f"Cb{kc}")
    for k in range(8):
        load("Ppool", src=io["Ppool"][k], tag=f"Ppool{k}")
        load("S_bs", src=io["S_bs"][k], tag=f"S_bs{k}")
        load("S_ms", src=io["S_ms"][k], tag=f"S_ms{k}")
    for e in range(E):
        load("adj_sl", src=io["adj_sl"][e], tag=f"adj{e}")
        load("adjB", src=io["adjB"][e], tag=f"adjB{e}")
        load("adjM", src=io["adjM"][e], tag=f"adjM{e}")
    for g in ("g1", "g2"):
        for gate in ("z", "r", "h"):
            load(f"{g}_w{gate}_dhi")
            load(f"{g}_w{gate}_dlo")
            load(f"{g}_w{gate}_st")
            load(f"{g}_u{gate}16")
            load(f"{g}_b{gate}16")
    for d in range(2):
        for p in range(2):
            load("bwhp", src=io["bwhp"][d, p], tag=f"bwhp{d}{p}")
            load("mwhp", src=io["mwhp"][d, p], tag=f"mwhp{d}{p}")
        for kc in range(2):
            load("bwip", src=io["bwip"][d, kc], tag=f"bwip{d}{kc}")
        load("mwip", src=io["mwip"][d, 0], tag=f"mwip{d}0")
        load("bbp", src=io["bbp"][d], tag=f"bbp{d}")
        load("mbp", src=io["mbp"][d], tag=f"mbp{d}")

    ones1 = const.tile([1, 512], F32, tag="ones1", name="ones1")
    nc.gpsimd.memset(ones1[:, :], 1.0)
    ones16 = const.tile([1, 512], F16, tag="ones16", name="ones16")
    nc.gpsimd.memset(ones16[:, :], 1.0)
    zsmall = const.tile([128, 2], F32, tag="zsmall", name="zsmall")
    nc.gpsimd.memset(zsmall[:, :], 0.0)
    z16 = const.tile([128, 2], F16, tag="z16", name="z16")
    nc.gpsimd.memset(z16[:, :], 0.0)
    ident = cst["ident"]
    ident16 = cst["ident16"]

    xb = [pers.tile([128, S], F16, tag=f"xb{k}", name=f"xb{k}")
          for k in range(8)]
    xl = pers.tile([128, S], F32, tag="xl", name="xl")
    nh16 = [pers.tile([128, SEC], F16, tag=f"nh16_{k}", name=f"nh16_{k}")
            for k in range(8)]
    nh216 = [pers.tile([128, SEC], F16, tag=f"nh216_{k}", name=f"nh216_{k}")
             for k in range(8)]
    bnT = [pers.tile([128, NB], F16, tag=f"bnT{h}", name=f"bnT{h}")
           for h in range(2)]
    bnTr = [pers.tile([128, NB], F16, tag=f"bnTr{h}", name=f"bnTr{h}")
            for h in range(2)]
    Hfb = pers.tile([BEAT, NB], F16, tag="Hfb", name="Hfb")
    Hbb = pers.tile([BEAT, NB], F16, tag="Hbb", name="Hbb")
    Hfm = pers.tile([MEAS, NM], F16, tag="Hfm", name="Hfm")
    Hbm = pers.tile([MEAS, NM], F16, tag="Hbm", name="Hbm")
    bh0 = pers.tile([128, 128], F32, tag="bh0", name="bh0")
    bh1 = pers.tile([128, 128], F32, tag="bh1", name="bh1")
    bh16 = [pers.tile([128, 128], F16, tag=f"bh16_{h}", name=f"bh16_{h}")
            for h in range(2)]
    bhT = pers.tile([128, NB], F32, tag="bhT", name="bhT")
    mh = pers.tile([NM, 2 * MEAS], F32, tag="mh", name="mh")
    mh16 = pers.tile([NM, 2 * MEAS], F16, tag="mh16", name="mh16")
    mnT = pers.tile([2 * BEAT, NM], F16, tag="mnT", name="mnT")
    mnTr = pers.tile([2 * BEAT, NM], F16, tag="mnTr", name="mnTr")
    mstat = pers.tile([128, 3 * SEC], F32, tag="mstat", name="mstat")
    nsl = pers.tile([128, SEC], F32, tag="nsl", name="nsl")

    def transpose_to(dst_ap, src_ap, rows):
        cols = src_ap.shape[-1]
        pt = ps_t.tile([128, 128], F32, tag="pt", name="pt")
        nc.tensor.transpose(pt[0:cols, 0:rows], src_ap,
                            ident[0:rows, 0:rows])
        nc.vector.tensor_copy(dst_ap, pt[0:cols, 0:rows])

    def transpose_new(src_ap, rows, tag="tr"):
        cols = src_ap.shape[-1]
        sb = acts.tile([cols, rows], F32, tag=tag, name=tag)
        transpose_to(sb[0:cols, 0:rows], src_ap, rows)
        return sb

    def transpose16(pool, src_ap, rows, tag):
        # fp16 src -> fp16 transposed SBUF tile
        cols = src_ap.shape[-1]
        pt = pool.tile([128, 128], F16, tag="pt16", name="pt16")
        nc.tensor.transpose(pt[0:cols, 0:rows], src_ap,
                            ident16[0:rows, 0:rows])
        sb = dynp.tile([128, 128], F16, tag=tag, name=tag)
        nc.vector.tensor_copy(sb[0:cols, 0:rows], pt[0:cols, 0:rows])
        return sb

    # ---------------- initial x ----------------
    for k in range(8):
        nc.gpsimd.memset(xb[k][:, 0:192], 0.0)
        pm = ps_m.tile([128, 512], F32, tag="pm", name="pm")
        MM(pm[:, 0:NOTE], cst["nodes_T16"][:, k * 128:(k + 1) * 128],
           cst["note_fc_w16"][:, :], start=True, stop=False)
        MM(pm[:, 0:NOTE], ones16[0:1, 0:128], cst["note_fc_b16"][:, :],
           start=False, stop=True)
        nc.scalar.activation(xb[k][:, 192:S], pm[:, 0:NOTE], AF.Tanh)
    nc.gpsimd.memset(xl[:, 0:192], 0.0)
    pm = ps_m.tile([128, 512], F32, tag="pm", name="pm")
    MM(pm[:, 0:NOTE], cst["nodes_T_loc"][:, :], cst["note_fc_w32"][:, :],
       start=True, stop=False)
    MM(pm[:, 0:NOTE], ones1[0:1, 0:LOC], cst["note_fc_b32"][:, :],
       start=False, stop=True)
    nc.scalar.activation(xl[:, 192:S], pm[:, 0:NOTE], AF.Tanh)
    dump(0, xb[0][:, 192:S], 128, 128)

    # ---------------- gated graph ----------------
    def gated_graph(g, static_mode, agins, agouts, nh_tiles, save_local):
        wdh = [cst[f"{g}_w{gt}_dhi"] for gt in "zrh"]
        wdl = [cst[f"{g}_w{gt}_dlo"] for gt in "zrh"]
        wst = [cst[f"{g}_w{gt}_st"] for gt in "zrh"]
        us = [cst[f"{g}_u{gt}16"] for gt in "zrh"]
        bs = [cst[f"{g}_b{gt}16"] for gt in "zrh"]
        with tc.tile_pool(name=f"ga{g}", bufs=1, space="PSUM") as gacc, \
                tc.tile_pool(name=f"gd{g}", bufs=2, space="PSUM") as gact:
            mt = [gacc.tile([128, SEC], F32, tag=f"m{gi}", name=f"m{gi}")
                  for gi in range(3)]
            # ---- static messages (constant across graph iters) ----
            if static_mode != "none":
                for e in range(E):
                    pa = gact.tile([128, 128], F32, tag="pact", name="pact")
                    if static_mode == "bh":
                        for ch in range(2):
                            MM(pa[:, :], bh16[ch][:, :],
                               cst[f"adjB{e}"][:, ch * 128:(ch + 1) * 128],
                               start=(ch == 0), stop=(ch == 1))
                    else:
                        for k in range(8):
                            MM(pa[:, :], xb[k][:, 0:128],
                               cst[f"adj{e}"][:, k * 128:(k + 1) * 128],
                               start=(k == 0), stop=(k == 7))
                    a0 = dynp.tile([128, 128], F16, tag="a0", name="a0")
                    nc.vector.tensor_copy(a0[:, :], pa[:, :])
                    pa1 = gact.tile([128, 128], F32, tag="pact", name="pact")
                    if static_mode == "bh":
                        MM(pa1[0:NM, :], mh16[:, :], cst[f"adjM{e}"][:, :],
                           start=True, stop=True)
                    else:
                        for k in range(8):
                            MM(pa1[0:64, :], xb[k][:, 128:192],
                               cst[f"adj{e}"][:, k * 128:(k + 1) * 128],
                               start=(k == 0), stop=(k == 7))
                    a1 = dynp.tile([128, 128], F16, tag="a1", name="a1")
                    nc.vector.tensor_copy(a1[0:64, :], pa1[0:64, :])
                    for gi in range(3):
                        MM(mt[gi][:, :], a0[:, :],
                           wst[gi][:, e * 256:e * 256 + 128],
                           start=(e == 0), stop=False)
                        MM(mt[gi][:, :], a1[0:64, :],
                           wst[gi][0:64, e * 256 + 128:e * 256 + 256],
                           start=False, stop=(e == E - 1))
                for gi in range(3):
                    nc.vector.tensor_copy(mstat[:, gi * SEC:(gi + 1) * SEC],
                                          mt[gi][:, :])
            # ---- graph iterations ----
            for it in range(GRAPH_ITER):
                last = it == GRAPH_ITER - 1
                for e in range(E):
                    pa = gact.tile([128, 128], F32, tag="pact", name="pact")
                    for k in range(8):
                        MM(pa[:, :], xb[k][:, 192:S],
                           cst[f"adj{e}"][:, k * 128:(k + 1) * 128],
                           start=(k == 0), stop=(k == 7))
                    ad = dynp.tile([128, 128], F16, tag="ad", name="ad")
                    nc.vector.tensor_copy(ad[:, :], pa[:, :])
                    for gi in range(3):
                        MM(mt[gi][:, :], ad[:, :],
                           wdh[gi][:, e * SEC:(e + 1) * SEC],
                           start=(e == 0), stop=False)
                        MM(mt[gi][:, :], ad[:, :],
                           wdl[gi][:, e * SEC:(e + 1) * SEC],
                           start=False, stop=False)
                xs = xl[:, 192:S]
                pt = ps_t.tile([128, 128], F32, tag="pt", name="pt")
                nc.tensor.transpose(pt[:, :], xs, ident[:, :])
                xsT = dynp.tile([128, 128], F16, tag="xsT", name="xsT")
                nc.vector.tensor_copy(xsT[:, :], pt[:, :])
                for gi in range(3):
                    MM(mt[gi][:, :], ones16[0:1, 0:128],
                       bs[gi][:, :], start=False, stop=False)
                for gi in range(2):
                    MM(mt[gi][:, :], xsT[:, :], us[gi][:, :],
                       start=False, stop=True)

                def gate_act(gi, func, dst):
                    reg = mt[gi][:, :]
                    if static_mode != "none":
                        tz = acts.tile([128, SEC], F32, tag="tz", name="tz")
                        nc.vector.tensor_tensor(
                            tz[:, :], reg, mstat[:, gi * SEC:(gi + 1) * SEC],
                            op=OP.add)
                        nc.scalar.activation(dst, tz[:, :], func)
                    else:
                        nc.scalar.activation(dst, reg, func)

                zt = acts.tile([128, SEC], F32, tag="zt", name="zt")
                rt = acts.tile([128, SEC], F32, tag="rt", name="rt")
                gate_act(0, AF.Sigmoid, zt[:, :])
                gate_act(1, AF.Sigmoid, rt[:, :])
                rx = acts.tile([128, SEC], F32, tag="rx", name="rx")
                nc.vector.tensor_tensor(rx[:, :], rt[:, :], xs, op=OP.mult)
                pt2 = ps_t.tile([128, 128], F32, tag="pt", name="pt")
                nc.tensor.transpose(pt2[:, :], rx[:, :], ident[:, :])
                rxT = dynp.tile([128, 128], F16, tag="rxT", name="rxT")
                nc.vector.tensor_copy(rxT[:, :], pt2[:, :])
                MM(mt[2][:, :], rxT[:, :], us[2][:, :],
                   start=False, stop=True)
                ht = acts.tile([128, SEC], F32, tag="ht", name="ht")
                gate_act(2, AF.Tanh, ht[:, :])
                t1 = acts.tile([128, SEC], F32, tag="t1", name="t1")
                nc.vector.tensor_tensor(t1[:, :], zt[:, :], xs, op=OP.mult)
                t2 = acts.tile([128, SEC], F32, tag="t2", name="t2")
                nc.vector.tensor_tensor(t2[:, :], rt[:, :], ht[:, :],
                                        op=OP.mult)
                ns = acts.tile([128, SEC], F32, tag="ns", name="ns")
                nc.vector.tensor_tensor(ns[:, :], xs, t1[:, :],
                                        op=OP.subtract)
                nc.vector.tensor_tensor(ns[:, :], ns[:, :], t2[:, :],
                                        op=OP.add)
                ns16 = acts.tile([128, SEC], F16, tag="ns16", name="ns16")
                nc.vector.tensor_copy(ns16[:, :], ns[:, :])
                if g == "g1" and it == 0 and dbgflag[0]:
                    dump(13, ns[:, :], 128, 128)
                a_in, a_out = agins[it], agouts[it]
                dma(a_in, ns16[:, :])
                nc.gpsimd.collective_compute(
                    "AllGather", OP.bypass, replica_groups=RG,
                    ins=[a_in], outs=[a_out])
                if not last:
                    for k in range(8):
                        dma(xb[k][:, 192:S], a_out[k * 128:(k + 1) * 128, :])
                else:
                    for k in range(8):
                        dma(nh_tiles[k][:, :],
                            a_out[k * 128:(k + 1) * 128, :])
                nc.vector.tensor_copy(xl[:, 192:S], ns[:, :])
                if last and save_local is not None:
                    nc.vector.tensor_copy(save_local[:, :], ns[:, :])

    # ---------------- nb (graph-between), replicated ----------------
    def nb_phase(s):
        fcs = [2] if s == 0 else [0, 1, 2]
        with tc.tile_pool(name="nbp", bufs=2, space="PSUM") as nbp:
          for k in range(8):
            pnb = ps_m.tile([128, 512], F32, tag="pm", name="pm")
            for fc in fcs:
                st, wd = FCS[fc]
                src = nh16[k][:, :] if fc == 2 else xb[k][:, st:st + wd]
                tT = transpose16(nbp, src, 128, tag="nbT")
                MM(pnb[:, 0:S], tT[0:wd, 0:128],
                   cst["gb_w16"][0:wd, fc * S:(fc + 1) * S],
                   start=(fc == fcs[0]), stop=False)
            MM(pnb[:, 0:S], ones16[0:1, 0:128], cst["gb_b16"][:, :],
               start=False, stop=True)
            nc.scalar.activation(xb[k][:, 0:S], pnb[:, 0:S], AF.Relu)
        # local f32 copy (xl holds nh locally: primary + nsl sec)
        pnl = ps_m.tile([128, 512], F32, tag="pm", name="pm")
        for fc in fcs:
            st, wd = FCS[fc]
            tT = transpose_new(xl[:, st:st + wd], 128, tag="nbT32")
            MM(pnl[:, 0:S], tT[0:wd, 0:128],
               cst["gb_w32"][0:wd, fc * S:(fc + 1) * S],
               start=(fc == fcs[0]), stop=False)
        MM(pnl[:, 0:S], ones1[0:1, 0:128], cst["gb_b32"][:, :],
           start=False, stop=True)
        nc.scalar.activation(xl[:, 0:S], pnl[:, 0:S], AF.Relu)

    # ---------------- beat attention ----------------
    def beat_attention():
        for k in range(8):
            cat_nm = acts.tile([128, 2 * NOTE], F32, tag="cat_nm",
                               name="cat_nm")
            nc.vector.tensor_copy(cat_nm[:, 0:NOTE], nh16[k][:, :])
            nc.vector.tensor_copy(cat_nm[:, NOTE:2 * NOTE], nh216[k][:, :])
            ct = [transpose_new(cat_nm[:, kc * 128:(kc + 1) * 128], 128,
                                tag=f"ct{kc}") for kc in range(2)]
            aT = []
            for mc in range(2):
                pa = ps_m.tile([128, 512], F32, tag="pm", name="pm")
                for kc in range(2):
                    MM(pa[:, 0:128],
                       cst[f"battw{kc}"][:, mc * 128:(mc + 1) * 128],
                       ct[kc][:, :], start=(kc == 0), stop=(kc == 1))
                sb = acts.tile([128, 128], F32, tag=f"aT{mc}", name=f"aT{mc}")
                nc.scalar.activation(sb[:, :], pa[:, 0:128], AF.Tanh,
                                     bias=cst["batt_b"][:, mc:mc + 1])
                aT.append(sb)
            psim = ps_t.tile([128, 128], F32, tag="pt", name="pt")
            for kc in range(2):
                MM(psim[0:HEADS, :], cst[f"Cb{kc}"][:, :], aT[kc][:, :],
                   start=(kc == 0), stop=(kc == 1))
            pp = acts.tile([HEADS, 128], F32, tag="pp", name="pp")
            nc.scalar.activation(pp[:, :], psim[0:HEADS, :], AF.Sigmoid)
            qq = acts.tile([HEADS, 128], F32, tag="qq", name="qq")
            nc.scalar.activation(qq[:, :], psim[0:HEADS, :], AF.Sigmoid,
                                 scale=-1.0)
            rq = acts.tile([HEADS, 128], F32, tag="rq", name="rq")
            nc.vector.reciprocal(rq[:, :], qq[:, :])
            wt = acts.tile([HEADS, 128], F32, tag="wt", name="wt")
            nc.vector.tensor_tensor(wt[:, :], pp[:, :], rq[:, :], op=OP.mult)
            pwe = ps_m.tile([128, 512], F32, tag="pm", name="pm")
            wexp = acts.tile([128, 2 * NOTE], F32, tag="wexp", name="wexp")
            MM(pwe[:, 0:256], wt[:, :], cst["Bfree_b"][:, :],
               start=True, stop=True)
            nc.vector.tensor_copy(wexp[:, :], pwe[:, 0:256])
            tt = acts.tile([128, 2 * NOTE], F32, tag="tt", name="tt")
            nc.vector.tensor_tensor(tt[:, :], cat_nm[:, :], wexp[:, :],
                                    op=OP.mult)
            pool = ps_m.tile([128, 512], F32, tag="pm", name="pm")
            MM(pool[0:32, 0:256], cst[f"Ppool{k}"][:, :], tt[:, :],
               start=True, stop=True)
            MM(pool[0:32, 256:512], cst[f"Ppool{k}"][:, :], wexp[:, :],
               start=True, stop=True)
            rd = acts.tile([32, 256], F32, tag="rd", name="rd")
            nc.vector.reciprocal(rd[:, :], pool[0:32, 256:512])
            bnk = acts.tile([32, 256], F32, tag="bnk", name="bnk")
            nc.vector.tensor_tensor(bnk[:, :], pool[0:32, 0:256], rd[:, :],
                                    op=OP.mult)
            for h in range(2):
                transpose_to(bnT[h][:, k * 32:(k + 1) * 32],
                             bnk[0:32, h * 128:(h + 1) * 128], 32)
                ptr = ps_t.tile([128, 128], F32, tag="pt", name="pt")
                nc.tensor.transpose(ptr[0:128, 0:32],
                                    bnk[0:32, h * 128:(h + 1) * 128],
                                    cst["J32"][:, :])
                nc.vector.tensor_copy(bnTr[h][:, (7 - k) * 32:(8 - k) * 32],
                                      ptr[0:128, 0:32])

    # ---------------- measure attention ----------------
    def measure_attention():
        paT = ps_m.tile([128, 512], F32, tag="pm", name="pm")
        MM(paT[:, 0:NB], cst["matt_w"][:, :], bhT[:, :],
           start=True, stop=True)
        amT = acts.tile([128, NB], F32, tag="amT", name="amT")
        nc.scalar.activation(amT[:, :], paT[:, 0:NB], AF.Tanh,
                             bias=cst["matt_b"][:, 0:1])
        psim = ps_t.tile([128, 128], F32, tag="pt", name="pt")
        pp = acts.tile([HEADS, NB], F32, tag="ppm", name="ppm")
        qq = acts.tile([HEADS, NB], F32, tag="qqm", name="qqm")
        for hc in range(2):
            MM(psim[0:HEADS, 0:128], cst["Cm"][:, :],
               amT[:, hc * 128:(hc + 1) * 128], start=True, stop=True)
            nc.scalar.activation(pp[:, hc * 128:(hc + 1) * 128],
                                 psim[0:HEADS, 0:128], AF.Sigmoid)
            nc.scalar.activation(qq[:, hc * 128:(hc + 1) * 128],
                                 psim[0:HEADS, 0:128], AF.Sigmoid,
                                 scale=-1.0)
        rq = acts.tile([HEADS, NB], F32, tag="rqm", name="rqm")
        nc.vector.reciprocal(rq[:, :], qq[:, :])
        wt = acts.tile([HEADS, NB], F32, tag="wtm", name="wtm")
        nc.vector.tensor_tensor(wt[:, :], pp[:, :], rq[:, :], op=OP.mult)
        for h in range(2):
            bh_h = bh0 if h == 0 else bh1
            pwe = ps_m.tile([128, 512], F32, tag="pm", name="pm")
            MM(pwe[:, 0:2 * BEAT], wt[:, h * 128:(h + 1) * 128],
               cst["Bfree_m"][:, :], start=True, stop=True)
            wexp = acts.tile([128, 2 * BEAT], F32, tag="wexpm", name="wexpm")
            nc.vector.tensor_copy(wexp[:, :], pwe[:, 0:2 * BEAT])
            tt = acts.tile([128, 2 * BEAT], F32, tag="ttm", name="ttm")
            nc.vector.tensor_tensor(tt[:, :], bh_h[:, :], wexp[:, :],
                                    op=OP.mult)
            pool = ps_m.tile([128, 512], F32, tag="pm", name="pm")
            MM(pool[0:32, 0:128], cst["Ppoolm"][:, h * 32:(h + 1) * 32],
               tt[:, :], start=True, stop=True)
            MM(pool[0:32, 128:256], cst["Ppoolm"][:, h * 32:(h + 1) * 32],
               wexp[:, :], start=True, stop=True)
            rd = acts.tile([32, 128], F32, tag="rdm", name="rdm")
            nc.vector.reciprocal(rd[:, :], pool[0:32, 128:256])
            mnk = acts.tile([32, 128], F32, tag="mnk", name="mnk")
            nc.vector.tensor_tensor(mnk[:, :], pool[0:32, 0:128], rd[:, :],
                                    op=OP.mult)
            transpose_to(mnT[:, h * 32:(h + 1) * 32], mnk[0:32, :], 32)
            ptr = ps_t.tile([128, 128], F32, tag="pt", name="pt")
            nc.tensor.transpose(ptr[0:128, 0:32], mnk[0:32, :],
                                cst["J32"][:, :])
            nc.vector.tensor_copy(mnTr[:, (1 - h) * 32:(2 - h) * 32],
                                  ptr[0:128, 0:32])

    # ---------------- LSTM ----------------
    def run_lstm2(H, T, inT, inTr, whp, wip, bp, nkc, Hf_t, Hb_t):
        """Decoupled fw/bw scan. Per dir PSUM U [2H, 2T]:
        pair0 cols 0:T = (f @ rows 0:H ; i @ rows H:2H),
        pair1 cols T:2T = (o ; 2*g~). All gates sigmoid (tanh folded as
        2*sigma(2x)-1 with weights/bias pre-scaled). c and h live at rows
        0:H; the i*g~ product is moved from rows H:2H via a PE
        identity-matmul."""
        Hs = [Hf_t, Hb_t]
        with tc.tile_pool(name=f"lu{H}", bufs=1, space="PSUM") as up, \
                tc.tile_pool(name=f"lm{H}", bufs=2, space="PSUM") as mp:
            U = [up.tile([2 * H, 2 * T], F32, tag=f"U{d}", name=f"U{H}{d}")
                 for d in range(2)]
            for d in range(2):
                srcs = inT if d == 0 else inTr
                for p in range(2):
                    reg = U[d][:, p * T:(p + 1) * T]
                    for kc in range(nkc):
                        MM(reg,
                           cst[f"{wip}{d}{kc}"][:, p * 2 * H:(p + 1) * 2 * H],
                           srcs[kc], start=(kc == 0 and p == 0), stop=False)
                    MM(reg, cst[f"{bp}{d}"][0:1, p * 2 * H:(p + 1) * 2 * H],
                       ones16[0:1, 0:T], start=False, stop=False)
            idm = cst["idmv64" if H == 64 else "idmv32"]
            cs = [None, None]
            for t in range(T):
                for d in range(2):
                    rhs = z16[0:H, 0:1] if t == 0 else Hs[d][0:H, t - 1:t]
                    MM(U[d][:, t:t + 1], cst[f"{whp}{d}0"][:, :], rhs,
                       start=False, stop=True)
                    MM(U[d][:, T + t:T + t + 1], cst[f"{whp}{d}1"][:, :],
                       rhs, start=False, stop=True)
                sio = []
                for d in range(2):
                    sv = lsp.tile([2 * H, 2], F32, tag=f"sio{H}{d}",
                                  name=f"sio{H}{d}")
                    Uv = U[d].rearrange("p (pr t) -> p pr t", pr=2, t=T)
                    nc.scalar.activation(
                        sv[0:2 * H, :].rearrange("p (a b) -> p a b",
                                                 a=2, b=1),
                        Uv[:, :, t:t + 1], AF.Sigmoid)
                    sio.append(sv)
                if H == 64 and t == 0 and dbgflag[0]:
                    dump(10, sio[0][:, :], 128, 2)
                    dump(11, U[0][:, 0:4], 128, 4)
                    dump(12, U[0][:, T:T + 4], 128, 4)
                # w = i*(sig(2g)-0.5) = i*g~/2 at rows H:2H, both dirs in
                # one fp16 tile so a single PE ident-matmul moves both to
                # rows 0:H.
                w16 = lsp.tile([2 * H, 2], F16, tag=f"w{H}", name=f"w{H}")
                for d in range(2):
                    nc.vector.scalar_tensor_tensor(
                        w16[H:2 * H, d:d + 1], sio[d][H:2 * H, 1:2], 0.5,
                        sio[d][H:2 * H, 0:1], op0=OP.subtract, op1=OP.mult)
                mv = mp.tile([128, 8], F32, tag=f"mv{H}", name=f"mv{H}")
                MM(mv[0:H, 0:2], idm[H:2 * H, 0:H], w16[H:2 * H, 0:2],
                   start=True, stop=True)
                cns = []
                for d in range(2):
                    cprev = zsmall[0:H, 0:1] if t == 0 else cs[d][0:H, 0:1]
                    cn = lsp.tile([2 * H, 2], F32, tag=f"cn{H}{d}",
                                  name=f"cn{H}{d}")
                    nc.vector.scalar_tensor_tensor(
                        cn[0:H, 0:1], cprev, sio[d][0:H, 0:1],
                        mv[0:H, d:d + 1], op0=OP.mult, op1=OP.add)
                    cns.append(cn)
                cs = cns
                tcs = []
                for d in range(2):
                    tcx = lsp.tile([2 * H, 2], F32, tag=f"tc{H}{d}",
                                   name=f"tc{H}{d}")
                    nc.scalar.activation(tcx[0:H, 0:1], cns[d][0:H, 0:1],
                                         AF.Sigmoid, scale=2.0)
                    tcs.append(tcx)
                for d in range(2):
                    # store y = h/2; the *2 is folded into Wh and the
                    # output transposes
                    nc.vector.scalar_tensor_tensor(
                        Hs[d][0:H, t:t + 1], tcs[d][0:H, 0:1], 0.5,
                        sio[d][0:H, 1:2], op0=OP.subtract, op1=OP.mult)

    def build_beat():
        with tc.tile_pool(name="bhp", bufs=2, space="PSUM") as bp2:
            for half in range(2):
                bh_h = bh0 if half == 0 else bh1
                ptA = bp2.tile([128, 128], F16, tag="ptA", name="ptA")
                nc.tensor.transpose(ptA[0:128, 0:BEAT],
                                    Hfb[:, half * 128:(half + 1) * 128],
                                    ident16[0:BEAT, 0:BEAT])
                nc.scalar.activation(bh_h[:, 0:BEAT], ptA[0:128, 0:BEAT],
                                     AF.Copy, scale=2.0)
                ptB = bp2.tile([128, 128], F16, tag="ptA", name="ptA")
                nc.tensor.transpose(
                    ptB[0:128, 0:BEAT],
                    Hbb[:, (1 - half) * 128:(2 - half) * 128],
                    ident16[0:BEAT, 0:BEAT])
                C1 = dynp.tile([128, 128], F16, tag="C1", name="C1")
                nc.vector.tensor_copy(C1[0:128, 0:BEAT],
                                      ptB[0:128, 0:BEAT])
                pf = bp2.tile([128, 128], F32, tag="ptF", name="ptF")
                MM(pf[0:128, 0:BEAT], cst["J128_2"][:, :],
                   C1[0:128, 0:BEAT], start=True, stop=True)
                nc.vector.tensor_copy(bh_h[:, BEAT:2 * BEAT],
                                      pf[0:128, 0:BEAT])
                nc.vector.tensor_copy(bh16[half][:, :], bh_h[:, :])
                transpose_to(bhT[:, half * 128:(half + 1) * 128],
                             bh_h[:, :], 128)

    def build_meas():
        with tc.tile_pool(name="mhp", bufs=2, space="PSUM") as mp2:
            ptA = mp2.tile([64, 64], F16, tag="ptA", name="ptA")
            nc.tensor.transpose(ptA[0:NM, 0:MEAS], Hfm[:, :],
                                ident16[0:MEAS, 0:MEAS])
            nc.scalar.activation(mh[:, 0:MEAS], ptA[0:NM, 0:MEAS],
                                 AF.Copy, scale=2.0)
            ptB = mp2.tile([64, 64], F16, tag="ptA", name="ptA")
            nc.tensor.transpose(ptB[0:NM, 0:MEAS], Hbm[:, :],
                                ident16[0:MEAS, 0:MEAS])
            C1 = dynp.tile([128, 128], F16, tag="C1", name="C1")
            nc.vector.tensor_copy(C1[0:NM, 0:MEAS], ptB[0:NM, 0:MEAS])
            pf = mp2.tile([64, 64], F32, tag="ptF", name="ptF")
            MM(pf[0:NM, 0:MEAS], cst["J64_2"][:, :], C1[0:NM, 0:MEAS],
               start=True, stop=True)
            nc.vector.tensor_copy(mh[:, MEAS:2 * MEAS], pf[0:NM, 0:MEAS])
            nc.vector.tensor_copy(mh16[:, :], mh[:, :])

    # ---------------- main sequence ----------------
    dbgflag = [True]
    for s in range(SEQ_ITER):
        dbgflag[0] = s == 0
        with nc.named_scope(f"g1_{s}"):
            gated_graph("g1", "none" if s == 0 else "bh",
                        [ag[f"sec_in{s * 4 + i}"] for i in range(2)],
                        [ag[f"sec_out{s * 4 + i}"] for i in range(2)],
                        nh16, nsl)
        dump(1 if s == 0 else 8, nh16[0][:, :], 128, 128)
        if s == 1:
            dump(9, mstat[:, :], 128, 384)
        with nc.named_scope(f"nb_{s}"):
            nb_phase(s)
        if s == 0:
            dump(2, xb[0][:, 0:S], 128, S)
        with nc.named_scope(f"g2_{s}"):
            gated_graph("g2", "generic",
                        [ag[f"sec_in{s * 4 + 2 + i}"] for i in range(2)],
                        [ag[f"sec_out{s * 4 + 2 + i}"] for i in range(2)],
                        nh216, None)
        if s == 0:
            dump(3, nh216[0][:, :], 128, 128)
        with nc.named_scope(f"batt_{s}"):
            beat_attention()
        if s == 0:
            dump(4, bnT[0][:, :], 128, 256)
        with nc.named_scope(f"blstm_{s}"):
            run_lstm2(BEAT, NB, [bnT[0][:, :], bnT[1][:, :]],
                      [bnTr[0][:, :], bnTr[1][:, :]], "bwhp", "bwip",
                      "bbp", 2, Hfb, Hbb)
            build_beat()
            if s == 0:
                dump(5, bh0[:, :], 128, 128)
        with nc.named_scope(f"matt_{s}"):
            measure_attention()
        with nc.named_scope(f"mlstm_{s}"):
            run_lstm2(MEAS, NM, [mnT[:, :]], [mnTr[:, :]], "mwhp",
                      "mwip", "mbp", 1, Hfm, Hbm)
            build_meas()
            if s == 0:
                dump(6, mh[:, :], NM, 2 * MEAS)
        # rebuild x tiles for next iteration / final output
        with nc.named_scope(f"rebuild_{s}"):
            for k in range(8):
                pbs = ps_m.tile([128, 512], F32, tag="pm", name="pm")
                for half in range(2):
                    MM(pbs[:, 0:128],
                       cst[f"S_bs{k}"][:, half * 128:(half + 1) * 128],
                       bh16[half][:, :], start=(half == 0), stop=(half == 1))
                MM(pbs[:, 128:192], cst[f"S_ms{k}"][:, :], mh16[:, :],
                   start=True, stop=True)
                if s + 1 < SEQ_ITER:
                    nc.vector.tensor_copy(xb[k][:, 0:192], pbs[:, 0:192])
                    nc.vector.tensor_copy(xb[k][:, 192:S], nh16[k][:, :])
                else:
                    outst = acts.tile([128, S + SEC], F32, tag="outst",
                                      name="outst")
                    nc.vector.tensor_copy(outst[:, 0:192], pbs[:, 0:192])
                    nc.vector.tensor_copy(outst[:, 192:S], nh16[k][:, :])
                    nc.vector.tensor_copy(outst[:, S:S + SEC], nh216[k][:, :])
                    dma(out_dram[0, k * 128:(k + 1) * 128, :], outst[:, :])
            if s + 1 < SEQ_ITER:
                pbs = ps_m.tile([128, 512], F32, tag="pm", name="pm")
                for half in range(2):
                    MM(pbs[:, 0:128],
                       cst["S_bs_loc"][:, half * 128:(half + 1) * 128],
                       bh16[half][:, :], start=(half == 0), stop=(half == 1))
                MM(pbs[:, 128:192], cst["S_ms_loc"][:, :], mh16[:, :],
                   start=True, stop=True)
                nc.vector.tensor_copy(xl[:, 0:192], pbs[:, 0:192])
                nc.vector.tensor_copy(xl[:, 192:S], nsl[:, :])
                dump(7, xb[0][:, 0:S], 128, S)
    stack.close()


# ================= host side =================

def _host_inputs(inputs):
    f32, f16 = np.float32, np.float16
    nodes = np.asarray(inputs["nodes"], f32)[0]
    adjacency = np.asarray(inputs["adjacency"], f32)
    beat = np.asarray(inputs["beat_numbers"], np.int64)
    meas = np.asarray(inputs["measure_numbers"], np.int64)
    rep = {}
    rep["nodes_T16"] = nodes.T.astype(f16)
    fw = np.asarray(inputs["note_fc_w"], f32)
    fb = np.asarray(inputs["note_fc_b"], f32)[None, :]
    rep["note_fc_w16"] = fw.astype(f16)
    rep["note_fc_b16"] = fb.astype(f16)
    rep["note_fc_w32"] = fw
    rep["note_fc_b32"] = fb
    gbw = np.zeros((128, 3, S), f32)
    gw = np.asarray(inputs["gb_w"], f32)
    for fc, (st, w) in enumerate(FCS):
        gbw[0:w, fc, :] = gw[st:st + w, :]
    rep["gb_w32"] = gbw.reshape(128, 3 * S)
    rep["gb_b32"] = np.asarray(inputs["gb_b"], f32)[None, :]
    rep["gb_w16"] = rep["gb_w32"].astype(f16)
    rep["gb_b16"] = rep["gb_b32"].astype(f16)
    rep["batt_w"] = np.asarray(inputs["batt_w"], f32).reshape(2, 128,
                                                             2 * NOTE)
    rep["batt_b"] = np.asarray(inputs["batt_b"],
                               f32).reshape(2, 128).T.copy()
    rep["matt_w"] = np.asarray(inputs["matt_w"], f32)
    rep["matt_b"] = np.asarray(inputs["matt_b"], f32)[:, None]
    bc = np.asarray(inputs["batt_c"], f32)
    Cb = np.zeros((2 * NOTE, HEADS), f32)
    for h in range(HEADS):
        Cb[h * 32:(h + 1) * 32, h] = bc[h]
    rep["Cb"] = Cb.reshape(2, 128, HEADS)
    mcc = np.asarray(inputs["matt_c"], f32)
    Cm = np.zeros((2 * BEAT, HEADS), f32)
    for h in range(HEADS):
        Cm[h * 16:(h + 1) * 16, h] = mcc[h]
    rep["Cm"] = Cm
    Bf = np.zeros((HEADS, 2 * NOTE), f32)
    for h in range(HEADS):
        Bf[h, h * 32:(h + 1) * 32] = 1.0
    rep["Bfree_b"] = Bf
    Bm = np.zeros((HEADS, 2 * BEAT), f32)
    for h in range(HEADS):
        Bm[h, h * 16:(h + 1) * 16] = 1.0
    rep["Bfree_m"] = Bm
    Ppool = np.zeros((8, 128, 32), f32)
    for k in range(8):
        for p in range(128):
            b = beat[k * 128 + p] - 32 * k
            assert 0 <= b < 32, "beats not aligned to 128-node chunks"
            Ppool[k, p, b] = 1.0
    rep["Ppool"] = Ppool
    b2m = np.full(NB, 1 << 40, np.int64)
    np.minimum.at(b2m, beat, meas)
    Ppoolm = np.zeros((2, 128, 32), f32)
    for half in range(2):
        for p in range(128):
            m_ = b2m[half * 128 + p] - 32 * half
            assert 0 <= m_ < 32
            Ppoolm[half, p, m_] = 1.0
    rep["Ppoolm"] = np.concatenate([Ppoolm[0], Ppoolm[1]], axis=1)
    S_bs = np.zeros((8, NB, 128), f32)
    S_ms = np.zeros((8, NM, 128), f32)
    for k in range(8):
        for p in range(128):
            S_bs[k, beat[k * 128 + p], p] = 1.0
            S_ms[k, meas[k * 128 + p], p] = 1.0
    S_bs_hc = np.zeros((8, 128, 2, 128), f32)
    for k in range(8):
        S_bs_hc[k, :, 0, :] = S_bs[k, 0:128, :]
        S_bs_hc[k, :, 1, :] = S_bs[k, 128:256, :]
    rep["S_bs"] = S_bs_hc.reshape(8, 128, 256).astype(f16)
    rep["S_ms"] = S_ms.astype(f16)
    rep["ident"] = np.eye(128, dtype=f32)
    rep["ident16"] = np.eye(128, dtype=f32).astype(f16)

    def lstm_pack2(wi_f, wh_f, b_f, wi_b, wh_b, b_b, H):
        # pair0 = (f; i), pair1 = (o; 2*g); reference gate order i,f,g,o
        KIN = wi_f.shape[1]
        nkc = max(1, KIN // 128)
        whp = np.zeros((2, 2, H, 2 * H), f32)
        wip = np.zeros((2, nkc, 128, 4 * H), f32)
        bp = np.zeros((2, 1, 4 * H), f32)
        for d, (wi, wh, b) in enumerate(((wi_f, wh_f, b_f),
                                         (wi_b, wh_b, b_b))):
            blk = [wh[i * H:(i + 1) * H] for i in range(4)]  # i,f,g,o
            whp[d, 0, :, 0:H] = blk[1].T
            whp[d, 0, :, H:2 * H] = blk[0].T
            whp[d, 1, :, 0:H] = blk[3].T
            whp[d, 1, :, H:2 * H] = 2.0 * blk[2].T
            wt = wi.T  # (KIN, 4H) col blocks i,f,g,o
            for kc in range(nkc):
                w = wt[kc * 128:(kc + 1) * 128]
                r = w.shape[0]
                wip[d, kc, 0:r, 0:H] = w[:, H:2 * H]
                wip[d, kc, 0:r, H:2 * H] = w[:, 0:H]
                wip[d, kc, 0:r, 2 * H:3 * H] = w[:, 3 * H:4 * H]
                wip[d, kc, 0:r, 3 * H:4 * H] = 2.0 * w[:, 2 * H:3 * H]
            bp[d, 0, 0:H] = b[H:2 * H]
            bp[d, 0, H:2 * H] = b[0:H]
            bp[d, 0, 2 * H:3 * H] = b[3 * H:4 * H]
            bp[d, 0, 3 * H:4 * H] = 2.0 * b[2 * H:3 * H]
        return whp.astype(f16), wip.astype(f16), bp.astype(f16)

    g = lambda n: np.asarray(inputs[n], f32)
    rep["bwhp"], rep["bwip"], rep["bbp"] = lstm_pack2(
        g("blstm_wi_f"), g("blstm_wh_f"), g("blstm_b_f"),
        g("blstm_wi_b"), g("blstm_wh_b"), g("blstm_b_b"), BEAT)
    rep["mwhp"], rep["mwip"], rep["mbp"] = lstm_pack2(
        g("mlstm_wi_f"), g("mlstm_wh_f"), g("mlstm_b_f"),
        g("mlstm_wi_b"), g("mlstm_wh_b"), g("mlstm_b_b"), MEAS)
    idmv64 = np.zeros((128, 64), f32)
    idmv64[64:128] = np.eye(64) * 2.0
    rep["idmv64"] = idmv64.astype(f16)
    idmv32 = np.zeros((64, 32), f32)
    idmv32[32:64] = np.eye(32) * 2.0
    rep["idmv32"] = idmv32.astype(f16)
    rep["bwhp"] = (rep["bwhp"].astype(f32) * 2.0).astype(f16)
    rep["mwhp"] = (rep["mwhp"].astype(f32) * 2.0).astype(f16)
    rep["ident2_16"] = (np.eye(128, dtype=f32) * 2.0).astype(f16)
    rep["J128_2"] = (np.eye(128, dtype=f32)[::-1] * 2.0).astype(f16)
    rep["J64_2"] = (np.eye(64, dtype=f32)[::-1] * 2.0).astype(f16)
    rep["J32"] = np.eye(32, dtype=f32)[::-1].copy()
    for gg in ("g1", "g2"):
        for gate in ("z", "r", "h"):
            w = np.asarray(inputs[f"{gg}_w{gate}"], f32)  # (E, S, SEC)
            dyn = w[:, 192:320, :]                        # (E, 128, SEC)
            dhi = dyn.astype(f16)
            dlo = (dyn - dhi.astype(f32)).astype(f16)
            rep[f"{gg}_w{gate}_dhi"] = \
                dhi.transpose(1, 0, 2).reshape(128, E * SEC).copy()
            rep[f"{gg}_w{gate}_dlo"] = \
                dlo.transpose(1, 0, 2).reshape(128, E * SEC).copy()
            ws = np.zeros((128, E, 2 * SEC), f16)
            ws[0:128, :, 0:SEC] = w[:, 0:128, :].transpose(1, 0, 2)
            ws[0:64, :, SEC:2 * SEC] = w[:, 128:192, :].transpose(1, 0, 2)
            rep[f"{gg}_w{gate}_st"] = ws.reshape(128, E * 2 * SEC)
            rep[f"{gg}_u{gate}16"] = \
                np.asarray(inputs[f"{gg}_u{gate}"], f32).astype(f16)
            rep[f"{gg}_b{gate}16"] = \
                np.asarray(inputs[f"{gg}_b{gate}"], f32)[None, :].astype(f16)

    # beat/measure aggregated adjacency (static message terms for g1)
    if np.array_equal(beat, np.arange(N) // 4):
        adjB_full = adjacency.reshape(E, NB, 4, N).sum(2)
    else:
        Sb = np.zeros((NB, N), f32)
        Sb[beat, np.arange(N)] = 1.0
        adjB_full = np.einsum("bm,emn->ebn", Sb, adjacency)
    if np.array_equal(b2m, np.arange(NB) // 4):
        adjM_full = adjB_full.reshape(E, NM, 4, N).sum(2)
    else:
        Sm = np.zeros((NM, NB), f32)
        Sm[b2m, np.arange(NB)] = 1.0
        adjM_full = np.einsum("mb,ebn->emn", Sm, adjB_full)

    specs = _input_specs()
    # convert replicated entries once
    for kk in list(rep.keys()):
        shape, dt = specs[kk]
        npdt = np.float16 if dt == F16 else np.float32
        rep[kk] = np.ascontiguousarray(
            np.asarray(rep[kk]).reshape(shape).astype(npdt))
    in_maps = []
    for c in range(NCORES):
        sl = slice(c * LOC, (c + 1) * LOC)
        m = dict(rep)
        m["nodes_T_loc"] = np.ascontiguousarray(nodes[sl].T)
        adjc = adjacency[:, :, sl]
        m["adj_sl"] = np.ascontiguousarray(
            adjc.reshape(E, 8, 128, LOC).transpose(0, 2, 1, 3)
            .reshape(E, 128, N).astype(f16))
        m["adjB"] = np.ascontiguousarray(
            adjB_full[:, :, sl].reshape(E, 2, 128, LOC)
            .transpose(0, 2, 1, 3).reshape(E, 128, 2 * LOC).astype(f16))
        m["adjM"] = np.ascontiguousarray(adjM_full[:, :, sl].astype(f16))
        m["S_bs_loc"] = rep["S_bs"][c]
        m["S_ms_loc"] = rep["S_ms"][c]
        in_maps.append(m)
    return in_maps


def kernel(**inputs):
    if "nc" not in _CACHE:
        _CACHE["nc"] = _build_program()
    nc = _CACHE["nc"]
    in_maps = _host_inputs(inputs)
    res = bass_utils.run_bass_kernel_spmd(nc, in_maps,
                                          core_ids=list(range(NCORES)))
    _CACHE["last_res"] = res
    out = res.results[0]["out"]
    return np.asarray(out, np.float32)


# revision 31
# speedup vs baseline: 1.2141x; 1.0745x over previous
"""Trainium2 Bass kernel for nn_IsgnBeatMeasEncoder (gnn_message_passing).

Sharding: destination-node sharding for the gated-graph message passing
(128 dest-nodes/core; per-core adjacency slice resident in SBUF, fp16);
fp16 AllGather of the updated secondary state per graph iteration; nb
(graph-between) computed replicated from the gathered state (no second
collective); static message terms for g1 computed via beat/measure
aggregated adjacency; attention pooling via host-built one-hot matmuls;
BiLSTMs replicated with fw/bw batched and gate inputs pinned in PSUM.
"""
import numpy as np

import concourse.bass as bass
import concourse.mybir as mybir
from concourse import bacc
from concourse.tile import TileContext
from concourse import bass_utils

F32 = mybir.dt.float32
F16 = mybir.dt.float16

N = 1024
E = 10
IN = 78
NOTE = 128
BEAT = 64
MEAS = 32
S = 320
SEC = 128
HEADS = 8
NB = 256
NM = 64
SEQ_ITER = 2
GRAPH_ITER = 2
NCORES = 8
LOC = N // NCORES

FCS = [(0, 128), (128, 64), (192, 128)]  # (start, width); 0,1 static; 2 dyn

_CACHE = {}


def _input_specs():
    sp = dict(
        nodes_T16=((IN, N), F16),
        nodes_T_loc=((IN, LOC), F32),
        note_fc_w16=((IN, NOTE), F16),
        note_fc_b16=((1, NOTE), F16),
        note_fc_w32=((IN, NOTE), F32),
        note_fc_b32=((1, NOTE), F32),
        adj_sl=((E, 128, N), F16),
        adjB=((E, 128, 2 * LOC), F16),
        adjM=((E, NM, LOC), F16),
        gb_w16=((128, 3 * S), F16),
        gb_b16=((1, S), F16),
        gb_w32=((128, 3 * S), F32),
        gb_b32=((1, S), F32),
        batt_w=((2, 128, 2 * NOTE), F32),
        batt_b=((128, 2), F32),
        matt_w=((128, 2 * BEAT), F32),
        matt_b=((128, 1), F32),
        Cb=((2, 128, HEADS), F32),
        Cm=((128, HEADS), F32),
        Bfree_b=((HEADS, 2 * NOTE), F32),
        Bfree_m=((HEADS, 2 * BEAT), F32),
        Ppool=((8, 128, 32), F32),
        Ppoolm=((128, 2 * 32), F32),
        S_bs=((8, 128, 2 * 128), F16),   # half-chunked: [k][p,(half,c)]
        S_ms=((8, NM, 128), F16),
        S_bs_loc=((128, 2 * 128), F16),
        S_ms_loc=((NM, 128), F16),
        ident=((128, 128), F32),
        ident16=((128, 128), F16),
        bwhp=((2, 2, BEAT, 2 * BEAT), F16),   # [d][pair][H,(gA,gB)]
        bwip=((2, 2, 128, 4 * BEAT), F16),    # [d][kc][feat,(p0,p1)]
        bbp=((2, 1, 4 * BEAT), F16),
        mwhp=((2, 2, MEAS, 2 * MEAS), F16),
        mwip=((2, 1, 128, 4 * MEAS), F16),
        mbp=((2, 1, 4 * MEAS), F16),
        idmv64=((128, 64), F16),
        idmv32=((64, 32), F16),
        ident2_16=((128, 128), F16),
        J128_2=((128, 128), F16),
        J64_2=((64, 64), F16),
        J32=((32, 32), F32),
    )
    for g in ("g1", "g2"):
        for gate in ("z", "r", "h"):
            sp[f"{g}_w{gate}_dhi"] = ((128, E * SEC), F16)
            sp[f"{g}_w{gate}_dlo"] = ((128, E * SEC), F16)
            sp[f"{g}_w{gate}_st"] = ((128, E * 2 * SEC), F16)
            sp[f"{g}_u{gate}16"] = ((SEC, SEC), F16)
            sp[f"{g}_b{gate}16"] = ((1, SEC), F16)
    return sp


def _build_program():
    nc = bacc.Bacc("TRN2", target_bir_lowering=False, debug=False,
                   num_devices=NCORES)
    io = {}
    for name, (shape, dt) in _input_specs().items():
        io[name] = nc.dram_tensor(name, list(shape), dt,
                                  kind="ExternalInput").ap()
    out_dram = nc.dram_tensor("out", [1, N, S + SEC], F32,
                              kind="ExternalOutput").ap()
    dbg_dram = nc.dram_tensor("dbg", [16, 128, 512], F32,
                              kind="ExternalOutput").ap()
    _CACHE["dbg_dram"] = dbg_dram
    ag = {}
    for i in range(8):
        ag[f"sec_in{i}"] = nc.dram_tensor(f"sec_in{i}", [LOC, SEC], F16).ap()
        ag[f"sec_out{i}"] = nc.dram_tensor(f"sec_out{i}", [N, SEC], F16,
                                           addr_space="Shared").ap()
    with TileContext(nc) as tc:
        _emit(nc, tc, io, out_dram, ag, dbg_dram)
    nc.compile()
    return nc


def _emit(nc, tc, io, out_dram, ag, dbg_dram):
    import contextlib
    RG = [list(range(NCORES))]
    AF = mybir.ActivationFunctionType
    OP = mybir.AluOpType
    MM = nc.tensor.matmul

    stack = contextlib.ExitStack()
    const = stack.enter_context(tc.tile_pool(name="const", bufs=1))
    pers = stack.enter_context(tc.tile_pool(name="pers", bufs=1))
    acts = stack.enter_context(tc.tile_pool(name="acts", bufs=2))
    dynp = stack.enter_context(tc.tile_pool(name="dynp", bufs=2))
    lsp = stack.enter_context(tc.tile_pool(name="lsp", bufs=6))
    ps_t = stack.enter_context(tc.tile_pool(name="ps_t", bufs=1, space="PSUM"))
    ps_m = stack.enter_context(tc.tile_pool(name="ps_m", bufs=2, space="PSUM"))

    def dma(dst, src):
        nc.sync.dma_start(out=dst, in_=src)

    def dump(idx, src_ap, rows, cols):
        st = acts.tile([128, 512], F32, tag="dbgst", name="dbgst")
        nc.vector.tensor_copy(st[0:rows, 0:cols], src_ap)
        dma(dbg_dram[idx, 0:rows, 0:cols], st[0:rows, 0:cols])

    cst = {}

    def load(name, dt=None, src=None, tag=None):
        src = io[name] if src is None else src
        if dt is None:
            dt = src.dtype
        t = const.tile([src.shape[-2], src.shape[-1]], dt, tag=tag or name)
        dma(t[:, :], src)
        cst[tag or name] = t
        return t

    for nm in ("nodes_T16", "nodes_T_loc", "note_fc_w16", "note_fc_b16",
               "note_fc_w32", "note_fc_b32", "gb_w16", "gb_b16", "gb_w32",
               "gb_b32", "batt_b", "matt_w", "matt_b", "Cm", "Bfree_b",
               "Bfree_m", "Ppoolm", "S_bs_loc", "S_ms_loc", "ident",
               "ident16", "idmv64", "idmv32", "ident2_16", "J128_2",
               "J64_2", "J32"):
        load(nm)
    for kc in range(2):
        load("batt_w", src=io["batt_w"][kc], tag=f"battw{kc}")
        load("Cb", src=io["Cb"][kc], tag=f"Cb{kc}")
    for k in range(8):
        load("Ppool", src=io["Ppool"][k], tag=f"Ppool{k}")
        load("S_bs", src=io["S_bs"][k], tag=f"S_bs{k}")
        load("S_ms", src=io["S_ms"][k], tag=f"S_ms{k}")
    for e in range(E):
        load("adj_sl", src=io["adj_sl"][e], tag=f"adj{e}")
        load("adjB", src=io["adjB"][e], tag=f"adjB{e}")
        load("adjM", src=io["adjM"][e], tag=f"adjM{e}")
    for g in ("g1", "g2"):
        for gate in ("z", "r", "h"):
            load(f"{g}_w{gate}_dhi")
            load(f"{g}_w{gate}_dlo")
            load(f"{g}_w{gate}_st")
            load(f"{g}_u{gate}16")
            load(f"{g}_b{gate}16")
    for d in range(2):
        for p in range(2):
            load("bwhp", src=io["bwhp"][d, p], tag=f"bwhp{d}{p}")
            load("mwhp", src=io["mwhp"][d, p], tag=f"mwhp{d}{p}")
        for kc in range(2):
            load("bwip", src=io["bwip"][d, kc], tag=f"bwip{d}{kc}")
        load("mwip", src=io["mwip"][d, 0], tag=f"mwip{d}0")
        load("bbp", src=io["bbp"][d], tag=f"bbp{d}")
        load("mbp", src=io["mbp"][d], tag=f"mbp{d}")

    ones1 = const.tile([1, 512], F32, tag="ones1", name="ones1")
    nc.gpsimd.memset(ones1[:, :], 1.0)
    ones16 = const.tile([1, 512], F16, tag="ones16", name="ones16")
    nc.gpsimd.memset(ones16[:, :], 1.0)
    zsmall = const.tile([128, 2], F32, tag="zsmall", name="zsmall")
    nc.gpsimd.memset(zsmall[:, :], 0.0)
    z16 = const.tile([128, 2], F16, tag="z16", name="z16")
    nc.gpsimd.memset(z16[:, :], 0.0)
    ident = cst["ident"]
    ident16 = cst["ident16"]

    xb = [pers.tile([128, S], F16, tag=f"xb{k}", name=f"xb{k}")
          for k in range(8)]
    xl = pers.tile([128, S], F32, tag="xl", name="xl")
    nh16 = [pers.tile([128, SEC], F16, tag=f"nh16_{k}", name=f"nh16_{k}")
            for k in range(8)]
    nh216 = [pers.tile([128, SEC], F16, tag=f"nh216_{k}", name=f"nh216_{k}")
             for k in range(8)]
    bnT = [pers.tile([128, NB], F16, tag=f"bnT{h}", name=f"bnT{h}")
           for h in range(2)]
    bnTr = [pers.tile([128, NB], F16, tag=f"bnTr{h}", name=f"bnTr{h}")
            for h in range(2)]
    Hfb = pers.tile([BEAT, NB], F16, tag="Hfb", name="Hfb")
    Hbb = pers.tile([BEAT, NB], F16, tag="Hbb", name="Hbb")
    Hfm = pers.tile([MEAS, NM], F16, tag="Hfm", name="Hfm")
    Hbm = pers.tile([MEAS, NM], F16, tag="Hbm", name="Hbm")
    bh0 = pers.tile([128, 128], F32, tag="bh0", name="bh0")
    bh1 = pers.tile([128, 128], F32, tag="bh1", name="bh1")
    bh16 = [pers.tile([128, 128], F16, tag=f"bh16_{h}", name=f"bh16_{h}")
            for h in range(2)]
    bhT = pers.tile([128, NB], F32, tag="bhT", name="bhT")
    mh = pers.tile([NM, 2 * MEAS], F32, tag="mh", name="mh")
    mh16 = pers.tile([NM, 2 * MEAS], F16, tag="mh16", name="mh16")
    mnT = pers.tile([2 * BEAT, NM], F16, tag="mnT", name="mnT")
    mnTr = pers.tile([2 * BEAT, NM], F16, tag="mnTr", name="mnTr")
    mstat = pers.tile([128, 3 * SEC], F32, tag="mstat", name="mstat")
    nsl = pers.tile([128, SEC], F32, tag="nsl", name="nsl")

    def transpose_to(dst_ap, src_ap, rows):
        cols = src_ap.shape[-1]
        pt = ps_t.tile([128, 128], F32, tag="pt", name="pt")
        nc.tensor.transpose(pt[0:cols, 0:rows], src_ap,
                            ident[0:rows, 0:rows])
        nc.vector.tensor_copy(dst_ap, pt[0:cols, 0:rows])

    def transpose_new(src_ap, rows, tag="tr"):
        cols = src_ap.shape[-1]
        sb = acts.tile([cols, rows], F32, tag=tag, name=tag)
        transpose_to(sb[0:cols, 0:rows], src_ap, rows)
        return sb

    def transpose16(pool, src_ap, rows, tag):
        # fp16 src -> fp16 transposed SBUF tile
        cols = src_ap.shape[-1]
        pt = pool.tile([128, 128], F16, tag="pt16", name="pt16")
        nc.tensor.transpose(pt[0:cols, 0:rows], src_ap,
                            ident16[0:rows, 0:rows])
        sb = dynp.tile([128, 128], F16, tag=tag, name=tag)
        nc.vector.tensor_copy(sb[0:cols, 0:rows], pt[0:cols, 0:rows])
        return sb

    # ---------------- initial x ----------------
    for k in range(8):
        nc.gpsimd.memset(xb[k][:, 0:192], 0.0)
        pm = ps_m.tile([128, 512], F32, tag="pm", name="pm")
        MM(pm[:, 0:NOTE], cst["nodes_T16"][:, k * 128:(k + 1) * 128],
           cst["note_fc_w16"][:, :], start=True, stop=False)
        MM(pm[:, 0:NOTE], ones16[0:1, 0:128], cst["note_fc_b16"][:, :],
           start=False, stop=True)
        nc.scalar.activation(xb[k][:, 192:S], pm[:, 0:NOTE], AF.Tanh)
    nc.gpsimd.memset(xl[:, 0:192], 0.0)
    pm = ps_m.tile([128, 512], F32, tag="pm", name="pm")
    MM(pm[:, 0:NOTE], cst["nodes_T_loc"][:, :], cst["note_fc_w32"][:, :],
       start=True, stop=False)
    MM(pm[:, 0:NOTE], ones1[0:1, 0:LOC], cst["note_fc_b32"][:, :],
       start=False, stop=True)
    nc.scalar.activation(xl[:, 192:S], pm[:, 0:NOTE], AF.Tanh)
    dump(0, xb[0][:, 192:S], 128, 128)

    # ---------------- gated graph ----------------
    def gated_graph(g, static_mode, agins, agouts, nh_tiles, save_local):
        wdh = [cst[f"{g}_w{gt}_dhi"] for gt in "zrh"]
        wdl = [cst[f"{g}_w{gt}_dlo"] for gt in "zrh"]
        wst = [cst[f"{g}_w{gt}_st"] for gt in "zrh"]
        us = [cst[f"{g}_u{gt}16"] for gt in "zrh"]
        bs = [cst[f"{g}_b{gt}16"] for gt in "zrh"]
        with tc.tile_pool(name=f"ga{g}", bufs=1, space="PSUM") as gacc, \
                tc.tile_pool(name=f"gd{g}", bufs=2, space="PSUM") as gact:
            mt = [gacc.tile([128, SEC], F32, tag=f"m{gi}", name=f"m{gi}")
                  for gi in range(3)]
            # ---- static messages (constant across graph iters) ----
            if static_mode != "none":
                for e in range(E):
                    pa = gact.tile([128, 128], F32, tag="pact", name="pact")
                    if static_mode == "bh":
                        for ch in range(2):
                            MM(pa[:, :], bh16[ch][:, :],
                               cst[f"adjB{e}"][:, ch * 128:(ch + 1) * 128],
                               start=(ch == 0), stop=(ch == 1))
                    else:
                        for k in range(8):
                            MM(pa[:, :], xb[k][:, 0:128],
                               cst[f"adj{e}"][:, k * 128:(k + 1) * 128],
                               start=(k == 0), stop=(k == 7))
                    a0 = dynp.tile([128, 128], F16, tag="a0", name="a0")
                    nc.vector.tensor_copy(a0[:, :], pa[:, :])
                    pa1 = gact.tile([128, 128], F32, tag="pact", name="pact")
                    if static_mode == "bh":
                        MM(pa1[0:NM, :], mh16[:, :], cst[f"adjM{e}"][:, :],
                           start=True, stop=True)
                    else:
                        for k in range(8):
                            MM(pa1[0:64, :], xb[k][:, 128:192],
                               cst[f"adj{e}"][:, k * 128:(k + 1) * 128],
                               start=(k == 0), stop=(k == 7))
                    a1 = dynp.tile([128, 128], F16, tag="a1", name="a1")
                    nc.vector.tensor_copy(a1[0:64, :], pa1[0:64, :])
                    for gi in range(3):
                        MM(mt[gi][:, :], a0[:, :],
                           wst[gi][:, e * 256:e * 256 + 128],
                           start=(e == 0), stop=False)
                        MM(mt[gi][:, :], a1[0:64, :],
                           wst[gi][0:64, e * 256 + 128:e * 256 + 256],
                           start=False, stop=(e == E - 1))
                for gi in range(3):
                    nc.vector.tensor_copy(mstat[:, gi * SEC:(gi + 1) * SEC],
                                          mt[gi][:, :])
            # ---- graph iterations ----
            for it in range(GRAPH_ITER):
                last = it == GRAPH_ITER - 1
                for e in range(E):
                    pa = gact.tile([128, 128], F32, tag="pact", name="pact")
                    for k in range(8):
                        MM(pa[:, :], xb[k][:, 192:S],
                           cst[f"adj{e}"][:, k * 128:(k + 1) * 128],
                           start=(k == 0), stop=(k == 7))
                    ad = dynp.tile([128, 128], F16, tag="ad", name="ad")
                    nc.vector.tensor_copy(ad[:, :], pa[:, :])
                    for gi in range(3):
                        MM(mt[gi][:, :], ad[:, :],
                           wdh[gi][:, e * SEC:(e + 1) * SEC],
                           start=(e == 0), stop=False)
                        MM(mt[gi][:, :], ad[:, :],
                           wdl[gi][:, e * SEC:(e + 1) * SEC],
                           start=False, stop=False)
                xs = xl[:, 192:S]
                pt = ps_t.tile([128, 128], F32, tag="pt", name="pt")
                nc.tensor.transpose(pt[:, :], xs, ident[:, :])
                xsT = dynp.tile([128, 128], F16, tag="xsT", name="xsT")
                nc.vector.tensor_copy(xsT[:, :], pt[:, :])
                for gi in range(3):
                    MM(mt[gi][:, :], ones16[0:1, 0:128],
                       bs[gi][:, :], start=False, stop=False)
                for gi in range(2):
                    MM(mt[gi][:, :], xsT[:, :], us[gi][:, :],
                       start=False, stop=True)

                def gate_act(gi, func, dst):
                    reg = mt[gi][:, :]
                    if static_mode != "none":
                        tz = acts.tile([128, SEC], F32, tag="tz", name="tz")
                        nc.vector.tensor_tensor(
                            tz[:, :], reg, mstat[:, gi * SEC:(gi + 1) * SEC],
                            op=OP.add)
                        nc.scalar.activation(dst, tz[:, :], func)
                    else:
                        nc.scalar.activation(dst, reg, func)

                zt = acts.tile([128, SEC], F32, tag="zt", name="zt")
                rt = acts.tile([128, SEC], F32, tag="rt", name="rt")
                gate_act(0, AF.Sigmoid, zt[:, :])
                gate_act(1, AF.Sigmoid, rt[:, :])
                rx = acts.tile([128, SEC], F32, tag="rx", name="rx")
                nc.vector.tensor_tensor(rx[:, :], rt[:, :], xs, op=OP.mult)
                pt2 = ps_t.tile([128, 128], F32, tag="pt", name="pt")
                nc.tensor.transpose(pt2[:, :], rx[:, :], ident[:, :])
                rxT = dynp.tile([128, 128], F16, tag="rxT", name="rxT")
                nc.vector.tensor_copy(rxT[:, :], pt2[:, :])
                MM(mt[2][:, :], rxT[:, :], us[2][:, :],
                   start=False, stop=True)
                ht = acts.tile([128, SEC], F32, tag="ht", name="ht")
                gate_act(2, AF.Tanh, ht[:, :])
                t1 = acts.tile([128, SEC], F32, tag="t1", name="t1")
                nc.vector.tensor_tensor(t1[:, :], zt[:, :], xs, op=OP.mult)
                t2 = acts.tile([128, SEC], F32, tag="t2", name="t2")
                nc.vector.tensor_tensor(t2[:, :], rt[:, :], ht[:, :],
                                        op=OP.mult)
                ns = acts.tile([128, SEC], F32, tag="ns", name="ns")
                nc.vector.tensor_tensor(ns[:, :], xs, t1[:, :],
                                        op=OP.subtract)
                nc.vector.tensor_tensor(ns[:, :], ns[:, :], t2[:, :],
                                        op=OP.add)
                ns16 = acts.tile([128, SEC], F16, tag="ns16", name="ns16")
                nc.vector.tensor_copy(ns16[:, :], ns[:, :])
                if g == "g1" and it == 0 and dbgflag[0]:
                    dump(13, ns[:, :], 128, 128)
                a_in, a_out = agins[it], agouts[it]
                dma(a_in, ns16[:, :])
                nc.gpsimd.collective_compute(
                    "AllGather", OP.bypass, replica_groups=RG,
                    ins=[a_in], outs=[a_out])
                if not last:
                    for k in range(8):
                        dma(xb[k][:, 192:S], a_out[k * 128:(k + 1) * 128, :])
                else:
                    for k in range(8):
                        dma(nh_tiles[k][:, :],
                            a_out[k * 128:(k + 1) * 128, :])
                nc.vector.tensor_copy(xl[:, 192:S], ns[:, :])
                if last and save_local is not None:
                    nc.vector.tensor_copy(save_local[:, :], ns[:, :])

    # ---------------- nb (graph-between), replicated ----------------
    def nb_phase(s):
        fcs = [2] if s == 0 else [0, 1, 2]
        with tc.tile_pool(name="nbp", bufs=2, space="PSUM") as nbp:
          for k in range(8):
            pnb = ps_m.tile([128, 512], F32, tag="pm", name="pm")
            for fc in fcs:
                st, wd = FCS[fc]
                src = nh16[k][:, :] if fc == 2 else xb[k][:, st:st + wd]
                tT = transpose16(nbp, src, 128, tag="nbT")
                MM(pnb[:, 0:S], tT[0:wd, 0:128],
                   cst["gb_w16"][0:wd, fc * S:(fc + 1) * S],
                   start=(fc == fcs[0]), stop=False)
            MM(pnb[:, 0:S], ones16[0:1, 0:128], cst["gb_b16"][:, :],
               start=False, stop=True)
            nc.scalar.activation(xb[k][:, 0:S], pnb[:, 0:S], AF.Relu)
        # local f32 copy (xl holds nh locally: primary + nsl sec)
        pnl = ps_m.tile([128, 512], F32, tag="pm", name="pm")
        for fc in fcs:
            st, wd = FCS[fc]
            tT = transpose_new(xl[:, st:st + wd], 128, tag="nbT32")
            MM(pnl[:, 0:S], tT[0:wd, 0:128],
               cst["gb_w32"][0:wd, fc * S:(fc + 1) * S],
               start=(fc == fcs[0]), stop=False)
        MM(pnl[:, 0:S], ones1[0:1, 0:128], cst["gb_b32"][:, :],
           start=False, stop=True)
        nc.scalar.activation(xl[:, 0:S], pnl[:, 0:S], AF.Relu)

    # ---------------- beat attention ----------------
    def beat_attention():
        for k in range(8):
            cat_nm = acts.tile([128, 2 * NOTE], F32, tag="cat_nm",
                               name="cat_nm")
            nc.vector.tensor_copy(cat_nm[:, 0:NOTE], nh16[k][:, :])
            nc.vector.tensor_copy(cat_nm[:, NOTE:2 * NOTE], nh216[k][:, :])
            ct = [transpose_new(cat_nm[:, kc * 128:(kc + 1) * 128], 128,
                                tag=f"ct{kc}") for kc in range(2)]
            aT = []
            for mc in range(2):
                pa = ps_m.tile([128, 512], F32, tag="pm", name="pm")
                for kc in range(2):
                    MM(pa[:, 0:128],
                       cst[f"battw{kc}"][:, mc * 128:(mc + 1) * 128],
                       ct[kc][:, :], start=(kc == 0), stop=(kc == 1))
                sb = acts.tile([128, 128], F32, tag=f"aT{mc}", name=f"aT{mc}")
                nc.scalar.activation(sb[:, :], pa[:, 0:128], AF.Tanh,
                                     bias=cst["batt_b"][:, mc:mc + 1])
                aT.append(sb)
            psim = ps_t.tile([128, 128], F32, tag="pt", name="pt")
            for kc in range(2):
                MM(psim[0:HEADS, :], cst[f"Cb{kc}"][:, :], aT[kc][:, :],
                   start=(kc == 0), stop=(kc == 1))
            pp = acts.tile([HEADS, 128], F32, tag="pp", name="pp")
            nc.scalar.activation(pp[:, :], psim[0:HEADS, :], AF.Sigmoid)
            qq = acts.tile([HEADS, 128], F32, tag="qq", name="qq")
            nc.scalar.activation(qq[:, :], psim[0:HEADS, :], AF.Sigmoid,
                                 scale=-1.0)
            rq = acts.tile([HEADS, 128], F32, tag="rq", name="rq")
            nc.vector.reciprocal(rq[:, :], qq[:, :])
            wt = acts.tile([HEADS, 128], F32, tag="wt", name="wt")
            nc.vector.tensor_tensor(wt[:, :], pp[:, :], rq[:, :], op=OP.mult)
            pwe = ps_m.tile([128, 512], F32, tag="pm", name="pm")
            wexp = acts.tile([128, 2 * NOTE], F32, tag="wexp", name="wexp")
            MM(pwe[:, 0:256], wt[:, :], cst["Bfree_b"][:, :],
               start=True, stop=True)
            nc.vector.tensor_copy(wexp[:, :], pwe[:, 0:256])
            tt = acts.tile([128, 2 * NOTE], F32, tag="tt", name="tt")
            nc.vector.tensor_tensor(tt[:, :], cat_nm[:, :], wexp[:, :],
                                    op=OP.mult)
            pool = ps_m.tile([128, 512], F32, tag="pm", name="pm")
            MM(pool[0:32, 0:256], cst[f"Ppool{k}"][:, :], tt[:, :],
               start=True, stop=True)
            MM(pool[0:32, 256:512], cst[f"Ppool{k}"][:, :], wexp[:, :],
               start=True, stop=True)
            rd = acts.tile([32, 256], F32, tag="rd", name="rd")
            nc.vector.reciprocal(rd[:, :], pool[0:32, 256:512])
            bnk = acts.tile([32, 256], F32, tag="bnk", name="bnk")
            nc.vector.tensor_tensor(bnk[:, :], pool[0:32, 0:256], rd[:, :],
                                    op=OP.mult)
            for h in range(2):
                transpose_to(bnT[h][:, k * 32:(k + 1) * 32],
                             bnk[0:32, h * 128:(h + 1) * 128], 32)
                ptr = ps_t.tile([128, 128], F32, tag="pt", name="pt")
                nc.tensor.transpose(ptr[0:128, 0:32],
                                    bnk[0:32, h * 128:(h + 1) * 128],
                                    cst["J32"][:, :])
                nc.vector.tensor_copy(bnTr[h][:, (7 - k) * 32:(8 - k) * 32],
                                      ptr[0:128, 0:32])

    # ---------------- measure attention ----------------
    def measure_attention():
        paT = ps_m.tile([128, 512], F32, tag="pm", name="pm")
        MM(paT[:, 0:NB], cst["matt_w"][:, :], bhT[:, :],
           start=True, stop=True)
        amT = acts.tile([128, NB], F32, tag="amT", name="amT")
        nc.scalar.activation(amT[:, :], paT[:, 0:NB], AF.Tanh,
                             bias=cst["matt_b"][:, 0:1])
        psim = ps_t.tile([128, 128], F32, tag="pt", name="pt")
        pp = acts.tile([HEADS, NB], F32, tag="ppm", name="ppm")
        qq = acts.tile([HEADS, NB], F32, tag="qqm", name="qqm")
        for hc in range(2):
            MM(psim[0:HEADS, 0:128], cst["Cm"][:, :],
               amT[:, hc * 128:(hc + 1) * 128], start=True, stop=True)
            nc.scalar.activation(pp[:, hc * 128:(hc + 1) * 128],
                                 psim[0:HEADS, 0:128], AF.Sigmoid)
            nc.scalar.activation(qq[:, hc * 128:(hc + 1) * 128],
                                 psim[0:HEADS, 0:128], AF.Sigmoid,
                                 scale=-1.0)
        rq = acts.tile([HEADS, NB], F32, tag="rqm", name="rqm")
        nc.vector.reciprocal(rq[:, :], qq[:, :])
        wt = acts.tile([HEADS, NB], F32, tag="wtm", name="wtm")
        nc.vector.tensor_tensor(wt[:, :], pp[:, :], rq[:, :], op=OP.mult)
        for h in range(2):
            bh_h = bh0 if h == 0 else bh1
            pwe = ps_m.tile([128, 512], F32, tag="pm", name="pm")
            MM(pwe[:, 0:2 * BEAT], wt[:, h * 128:(h + 1) * 128],
               cst["Bfree_m"][:, :], start=True, stop=True)
            wexp = acts.tile([128, 2 * BEAT], F32, tag="wexpm", name="wexpm")
            nc.vector.tensor_copy(wexp[:, :], pwe[:, 0:2 * BEAT])
            tt = acts.tile([128, 2 * BEAT], F32, tag="ttm", name="ttm")
            nc.vector.tensor_tensor(tt[:, :], bh_h[:, :], wexp[:, :],
                                    op=OP.mult)
            pool = ps_m.tile([128, 512], F32, tag="pm", name="pm")
            MM(pool[0:32, 0:128], cst["Ppoolm"][:, h * 32:(h + 1) * 32],
               tt[:, :], start=True, stop=True)
            MM(pool[0:32, 128:256], cst["Ppoolm"][:, h * 32:(h + 1) * 32],
               wexp[:, :], start=True, stop=True)
            rd = acts.tile([32, 128], F32, tag="rdm", name="rdm")
            nc.vector.reciprocal(rd[:, :], pool[0:32, 128:256])
            mnk = acts.tile([32, 128], F32, tag="mnk", name="mnk")
            nc.vector.tensor_tensor(mnk[:, :], pool[0:32, 0:128], rd[:, :],
                                    op=OP.mult)
            transpose_to(mnT[:, h * 32:(h + 1) * 32], mnk[0:32, :], 32)
            ptr = ps_t.tile([128, 128], F32, tag="pt", name="pt")
            nc.tensor.transpose(ptr[0:128, 0:32], mnk[0:32, :],
                                cst["J32"][:, :])
            nc.vector.tensor_copy(mnTr[:, (1 - h) * 32:(2 - h) * 32],
                                  ptr[0:128, 0:32])

    # ---------------- LSTM ----------------
    def run_lstm2(H, T, inT, inTr, whp, wip, bp, nkc, Hf_t, Hb_t):
        """Decoupled fw/bw scan. Per dir PSUM U [2H, 2T]:
        pair0 cols 0:T = (f @ rows 0:H ; i @ rows H:2H),
        pair1 cols T:2T = (o ; 2*g~). All gates sigmoid (tanh folded as
        2*sigma(2x)-1 with weights/bias pre-scaled). c and h live at rows
        0:H; the i*g~ product is moved from rows H:2H via a PE
        identity-matmul."""
        Hs = [Hf_t, Hb_t]
        with tc.tile_pool(name=f"lu{H}", bufs=1, space="PSUM") as up, \
                tc.tile_pool(name=f"lm{H}", bufs=2, space="PSUM") as mp:
            U = [up.tile([2 * H, 2 * T], F32, tag=f"U{d}", name=f"U{H}{d}")
                 for d in range(2)]
            for d in range(2):
                srcs = inT if d == 0 else inTr
                for p in range(2):
                    reg = U[d][:, p * T:(p + 1) * T]
                    for kc in range(nkc):
                        MM(reg,
                           cst[f"{wip}{d}{kc}"][:, p * 2 * H:(p + 1) * 2 * H],
                           srcs[kc], start=(kc == 0 and p == 0), stop=False)
                    MM(reg, cst[f"{bp}{d}"][0:1, p * 2 * H:(p + 1) * 2 * H],
                       ones16[0:1, 0:T], start=False, stop=False)
            idm = cst["idmv64" if H == 64 else "idmv32"]
            cs = [None, None]
            sio_st = [None, None]
            mv_st = [None, None]

            def mm_stage(d, t):
                rhs = z16[0:H, 0:1] if t == 0 else Hs[d][0:H, t - 1:t]
                MM(U[d][:, t:t + 1], cst[f"{whp}{d}0"][:, :], rhs,
                   start=False, stop=True)
                MM(U[d][:, T + t:T + t + 1], cst[f"{whp}{d}1"][:, :],
                   rhs, start=False, stop=True)

            def sig_stage(d, t):
                sv = lsp.tile([2 * H, 2], F32, tag=f"sio{H}{d}",
                              name=f"sio{H}{d}")
                Uv = U[d].rearrange("p (pr t) -> p pr t", pr=2, t=T)
                nc.scalar.activation(
                    sv[0:2 * H, :].rearrange("p (a b) -> p a b", a=2, b=1),
                    Uv[:, :, t:t + 1], AF.Sigmoid)
                sio_st[d] = sv
                w16 = lsp.tile([2 * H, 2], F16, tag=f"w{H}{d}",
                               name=f"w{H}{d}")
                nc.vector.scalar_tensor_tensor(
                    w16[H:2 * H, 0:1], sv[H:2 * H, 1:2], 0.5,
                    sv[H:2 * H, 0:1], op0=OP.subtract, op1=OP.mult)
                mv = mp.tile([128, 8], F32, tag=f"mv{H}", name=f"mv{H}")
                MM(mv[0:H, 0:1], idm[H:2 * H, 0:H], w16[H:2 * H, 0:1],
                   start=True, stop=True)
                mv_st[d] = mv

            def tail_stage(d, t):
                sv = sio_st[d]
                cprev = zsmall[0:H, 0:1] if t == 0 else cs[d][0:H, 0:1]
                cn = lsp.tile([2 * H, 2], F32, tag=f"cn{H}{d}",
                              name=f"cn{H}{d}")
                nc.vector.scalar_tensor_tensor(
                    cn[0:H, 0:1], cprev, sv[0:H, 0:1],
                    mv_st[d][0:H, 0:1], op0=OP.mult, op1=OP.add)
                cs[d] = cn
                tcx = lsp.tile([2 * H, 2], F32, tag=f"tc{H}{d}",
                               name=f"tc{H}{d}")
                nc.scalar.activation(tcx[0:H, 0:1], cn[0:H, 0:1],
                                     AF.Sigmoid, scale=2.0)
                nc.vector.scalar_tensor_tensor(
                    Hs[d][0:H, t:t + 1], tcx[0:H, 0:1], 0.5,
                    sv[0:H, 1:2], op0=OP.subtract, op1=OP.mult)

            for t in range(T):
                mm_stage(0, t)
                if t > 0:
                    tail_stage(1, t - 1)
                sig_stage(0, t)
                mm_stage(1, t)
                tail_stage(0, t)
                sig_stage(1, t)
            tail_stage(1, T - 1)

    def build_beat():
        with tc.tile_pool(name="bhp", bufs=2, space="PSUM") as bp2:
            for half in range(2):
                bh_h = bh0 if half == 0 else bh1
                ptA = bp2.tile([128, 128], F16, tag="ptA", name="ptA")
                nc.tensor.transpose(ptA[0:128, 0:BEAT],
                                    Hfb[:, half * 128:(half + 1) * 128],
                                    ident16[0:BEAT, 0:BEAT])
                nc.scalar.activation(bh_h[:, 0:BEAT], ptA[0:128, 0:BEAT],
                                     AF.Copy, scale=2.0)
                ptB = bp2.tile([128, 128], F16, tag="ptA", name="ptA")
                nc.tensor.transpose(
                    ptB[0:128, 0:BEAT],
                    Hbb[:, (1 - half) * 128:(2 - half) * 128],
                    ident16[0:BEAT, 0:BEAT])
                C1 = dynp.tile([128, 128], F16, tag="C1", name="C1")
                nc.vector.tensor_copy(C1[0:128, 0:BEAT],
                                      ptB[0:128, 0:BEAT])
                pf = bp2.tile([128, 128], F32, tag="ptF", name="ptF")
                MM(pf[0:128, 0:BEAT], cst["J128_2"][:, :],
                   C1[0:128, 0:BEAT], start=True, stop=True)
                nc.vector.tensor_copy(bh_h[:, BEAT:2 * BEAT],
                                      pf[0:128, 0:BEAT])
                nc.vector.tensor_copy(bh16[half][:, :], bh_h[:, :])
                transpose_to(bhT[:, half * 128:(half + 1) * 128],
                             bh_h[:, :], 128)

    def build_meas():
        with tc.tile_pool(name="mhp", bufs=2, space="PSUM") as mp2:
            ptA = mp2.tile([64, 64], F16, tag="ptA", name="ptA")
            nc.tensor.transpose(ptA[0:NM, 0:MEAS], Hfm[:, :],
                                ident16[0:MEAS, 0:MEAS])
            nc.scalar.activation(mh[:, 0:MEAS], ptA[0:NM, 0:MEAS],
                                 AF.Copy, scale=2.0)
            ptB = mp2.tile([64, 64], F16, tag="ptA", name="ptA")
            nc.tensor.transpose(ptB[0:NM, 0:MEAS], Hbm[:, :],
                                ident16[0:MEAS, 0:MEAS])
            C1 = dynp.tile([128, 128], F16, tag="C1", name="C1")
            nc.vector.tensor_copy(C1[0:NM, 0:MEAS], ptB[0:NM, 0:MEAS])
            pf = mp2.tile([64, 64], F32, tag="ptF", name="ptF")
            MM(pf[0:NM, 0:MEAS], cst["J64_2"][:, :], C1[0:NM, 0:MEAS],
               start=True, stop=True)
            nc.vector.tensor_copy(mh[:, MEAS:2 * MEAS], pf[0:NM, 0:MEAS])
            nc.vector.tensor_copy(mh16[:, :], mh[:, :])

    # ---------------- main sequence ----------------
    dbgflag = [True]
    for s in range(SEQ_ITER):
        dbgflag[0] = s == 0
        with nc.named_scope(f"g1_{s}"):
            gated_graph("g1", "none" if s == 0 else "bh",
                        [ag[f"sec_in{s * 4 + i}"] for i in range(2)],
                        [ag[f"sec_out{s * 4 + i}"] for i in range(2)],
                        nh16, nsl)
        dump(1 if s == 0 else 8, nh16[0][:, :], 128, 128)
        if s == 1:
            dump(9, mstat[:, :], 128, 384)
        with nc.named_scope(f"nb_{s}"):
            nb_phase(s)
        if s == 0:
            dump(2, xb[0][:, 0:S], 128, S)
        with nc.named_scope(f"g2_{s}"):
            gated_graph("g2", "generic",
                        [ag[f"sec_in{s * 4 + 2 + i}"] for i in range(2)],
                        [ag[f"sec_out{s * 4 + 2 + i}"] for i in range(2)],
                        nh216, None)
        if s == 0:
            dump(3, nh216[0][:, :], 128, 128)
        with nc.named_scope(f"batt_{s}"):
            beat_attention()
        if s == 0:
            dump(4, bnT[0][:, :], 128, 256)
        with nc.named_scope(f"blstm_{s}"):
            run_lstm2(BEAT, NB, [bnT[0][:, :], bnT[1][:, :]],
                      [bnTr[0][:, :], bnTr[1][:, :]], "bwhp", "bwip",
                      "bbp", 2, Hfb, Hbb)
            build_beat()
            if s == 0:
                dump(5, bh0[:, :], 128, 128)
        with nc.named_scope(f"matt_{s}"):
            measure_attention()
        with nc.named_scope(f"mlstm_{s}"):
            run_lstm2(MEAS, NM, [mnT[:, :]], [mnTr[:, :]], "mwhp",
                      "mwip", "mbp", 1, Hfm, Hbm)
            build_meas()
            if s == 0:
                dump(6, mh[:, :], NM, 2 * MEAS)
        # rebuild x tiles for next iteration / final output
        with nc.named_scope(f"rebuild_{s}"):
            for k in range(8):
                pbs = ps_m.tile([128, 512], F32, tag="pm", name="pm")
                for half in range(2):
                    MM(pbs[:, 0:128],
                       cst[f"S_bs{k}"][:, half * 128:(half + 1) * 128],
                       bh16[half][:, :], start=(half == 0), stop=(half == 1))
                MM(pbs[:, 128:192], cst[f"S_ms{k}"][:, :], mh16[:, :],
                   start=True, stop=True)
                if s + 1 < SEQ_ITER:
                    nc.vector.tensor_copy(xb[k][:, 0:192], pbs[:, 0:192])
                    nc.vector.tensor_copy(xb[k][:, 192:S], nh16[k][:, :])
                else:
                    outst = acts.tile([128, S + SEC], F32, tag="outst",
                                      name="outst")
                    nc.vector.tensor_copy(outst[:, 0:192], pbs[:, 0:192])
                    nc.vector.tensor_copy(outst[:, 192:S], nh16[k][:, :])
                    nc.vector.tensor_copy(outst[:, S:S + SEC], nh216[k][:, :])
                    dma(out_dram[0, k * 128:(k + 1) * 128, :], outst[:, :])
            if s + 1 < SEQ_ITER:
                pbs = ps_m.tile([128, 512], F32, tag="pm", name="pm")
                for half in range(2):
                    MM(pbs[:, 0:128],
                       cst["S_bs_loc"][:, half * 128:(half + 1) * 128],
                       bh16[half][:, :], start=(half == 0), stop=(half == 1))
                MM(pbs[:, 128:192], cst["S_ms_loc"][:, :], mh16[:, :],
                   start=True, stop=True)
                nc.vector.tensor_copy(xl[:, 0:192], pbs[:, 0:192])
                nc.vector.tensor_copy(xl[:, 192:S], nsl[:, :])
                dump(7, xb[0][:, 0:S], 128, S)
    stack.close()


# ================= host side =================

def _host_inputs(inputs):
    f32, f16 = np.float32, np.float16
    nodes = np.asarray(inputs["nodes"], f32)[0]
    adjacency = np.asarray(inputs["adjacency"], f32)
    beat = np.asarray(inputs["beat_numbers"], np.int64)
    meas = np.asarray(inputs["measure_numbers"], np.int64)
    rep = {}
    rep["nodes_T16"] = nodes.T.astype(f16)
    fw = np.asarray(inputs["note_fc_w"], f32)
    fb = np.asarray(inputs["note_fc_b"], f32)[None, :]
    rep["note_fc_w16"] = fw.astype(f16)
    rep["note_fc_b16"] = fb.astype(f16)
    rep["note_fc_w32"] = fw
    rep["note_fc_b32"] = fb
    gbw = np.zeros((128, 3, S), f32)
    gw = np.asarray(inputs["gb_w"], f32)
    for fc, (st, w) in enumerate(FCS):
        gbw[0:w, fc, :] = gw[st:st + w, :]
    rep["gb_w32"] = gbw.reshape(128, 3 * S)
    rep["gb_b32"] = np.asarray(inputs["gb_b"], f32)[None, :]
    rep["gb_w16"] = rep["gb_w32"].astype(f16)
    rep["gb_b16"] = rep["gb_b32"].astype(f16)
    rep["batt_w"] = np.asarray(inputs["batt_w"], f32).reshape(2, 128,
                                                             2 * NOTE)
    rep["batt_b"] = np.asarray(inputs["batt_b"],
                               f32).reshape(2, 128).T.copy()
    rep["matt_w"] = np.asarray(inputs["matt_w"], f32)
    rep["matt_b"] = np.asarray(inputs["matt_b"], f32)[:, None]
    bc = np.asarray(inputs["batt_c"], f32)
    Cb = np.zeros((2 * NOTE, HEADS), f32)
    for h in range(HEADS):
        Cb[h * 32:(h + 1) * 32, h] = bc[h]
    rep["Cb"] = Cb.reshape(2, 128, HEADS)
    mcc = np.asarray(inputs["matt_c"], f32)
    Cm = np.zeros((2 * BEAT, HEADS), f32)
    for h in range(HEADS):
        Cm[h * 16:(h + 1) * 16, h] = mcc[h]
    rep["Cm"] = Cm
    Bf = np.zeros((HEADS, 2 * NOTE), f32)
    for h in range(HEADS):
        Bf[h, h * 32:(h + 1) * 32] = 1.0
    rep["Bfree_b"] = Bf
    Bm = np.zeros((HEADS, 2 * BEAT), f32)
    for h in range(HEADS):
        Bm[h, h * 16:(h + 1) * 16] = 1.0
    rep["Bfree_m"] = Bm
    Ppool = np.zeros((8, 128, 32), f32)
    for k in range(8):
        for p in range(128):
            b = beat[k * 128 + p] - 32 * k
            assert 0 <= b < 32, "beats not aligned to 128-node chunks"
            Ppool[k, p, b] = 1.0
    rep["Ppool"] = Ppool
    b2m = np.full(NB, 1 << 40, np.int64)
    np.minimum.at(b2m, beat, meas)
    Ppoolm = np.zeros((2, 128, 32), f32)
    for half in range(2):
        for p in range(128):
            m_ = b2m[half * 128 + p] - 32 * half
            assert 0 <= m_ < 32
            Ppoolm[half, p, m_] = 1.0
    rep["Ppoolm"] = np.concatenate([Ppoolm[0], Ppoolm[1]], axis=1)
    S_bs = np.zeros((8, NB, 128), f32)
    S_ms = np.zeros((8, NM, 128), f32)
    for k in range(8):
        for p in range(128):
            S_bs[k, beat[k * 128 + p], p] = 1.0
            S_ms[k, meas[k * 128 + p], p] = 1.0
    S_bs_hc = np.zeros((8, 128, 2, 128), f32)
    for k in range(8):
        S_bs_hc[k, :, 0, :] = S_bs[k, 0:128, :]
        S_bs_hc[k, :, 1, :] = S_bs[k, 128:256, :]
    rep["S_bs"] = S_bs_hc.reshape(8, 128, 256).astype(f16)
    rep["S_ms"] = S_ms.astype(f16)
    rep["ident"] = np.eye(128, dtype=f32)
    rep["ident16"] = np.eye(128, dtype=f32).astype(f16)

    def lstm_pack2(wi_f, wh_f, b_f, wi_b, wh_b, b_b, H):
        # pair0 = (f; i), pair1 = (o; 2*g); reference gate order i,f,g,o
        KIN = wi_f.shape[1]
        nkc = max(1, KIN // 128)
        whp = np.zeros((2, 2, H, 2 * H), f32)
        wip = np.zeros((2, nkc, 128, 4 * H), f32)
        bp = np.zeros((2, 1, 4 * H), f32)
        for d, (wi, wh, b) in enumerate(((wi_f, wh_f, b_f),
                                         (wi_b, wh_b, b_b))):
            blk = [wh[i * H:(i + 1) * H] for i in range(4)]  # i,f,g,o
            whp[d, 0, :, 0:H] = blk[1].T
            whp[d, 0, :, H:2 * H] = blk[0].T
            whp[d, 1, :, 0:H] = blk[3].T
            whp[d, 1, :, H:2 * H] = 2.0 * blk[2].T
            wt = wi.T  # (KIN, 4H) col blocks i,f,g,o
            for kc in range(nkc):
                w = wt[kc * 128:(kc + 1) * 128]
                r = w.shape[0]
                wip[d, kc, 0:r, 0:H] = w[:, H:2 * H]
                wip[d, kc, 0:r, H:2 * H] = w[:, 0:H]
                wip[d, kc, 0:r, 2 * H:3 * H] = w[:, 3 * H:4 * H]
                wip[d, kc, 0:r, 3 * H:4 * H] = 2.0 * w[:, 2 * H:3 * H]
            bp[d, 0, 0:H] = b[H:2 * H]
            bp[d, 0, H:2 * H] = b[0:H]
            bp[d, 0, 2 * H:3 * H] = b[3 * H:4 * H]
            bp[d, 0, 3 * H:4 * H] = 2.0 * b[2 * H:3 * H]
        return whp.astype(f16), wip.astype(f16), bp.astype(f16)

    g = lambda n: np.asarray(inputs[n], f32)
    rep["bwhp"], rep["bwip"], rep["bbp"] = lstm_pack2(
        g("blstm_wi_f"), g("blstm_wh_f"), g("blstm_b_f"),
        g("blstm_wi_b"), g("blstm_wh_b"), g("blstm_b_b"), BEAT)
    rep["mwhp"], rep["mwip"], rep["mbp"] = lstm_pack2(
        g("mlstm_wi_f"), g("mlstm_wh_f"), g("mlstm_b_f"),
        g("mlstm_wi_b"), g("mlstm_wh_b"), g("mlstm_b_b"), MEAS)
    idmv64 = np.zeros((128, 64), f32)
    idmv64[64:128] = np.eye(64) * 2.0
    rep["idmv64"] = idmv64.astype(f16)
    idmv32 = np.zeros((64, 32), f32)
    idmv32[32:64] = np.eye(32) * 2.0
    rep["idmv32"] = idmv32.astype(f16)
    rep["bwhp"] = (rep["bwhp"].astype(f32) * 2.0).astype(f16)
    rep["mwhp"] = (rep["mwhp"].astype(f32) * 2.0).astype(f16)
    rep["ident2_16"] = (np.eye(128, dtype=f32) * 2.0).astype(f16)
    rep["J128_2"] = (np.eye(128, dtype=f32)[::-1] * 2.0).astype(f16)
    rep["J64_2"] = (np.eye(64, dtype=f32)[::-1] * 2.0).astype(f16)
    rep["J32"] = np.eye(32, dtype=f32)[::-1].copy()
    for gg in ("g1", "g2"):
        for gate in ("z", "r", "h"):
            w = np.asarray(inputs[f"{gg}_w{gate}"], f32)  # (E, S, SEC)
            dyn = w[:, 192:320, :]                        # (E, 128, SEC)
            dhi = dyn.astype(f16)
            dlo = (dyn - dhi.astype(f32)).astype(f16)
            rep[f"{gg}_w{gate}_dhi"] = \
                dhi.transpose(1, 0, 2).reshape(128, E * SEC).copy()
            rep[f"{gg}_w{gate}_dlo"] = \
                dlo.transpose(1, 0, 2).reshape(128, E * SEC).copy()
            ws = np.zeros((128, E, 2 * SEC), f16)
            ws[0:128, :, 0:SEC] = w[:, 0:128, :].transpose(1, 0, 2)
            ws[0:64, :, SEC:2 * SEC] = w[:, 128:192, :].transpose(1, 0, 2)
            rep[f"{gg}_w{gate}_st"] = ws.reshape(128, E * 2 * SEC)
            rep[f"{gg}_u{gate}16"] = \
                np.asarray(inputs[f"{gg}_u{gate}"], f32).astype(f16)
            rep[f"{gg}_b{gate}16"] = \
                np.asarray(inputs[f"{gg}_b{gate}"], f32)[None, :].astype(f16)

    # beat/measure aggregated adjacency (static message terms for g1)
    if np.array_equal(beat, np.arange(N) // 4):
        adjB_full = adjacency.reshape(E, NB, 4, N).sum(2)
    else:
        Sb = np.zeros((NB, N), f32)
        Sb[beat, np.arange(N)] = 1.0
        adjB_full = np.einsum("bm,emn->ebn", Sb, adjacency)
    if np.array_equal(b2m, np.arange(NB) // 4):
        adjM_full = adjB_full.reshape(E, NM, 4, N).sum(2)
    else:
        Sm = np.zeros((NM, NB), f32)
        Sm[b2m, np.arange(NB)] = 1.0
        adjM_full = np.einsum("mb,ebn->emn", Sm, adjB_full)

    specs = _input_specs()
    # convert replicated entries once
    for kk in list(rep.keys()):
        shape, dt = specs[kk]
        npdt = np.float16 if dt == F16 else np.float32
        rep[kk] = np.ascontiguousarray(
            np.asarray(rep[kk]).reshape(shape).astype(npdt))
    in_maps = []
    for c in range(NCORES):
        sl = slice(c * LOC, (c + 1) * LOC)
        m = dict(rep)
        m["nodes_T_loc"] = np.ascontiguousarray(nodes[sl].T)
        adjc = adjacency[:, :, sl]
        m["adj_sl"] = np.ascontiguousarray(
            adjc.reshape(E, 8, 128, LOC).transpose(0, 2, 1, 3)
            .reshape(E, 128, N).astype(f16))
        m["adjB"] = np.ascontiguousarray(
            adjB_full[:, :, sl].reshape(E, 2, 128, LOC)
            .transpose(0, 2, 1, 3).reshape(E, 128, 2 * LOC).astype(f16))
        m["adjM"] = np.ascontiguousarray(adjM_full[:, :, sl].astype(f16))
        m["S_bs_loc"] = rep["S_bs"][c]
        m["S_ms_loc"] = rep["S_ms"][c]
        in_maps.append(m)
    return in_maps


def kernel(**inputs):
    if "nc" not in _CACHE:
        _CACHE["nc"] = _build_program()
    nc = _CACHE["nc"]
    in_maps = _host_inputs(inputs)
    res = bass_utils.run_bass_kernel_spmd(nc, in_maps,
                                          core_ids=list(range(NCORES)))
    _CACHE["last_res"] = res
    out = res.results[0]["out"]
    return np.asarray(out, np.float32)


# revision 33
# speedup vs baseline: 1.4839x; 1.2223x over previous
"""Trainium2 Bass kernel for nn_IsgnBeatMeasEncoder (gnn_message_passing).

Sharding: destination-node sharding for the gated-graph message passing
(128 dest-nodes/core; per-core adjacency slice resident in SBUF, fp16);
fp16 AllGather of the updated secondary state per graph iteration; nb
(graph-between) computed replicated from the gathered state (no second
collective); static message terms for g1 computed via beat/measure
aggregated adjacency; attention pooling via host-built one-hot matmuls;
BiLSTMs replicated with fw/bw batched and gate inputs pinned in PSUM.
"""
import numpy as np

import concourse.bass as bass
import concourse.mybir as mybir
from concourse import bacc
from concourse.tile import TileContext
from concourse import bass_utils

F32 = mybir.dt.float32
F16 = mybir.dt.float16

N = 1024
E = 10
IN = 78
NOTE = 128
BEAT = 64
MEAS = 32
S = 320
SEC = 128
HEADS = 8
NB = 256
NM = 64
SEQ_ITER = 2
GRAPH_ITER = 2
NCORES = 8
LOC = N // NCORES

FCS = [(0, 128), (128, 64), (192, 128)]  # (start, width); 0,1 static; 2 dyn

_CACHE = {}


def _input_specs():
    sp = dict(
        nodes_T16=((IN, N), F16),
        nodes_T_loc=((IN, LOC), F32),
        note_fc_w16=((IN, NOTE), F16),
        note_fc_b16=((1, NOTE), F16),
        note_fc_w32=((IN, NOTE), F32),
        note_fc_b32=((1, NOTE), F32),
        adj_sl=((E, 128, N), F16),
        adjB=((E, 128, 2 * LOC), F16),
        adjM=((E, NM, LOC), F16),
        gb_w16=((128, 3 * S), F16),
        gb_b16=((1, S), F16),
        gb_w32=((128, 3 * S), F32),
        gb_b32=((1, S), F32),
        batt_w=((2, 128, 2 * NOTE), F16),
        batt_b=((128, 2), F32),
        matt_w=((128, 2 * BEAT), F32),
        matt_b=((128, 1), F32),
        Cb=((2, 128, HEADS), F16),
        Cm=((128, HEADS), F32),
        Bfree_b=((HEADS, 2 * NOTE), F32),
        Bfree_m=((HEADS, 2 * BEAT), F32),
        Ppool=((8, 128, 32), F32),
        Ppoolm=((128, 2 * 32), F32),
        S_bs=((8, 128, 2 * 128), F16),   # half-chunked: [k][p,(half,c)]
        S_ms=((8, NM, 128), F16),
        S_bs_loc=((128, 2 * 128), F16),
        S_ms_loc=((NM, 128), F16),
        ident=((128, 128), F32),
        ident16=((128, 128), F16),
        bwhp=((2, 2, BEAT, 2 * BEAT), F16),   # [d][pair][H,(gA,gB)]
        bwip=((2, 2, 128, 4 * BEAT), F16),    # [d][kc][feat,(p0,p1)]
        bbp=((2, 1, 4 * BEAT), F16),
        mwhp=((2, 2, MEAS, 2 * MEAS), F16),
        mwip=((2, 1, 128, 4 * MEAS), F16),
        mbp=((2, 1, 4 * MEAS), F16),
        idmv64=((128, 64), F16),
        idmv32=((64, 32), F16),
        ident2_16=((128, 128), F16),
        J128_2=((128, 128), F16),
        J64_2=((64, 64), F16),
        J32=((32, 32), F32),
    )
    for g in ("g1", "g2"):
        for gate in ("z", "r", "h"):
            sp[f"{g}_w{gate}_dhi"] = ((128, E * SEC), F16)
            sp[f"{g}_w{gate}_dlo"] = ((128, E * SEC), F16)
            sp[f"{g}_w{gate}_st"] = ((128, E * 2 * SEC), F16)
            sp[f"{g}_u{gate}16"] = ((SEC, SEC), F16)
            sp[f"{g}_b{gate}16"] = ((1, SEC), F16)
    return sp


def _build_program():
    nc = bacc.Bacc("TRN2", target_bir_lowering=False, debug=False,
                   num_devices=NCORES)
    io = {}
    for name, (shape, dt) in _input_specs().items():
        io[name] = nc.dram_tensor(name, list(shape), dt,
                                  kind="ExternalInput").ap()
    out_dram = nc.dram_tensor("out", [1, N, S + SEC], F32,
                              kind="ExternalOutput").ap()
    dbg_dram = nc.dram_tensor("dbg", [16, 128, 512], F32,
                              kind="ExternalOutput").ap()
    _CACHE["dbg_dram"] = dbg_dram
    ag = {}
    for i in range(8):
        ag[f"sec_in{i}"] = nc.dram_tensor(f"sec_in{i}", [LOC, SEC], F16).ap()
        ag[f"sec_out{i}"] = nc.dram_tensor(f"sec_out{i}", [N, SEC], F16,
                                           addr_space="Shared").ap()
    with TileContext(nc) as tc:
        _emit(nc, tc, io, out_dram, ag, dbg_dram)
    nc.compile()
    return nc


def _emit(nc, tc, io, out_dram, ag, dbg_dram):
    import contextlib
    RG = [list(range(NCORES))]
    AF = mybir.ActivationFunctionType
    OP = mybir.AluOpType
    MM = nc.tensor.matmul

    stack = contextlib.ExitStack()
    const = stack.enter_context(tc.tile_pool(name="const", bufs=1))
    pers = stack.enter_context(tc.tile_pool(name="pers", bufs=1))
    acts = stack.enter_context(tc.tile_pool(name="acts", bufs=2))
    dynp = stack.enter_context(tc.tile_pool(name="dynp", bufs=2))
    lsp = stack.enter_context(tc.tile_pool(name="lsp", bufs=6))
    ps_t = stack.enter_context(tc.tile_pool(name="ps_t", bufs=1, space="PSUM"))
    ps_m = stack.enter_context(tc.tile_pool(name="ps_m", bufs=2, space="PSUM"))

    def dma(dst, src):
        nc.sync.dma_start(out=dst, in_=src)

    def dump(idx, src_ap, rows, cols):
        st = acts.tile([128, 512], F32, tag="dbgst", name="dbgst")
        nc.vector.tensor_copy(st[0:rows, 0:cols], src_ap)
        dma(dbg_dram[idx, 0:rows, 0:cols], st[0:rows, 0:cols])

    cst = {}

    def load(name, dt=None, src=None, tag=None):
        src = io[name] if src is None else src
        if dt is None:
            dt = src.dtype
        t = const.tile([src.shape[-2], src.shape[-1]], dt, tag=tag or name)
        dma(t[:, :], src)
        cst[tag or name] = t
        return t

    for nm in ("nodes_T16", "nodes_T_loc", "note_fc_w16", "note_fc_b16",
               "note_fc_w32", "note_fc_b32", "gb_w16", "gb_b16", "gb_w32",
               "gb_b32", "batt_b", "matt_w", "matt_b", "Cm", "Bfree_b",
               "Bfree_m", "Ppoolm", "S_bs_loc", "S_ms_loc", "ident",
               "ident16", "idmv64", "idmv32", "ident2_16", "J128_2",
               "J64_2", "J32"):
        load(nm)
    for kc in range(2):
        load("batt_w", src=io["batt_w"][kc], tag=f"battw{kc}")
        load("Cb", src=io["Cb"][kc], tag=f"Cb{kc}")
    for k in range(8):
        load("Ppool", src=io["Ppool"][k], tag=f"Ppool{k}")
        load("S_bs", src=io["S_bs"][k], tag=f"S_bs{k}")
        load("S_ms", src=io["S_ms"][k], tag=f"S_ms{k}")
    for e in range(E):
        load("adj_sl", src=io["adj_sl"][e], tag=f"adj{e}")
        load("adjB", src=io["adjB"][e], tag=f"adjB{e}")
        load("adjM", src=io["adjM"][e], tag=f"adjM{e}")
    for g in ("g1", "g2"):
        for gate in ("z", "r", "h"):
            load(f"{g}_w{gate}_dhi")
            load(f"{g}_w{gate}_dlo")
            load(f"{g}_w{gate}_st")
            load(f"{g}_u{gate}16")
            load(f"{g}_b{gate}16")
    for d in range(2):
        for p in range(2):
            load("bwhp", src=io["bwhp"][d, p], tag=f"bwhp{d}{p}")
            load("mwhp", src=io["mwhp"][d, p], tag=f"mwhp{d}{p}")
        for kc in range(2):
            load("bwip", src=io["bwip"][d, kc], tag=f"bwip{d}{kc}")
        load("mwip", src=io["mwip"][d, 0], tag=f"mwip{d}0")
        load("bbp", src=io["bbp"][d], tag=f"bbp{d}")
        load("mbp", src=io["mbp"][d], tag=f"mbp{d}")

    ones1 = const.tile([1, 512], F32, tag="ones1", name="ones1")
    nc.gpsimd.memset(ones1[:, :], 1.0)
    ones16 = const.tile([1, 512], F16, tag="ones16", name="ones16")
    nc.gpsimd.memset(ones16[:, :], 1.0)
    zsmall = const.tile([128, 2], F32, tag="zsmall", name="zsmall")
    nc.gpsimd.memset(zsmall[:, :], 0.0)
    z16 = const.tile([128, 2], F16, tag="z16", name="z16")
    nc.gpsimd.memset(z16[:, :], 0.0)
    ident = cst["ident"]
    ident16 = cst["ident16"]

    xb = [pers.tile([128, S], F16, tag=f"xb{k}", name=f"xb{k}")
          for k in range(8)]
    xl = pers.tile([128, S], F32, tag="xl", name="xl")
    nh16 = [pers.tile([128, SEC], F16, tag=f"nh16_{k}", name=f"nh16_{k}")
            for k in range(8)]
    nh216 = [pers.tile([128, SEC], F16, tag=f"nh216_{k}", name=f"nh216_{k}")
             for k in range(8)]
    bnT = [pers.tile([128, NB], F16, tag=f"bnT{h}", name=f"bnT{h}")
           for h in range(2)]
    bnTr = [pers.tile([128, NB], F16, tag=f"bnTr{h}", name=f"bnTr{h}")
            for h in range(2)]
    Hfb = pers.tile([BEAT, NB], F16, tag="Hfb", name="Hfb")
    Hbb = pers.tile([BEAT, NB], F16, tag="Hbb", name="Hbb")
    Hfm = pers.tile([MEAS, NM], F16, tag="Hfm", name="Hfm")
    Hbm = pers.tile([MEAS, NM], F16, tag="Hbm", name="Hbm")
    bh0 = pers.tile([128, 128], F32, tag="bh0", name="bh0")
    bh1 = pers.tile([128, 128], F32, tag="bh1", name="bh1")
    bh16 = [pers.tile([128, 128], F16, tag=f"bh16_{h}", name=f"bh16_{h}")
            for h in range(2)]
    bhT = pers.tile([128, NB], F32, tag="bhT", name="bhT")
    mh = pers.tile([NM, 2 * MEAS], F32, tag="mh", name="mh")
    mh16 = pers.tile([NM, 2 * MEAS], F16, tag="mh16", name="mh16")
    mnT = pers.tile([2 * BEAT, NM], F16, tag="mnT", name="mnT")
    mnTr = pers.tile([2 * BEAT, NM], F16, tag="mnTr", name="mnTr")
    mstat = pers.tile([128, 3 * SEC], F32, tag="mstat", name="mstat")
    nsl = pers.tile([128, SEC], F32, tag="nsl", name="nsl")

    def transpose_to(dst_ap, src_ap, rows):
        cols = src_ap.shape[-1]
        pt = ps_t.tile([128, 128], F32, tag="pt", name="pt")
        nc.tensor.transpose(pt[0:cols, 0:rows], src_ap,
                            ident[0:rows, 0:rows])
        nc.vector.tensor_copy(dst_ap, pt[0:cols, 0:rows])

    def transpose_new(src_ap, rows, tag="tr"):
        cols = src_ap.shape[-1]
        sb = acts.tile([cols, rows], F32, tag=tag, name=tag)
        transpose_to(sb[0:cols, 0:rows], src_ap, rows)
        return sb

    def transpose16(pool, src_ap, rows, tag):
        # fp16 src -> fp16 transposed SBUF tile
        cols = src_ap.shape[-1]
        pt = pool.tile([128, 128], F16, tag="pt16", name="pt16")
        nc.tensor.transpose(pt[0:cols, 0:rows], src_ap,
                            ident16[0:rows, 0:rows])
        sb = dynp.tile([128, 128], F16, tag=tag, name=tag)
        nc.vector.tensor_copy(sb[0:cols, 0:rows], pt[0:cols, 0:rows])
        return sb

    # ---------------- initial x ----------------
    for k in range(8):
        nc.gpsimd.memset(xb[k][:, 0:192], 0.0)
        pm = ps_m.tile([128, 512], F32, tag="pm", name="pm")
        MM(pm[:, 0:NOTE], cst["nodes_T16"][:, k * 128:(k + 1) * 128],
           cst["note_fc_w16"][:, :], start=True, stop=False)
        MM(pm[:, 0:NOTE], ones16[0:1, 0:128], cst["note_fc_b16"][:, :],
           start=False, stop=True)
        nc.scalar.activation(xb[k][:, 192:S], pm[:, 0:NOTE], AF.Tanh)
    nc.gpsimd.memset(xl[:, 0:192], 0.0)
    pm = ps_m.tile([128, 512], F32, tag="pm", name="pm")
    MM(pm[:, 0:NOTE], cst["nodes_T_loc"][:, :], cst["note_fc_w32"][:, :],
       start=True, stop=False)
    MM(pm[:, 0:NOTE], ones1[0:1, 0:LOC], cst["note_fc_b32"][:, :],
       start=False, stop=True)
    nc.scalar.activation(xl[:, 192:S], pm[:, 0:NOTE], AF.Tanh)
    dump(0, xb[0][:, 192:S], 128, 128)

    # ---------------- gated graph ----------------
    def gated_graph(g, static_mode, agins, agouts, nh_tiles, save_local):
        wdh = [cst[f"{g}_w{gt}_dhi"] for gt in "zrh"]
        wdl = [cst[f"{g}_w{gt}_dlo"] for gt in "zrh"]
        wst = [cst[f"{g}_w{gt}_st"] for gt in "zrh"]
        us = [cst[f"{g}_u{gt}16"] for gt in "zrh"]
        bs = [cst[f"{g}_b{gt}16"] for gt in "zrh"]
        with tc.tile_pool(name=f"ga{g}", bufs=1, space="PSUM") as gacc, \
                tc.tile_pool(name=f"gd{g}", bufs=2, space="PSUM") as gact:
            mt = [gacc.tile([128, SEC], F32, tag=f"m{gi}", name=f"m{gi}")
                  for gi in range(3)]
            # ---- static messages (constant across graph iters) ----
            if static_mode != "none":
                for e in range(E):
                    pa = gact.tile([128, 128], F32, tag="pact", name="pact")
                    if static_mode == "bh":
                        for ch in range(2):
                            MM(pa[:, :], bh16[ch][:, :],
                               cst[f"adjB{e}"][:, ch * 128:(ch + 1) * 128],
                               start=(ch == 0), stop=(ch == 1))
                    else:
                        for k in range(8):
                            MM(pa[:, :], xb[k][:, 0:128],
                               cst[f"adj{e}"][:, k * 128:(k + 1) * 128],
                               start=(k == 0), stop=(k == 7))
                    a0 = dynp.tile([128, 128], F16, tag="a0", name="a0")
                    nc.vector.tensor_copy(a0[:, :], pa[:, :])
                    pa1 = gact.tile([128, 128], F32, tag="pact", name="pact")
                    if static_mode == "bh":
                        MM(pa1[0:NM, :], mh16[:, :], cst[f"adjM{e}"][:, :],
                           start=True, stop=True)
                    else:
                        for k in range(8):
                            MM(pa1[0:64, :], xb[k][:, 128:192],
                               cst[f"adj{e}"][:, k * 128:(k + 1) * 128],
                               start=(k == 0), stop=(k == 7))
                    a1 = dynp.tile([128, 128], F16, tag="a1", name="a1")
                    nc.vector.tensor_copy(a1[0:64, :], pa1[0:64, :])
                    for gi in range(3):
                        MM(mt[gi][:, :], a0[:, :],
                           wst[gi][:, e * 256:e * 256 + 128],
                           start=(e == 0), stop=False)
                        MM(mt[gi][:, :], a1[0:64, :],
                           wst[gi][0:64, e * 256 + 128:e * 256 + 256],
                           start=False, stop=(e == E - 1))
                for gi in range(3):
                    nc.vector.tensor_copy(mstat[:, gi * SEC:(gi + 1) * SEC],
                                          mt[gi][:, :])
            # ---- graph iterations ----
            for it in range(GRAPH_ITER):
                last = it == GRAPH_ITER - 1
                for e in range(E):
                    pa = gact.tile([128, 128], F32, tag="pact", name="pact")
                    for k in range(8):
                        MM(pa[:, :], xb[k][:, 192:S],
                           cst[f"adj{e}"][:, k * 128:(k + 1) * 128],
                           start=(k == 0), stop=(k == 7))
                    ad = dynp.tile([128, 128], F16, tag="ad", name="ad")
                    nc.vector.tensor_copy(ad[:, :], pa[:, :])
                    for gi in range(3):
                        MM(mt[gi][:, :], ad[:, :],
                           wdh[gi][:, e * SEC:(e + 1) * SEC],
                           start=(e == 0), stop=False)
                        MM(mt[gi][:, :], ad[:, :],
                           wdl[gi][:, e * SEC:(e + 1) * SEC],
                           start=False, stop=False)
                xs = xl[:, 192:S]
                pt = ps_t.tile([128, 128], F32, tag="pt", name="pt")
                nc.tensor.transpose(pt[:, :], xs, ident[:, :])
                xsT = dynp.tile([128, 128], F16, tag="xsT", name="xsT")
                nc.vector.tensor_copy(xsT[:, :], pt[:, :])
                for gi in range(3):
                    MM(mt[gi][:, :], ones16[0:1, 0:128],
                       bs[gi][:, :], start=False, stop=False)
                for gi in range(2):
                    MM(mt[gi][:, :], xsT[:, :], us[gi][:, :],
                       start=False, stop=True)

                def gate_act(gi, func, dst):
                    reg = mt[gi][:, :]
                    if static_mode != "none":
                        tz = acts.tile([128, SEC], F32, tag="tz", name="tz")
                        nc.vector.tensor_tensor(
                            tz[:, :], reg, mstat[:, gi * SEC:(gi + 1) * SEC],
                            op=OP.add)
                        nc.scalar.activation(dst, tz[:, :], func)
                    else:
                        nc.scalar.activation(dst, reg, func)

                zt = acts.tile([128, SEC], F32, tag="zt", name="zt")
                rt = acts.tile([128, SEC], F32, tag="rt", name="rt")
                gate_act(0, AF.Sigmoid, zt[:, :])
                gate_act(1, AF.Sigmoid, rt[:, :])
                rx = acts.tile([128, SEC], F32, tag="rx", name="rx")
                nc.vector.tensor_tensor(rx[:, :], rt[:, :], xs, op=OP.mult)
                pt2 = ps_t.tile([128, 128], F32, tag="pt", name="pt")
                nc.tensor.transpose(pt2[:, :], rx[:, :], ident[:, :])
                rxT = dynp.tile([128, 128], F16, tag="rxT", name="rxT")
                nc.vector.tensor_copy(rxT[:, :], pt2[:, :])
                MM(mt[2][:, :], rxT[:, :], us[2][:, :],
                   start=False, stop=True)
                ht = acts.tile([128, SEC], F32, tag="ht", name="ht")
                gate_act(2, AF.Tanh, ht[:, :])
                t1 = acts.tile([128, SEC], F32, tag="t1", name="t1")
                nc.vector.tensor_tensor(t1[:, :], zt[:, :], xs, op=OP.mult)
                t2 = acts.tile([128, SEC], F32, tag="t2", name="t2")
                nc.vector.tensor_tensor(t2[:, :], rt[:, :], ht[:, :],
                                        op=OP.mult)
                ns = acts.tile([128, SEC], F32, tag="ns", name="ns")
                nc.vector.tensor_tensor(ns[:, :], xs, t1[:, :],
                                        op=OP.subtract)
                nc.vector.tensor_tensor(ns[:, :], ns[:, :], t2[:, :],
                                        op=OP.add)
                ns16 = acts.tile([128, SEC], F16, tag="ns16", name="ns16")
                nc.vector.tensor_copy(ns16[:, :], ns[:, :])
                if g == "g1" and it == 0 and dbgflag[0]:
                    dump(13, ns[:, :], 128, 128)
                a_in, a_out = agins[it], agouts[it]
                dma(a_in, ns16[:, :])
                nc.gpsimd.collective_compute(
                    "AllGather", OP.bypass, replica_groups=RG,
                    ins=[a_in], outs=[a_out])
                if not last:
                    for k in range(8):
                        dma(xb[k][:, 192:S], a_out[k * 128:(k + 1) * 128, :])
                else:
                    for k in range(8):
                        dma(nh_tiles[k][:, :],
                            a_out[k * 128:(k + 1) * 128, :])
                nc.vector.tensor_copy(xl[:, 192:S], ns[:, :])
                if last and save_local is not None:
                    nc.vector.tensor_copy(save_local[:, :], ns[:, :])

    # ---------------- nb (graph-between), replicated ----------------
    def nb_phase(s):
        fcs = [2] if s == 0 else [0, 1, 2]
        with tc.tile_pool(name="nbp", bufs=2, space="PSUM") as nbp:
          for k in range(8):
            pnb = ps_m.tile([128, 512], F32, tag="pm", name="pm")
            for fc in fcs:
                st, wd = FCS[fc]
                src = nh16[k][:, :] if fc == 2 else xb[k][:, st:st + wd]
                tT = transpose16(nbp, src, 128, tag="nbT")
                MM(pnb[:, 0:S], tT[0:wd, 0:128],
                   cst["gb_w16"][0:wd, fc * S:(fc + 1) * S],
                   start=(fc == fcs[0]), stop=False)
            MM(pnb[:, 0:S], ones16[0:1, 0:128], cst["gb_b16"][:, :],
               start=False, stop=True)
            nc.scalar.activation(xb[k][:, 0:S], pnb[:, 0:S], AF.Relu)
        # local f32 copy (xl holds nh locally: primary + nsl sec)
        pnl = ps_m.tile([128, 512], F32, tag="pm", name="pm")
        for fc in fcs:
            st, wd = FCS[fc]
            tT = transpose_new(xl[:, st:st + wd], 128, tag="nbT32")
            MM(pnl[:, 0:S], tT[0:wd, 0:128],
               cst["gb_w32"][0:wd, fc * S:(fc + 1) * S],
               start=(fc == fcs[0]), stop=False)
        MM(pnl[:, 0:S], ones1[0:1, 0:128], cst["gb_b32"][:, :],
           start=False, stop=True)
        nc.scalar.activation(xl[:, 0:S], pnl[:, 0:S], AF.Relu)

    # ---------------- beat attention ----------------
    def beat_attention():
      with tc.tile_pool(name="bat", bufs=2, space="PSUM") as bap:
        for k in range(8):
            cat_nm = acts.tile([128, 2 * NOTE], F32, tag="cat_nm",
                               name="cat_nm")
            nc.vector.tensor_copy(cat_nm[:, 0:NOTE], nh16[k][:, :])
            nc.vector.tensor_copy(cat_nm[:, NOTE:2 * NOTE], nh216[k][:, :])
            ct = []
            for kc in range(2):
                srct = nh16[k] if kc == 0 else nh216[k]
                ptc = bap.tile([128, 128], F16, tag="ptc", name="ptc")
                nc.tensor.transpose(ptc[:, :], srct[:, :], ident16[:, :])
                sb = dynp.tile([128, 128], F16, tag=f"ct{kc}",
                               name=f"ct{kc}")
                nc.vector.tensor_copy(sb[:, :], ptc[:, :])
                ct.append(sb)
            aT = []
            for mc in range(2):
                pa = bap.tile([128, 128], F32, tag="pa", name="pa")
                for kc in range(2):
                    MM(pa[:, :],
                       cst[f"battw{kc}"][:, mc * 128:(mc + 1) * 128],
                       ct[kc][:, :], start=(kc == 0), stop=(kc == 1))
                sb = acts.tile([128, 128], F16, tag=f"aT{mc}",
                               name=f"aT{mc}")
                nc.scalar.activation(sb[:, :], pa[:, :], AF.Tanh,
                                     bias=cst["batt_b"][:, mc:mc + 1])
                aT.append(sb)
            psim = ps_m.tile([128, 512], F32, tag="pm", name="pm")
            for kc in range(2):
                MM(psim[0:HEADS, 0:128], cst[f"Cb{kc}"][:, :], aT[kc][:, :],
                   start=(kc == 0), stop=(kc == 1))
            pp = acts.tile([HEADS, 128], F32, tag="pp", name="pp")
            nc.scalar.activation(pp[:, :], psim[0:HEADS, 0:128], AF.Sigmoid)
            qq = acts.tile([HEADS, 128], F32, tag="qq", name="qq")
            nc.scalar.activation(qq[:, :], psim[0:HEADS, 0:128], AF.Sigmoid,
                                 scale=-1.0)
            rq = acts.tile([HEADS, 128], F32, tag="rq", name="rq")
            nc.vector.reciprocal(rq[:, :], qq[:, :])
            wt = acts.tile([HEADS, 128], F32, tag="wt", name="wt")
            nc.vector.tensor_tensor(wt[:, :], pp[:, :], rq[:, :], op=OP.mult)
            pwe = ps_m.tile([128, 512], F32, tag="pm", name="pm")
            wexp = acts.tile([128, 2 * NOTE], F32, tag="wexp", name="wexp")
            MM(pwe[:, 0:256], wt[:, :], cst["Bfree_b"][:, :],
               start=True, stop=True)
            nc.vector.tensor_copy(wexp[:, :], pwe[:, 0:256])
            tt = acts.tile([128, 2 * NOTE], F32, tag="tt", name="tt")
            nc.vector.tensor_tensor(tt[:, :], cat_nm[:, :], wexp[:, :],
                                    op=OP.mult)
            pool = ps_m.tile([128, 512], F32, tag="pm", name="pm")
            MM(pool[0:32, 0:256], cst[f"Ppool{k}"][:, :], tt[:, :],
               start=True, stop=True)
            MM(pool[0:32, 256:512], cst[f"Ppool{k}"][:, :], wexp[:, :],
               start=True, stop=True)
            rd = acts.tile([32, 256], F32, tag="rd", name="rd")
            nc.vector.reciprocal(rd[:, :], pool[0:32, 256:512])
            bnk = acts.tile([32, 256], F32, tag="bnk", name="bnk")
            nc.vector.tensor_tensor(bnk[:, :], pool[0:32, 0:256], rd[:, :],
                                    op=OP.mult)
            for h in range(2):
                ptb = ps_m.tile([128, 512], F32, tag="pm", name="pm")
                nc.tensor.transpose(ptb[0:128, 0:32],
                                    bnk[0:32, h * 128:(h + 1) * 128],
                                    ident[0:32, 0:32])
                nc.tensor.transpose(ptb[0:128, 32:64],
                                    bnk[0:32, h * 128:(h + 1) * 128],
                                    cst["J32"][:, :])
                nc.vector.tensor_copy(bnT[h][:, k * 32:(k + 1) * 32],
                                      ptb[0:128, 0:32])
                nc.vector.tensor_copy(bnTr[h][:, (7 - k) * 32:(8 - k) * 32],
                                      ptb[0:128, 32:64])

    # ---------------- measure attention ----------------
    def measure_attention():
        paT = ps_m.tile([128, 512], F32, tag="pm", name="pm")
        MM(paT[:, 0:NB], cst["matt_w"][:, :], bhT[:, :],
           start=True, stop=True)
        amT = acts.tile([128, NB], F32, tag="amT", name="amT")
        nc.scalar.activation(amT[:, :], paT[:, 0:NB], AF.Tanh,
                             bias=cst["matt_b"][:, 0:1])
        psim = ps_t.tile([128, 128], F32, tag="pt", name="pt")
        pp = acts.tile([HEADS, NB], F32, tag="ppm", name="ppm")
        qq = acts.tile([HEADS, NB], F32, tag="qqm", name="qqm")
        for hc in range(2):
            MM(psim[0:HEADS, 0:128], cst["Cm"][:, :],
               amT[:, hc * 128:(hc + 1) * 128], start=True, stop=True)
            nc.scalar.activation(pp[:, hc * 128:(hc + 1) * 128],
                                 psim[0:HEADS, 0:128], AF.Sigmoid)
            nc.scalar.activation(qq[:, hc * 128:(hc + 1) * 128],
                                 psim[0:HEADS, 0:128], AF.Sigmoid,
                                 scale=-1.0)
        rq = acts.tile([HEADS, NB], F32, tag="rqm", name="rqm")
        nc.vector.reciprocal(rq[:, :], qq[:, :])
        wt = acts.tile([HEADS, NB], F32, tag="wtm", name="wtm")
        nc.vector.tensor_tensor(wt[:, :], pp[:, :], rq[:, :], op=OP.mult)
        for h in range(2):
            bh_h = bh0 if h == 0 else bh1
            pwe = ps_m.tile([128, 512], F32, tag="pm", name="pm")
            MM(pwe[:, 0:2 * BEAT], wt[:, h * 128:(h + 1) * 128],
               cst["Bfree_m"][:, :], start=True, stop=True)
            wexp = acts.tile([128, 2 * BEAT], F32, tag="wexpm", name="wexpm")
            nc.vector.tensor_copy(wexp[:, :], pwe[:, 0:2 * BEAT])
            tt = acts.tile([128, 2 * BEAT], F32, tag="ttm", name="ttm")
            nc.vector.tensor_tensor(tt[:, :], bh_h[:, :], wexp[:, :],
                                    op=OP.mult)
            pool = ps_m.tile([128, 512], F32, tag="pm", name="pm")
            MM(pool[0:32, 0:128], cst["Ppoolm"][:, h * 32:(h + 1) * 32],
               tt[:, :], start=True, stop=True)
            MM(pool[0:32, 128:256], cst["Ppoolm"][:, h * 32:(h + 1) * 32],
               wexp[:, :], start=True, stop=True)
            rd = acts.tile([32, 128], F32, tag="rdm", name="rdm")
            nc.vector.reciprocal(rd[:, :], pool[0:32, 128:256])
            mnk = acts.tile([32, 128], F32, tag="mnk", name="mnk")
            nc.vector.tensor_tensor(mnk[:, :], pool[0:32, 0:128], rd[:, :],
                                    op=OP.mult)
            transpose_to(mnT[:, h * 32:(h + 1) * 32], mnk[0:32, :], 32)
            ptr = ps_t.tile([128, 128], F32, tag="pt", name="pt")
            nc.tensor.transpose(ptr[0:128, 0:32], mnk[0:32, :],
                                cst["J32"][:, :])
            nc.vector.tensor_copy(mnTr[:, (1 - h) * 32:(2 - h) * 32],
                                  ptr[0:128, 0:32])

    # ---------------- LSTM ----------------
    def run_lstm2(H, T, inT, inTr, whp, wip, bp, nkc, Hf_t, Hb_t):
        """Decoupled fw/bw scan. Per dir PSUM U [2H, 2T]:
        pair0 cols 0:T = (f @ rows 0:H ; i @ rows H:2H),
        pair1 cols T:2T = (o ; 2*g~). All gates sigmoid (tanh folded as
        2*sigma(2x)-1 with weights/bias pre-scaled). c and h live at rows
        0:H; the i*g~ product is moved from rows H:2H via a PE
        identity-matmul."""
        Hs = [Hf_t, Hb_t]
        with tc.tile_pool(name=f"lu{H}", bufs=1, space="PSUM") as up, \
                tc.tile_pool(name=f"lm{H}", bufs=2, space="PSUM") as mp:
            U = [up.tile([2 * H, 2 * T], F32, tag=f"U{d}", name=f"U{H}{d}")
                 for d in range(2)]
            for d in range(2):
                srcs = inT if d == 0 else inTr
                for p in range(2):
                    reg = U[d][:, p * T:(p + 1) * T]
                    for kc in range(nkc):
                        MM(reg,
                           cst[f"{wip}{d}{kc}"][:, p * 2 * H:(p + 1) * 2 * H],
                           srcs[kc], start=(kc == 0 and p == 0), stop=False)
                    MM(reg, cst[f"{bp}{d}"][0:1, p * 2 * H:(p + 1) * 2 * H],
                       ones16[0:1, 0:T], start=False, stop=False)
            idm = cst["idmv64" if H == 64 else "idmv32"]
            cs = [None, None]
            sio_st = [None, None]
            mv_st = [None, None]

            def mm_stage(d, t):
                rhs = z16[0:H, 0:1] if t == 0 else Hs[d][0:H, t - 1:t]
                MM(U[d][:, t:t + 1], cst[f"{whp}{d}0"][:, :], rhs,
                   start=False, stop=True)
                MM(U[d][:, T + t:T + t + 1], cst[f"{whp}{d}1"][:, :],
                   rhs, start=False, stop=True)

            def sig_stage(d, t):
                sv = lsp.tile([2 * H, 2], F32, tag=f"sio{H}{d}",
                              name=f"sio{H}{d}")
                Uv = U[d].rearrange("p (pr t) -> p pr t", pr=2, t=T)
                nc.scalar.activation(
                    sv[0:2 * H, :].rearrange("p (a b) -> p a b", a=2, b=1),
                    Uv[:, :, t:t + 1], AF.Sigmoid)
                sio_st[d] = sv
                w16 = lsp.tile([2 * H, 2], F16, tag=f"w{H}{d}",
                               name=f"w{H}{d}")
                nc.vector.scalar_tensor_tensor(
                    w16[H:2 * H, 0:1], sv[H:2 * H, 1:2], 0.5,
                    sv[H:2 * H, 0:1], op0=OP.subtract, op1=OP.mult)
                mv = mp.tile([128, 8], F32, tag=f"mv{H}", name=f"mv{H}")
                MM(mv[0:H, 0:1], idm[H:2 * H, 0:H], w16[H:2 * H, 0:1],
                   start=True, stop=True)
                mv_st[d] = mv

            def tail_stage(d, t):
                sv = sio_st[d]
                cprev = zsmall[0:H, 0:1] if t == 0 else cs[d][0:H, 0:1]
                cn = lsp.tile([2 * H, 2], F32, tag=f"cn{H}{d}",
                              name=f"cn{H}{d}")
                nc.vector.scalar_tensor_tensor(
                    cn[0:H, 0:1], cprev, sv[0:H, 0:1],
                    mv_st[d][0:H, 0:1], op0=OP.mult, op1=OP.add)
                cs[d] = cn
                tcx = lsp.tile([2 * H, 2], F32, tag=f"tc{H}{d}",
                               name=f"tc{H}{d}")
                nc.scalar.activation(tcx[0:H, 0:1], cn[0:H, 0:1],
                                     AF.Sigmoid, scale=2.0)
                nc.vector.scalar_tensor_tensor(
                    Hs[d][0:H, t:t + 1], tcx[0:H, 0:1], 0.5,
                    sv[0:H, 1:2], op0=OP.subtract, op1=OP.mult)

            for t in range(T):
                mm_stage(0, t)
                if t > 0:
                    tail_stage(1, t - 1)
                sig_stage(0, t)
                mm_stage(1, t)
                tail_stage(0, t)
                sig_stage(1, t)
            tail_stage(1, T - 1)

    def build_beat():
        with tc.tile_pool(name="bhp", bufs=2, space="PSUM") as bp2:
            for half in range(2):
                bh_h = bh0 if half == 0 else bh1
                ptA = bp2.tile([128, 128], F16, tag="ptA", name="ptA")
                nc.tensor.transpose(ptA[0:128, 0:BEAT],
                                    Hfb[:, half * 128:(half + 1) * 128],
                                    ident16[0:BEAT, 0:BEAT])
                nc.scalar.activation(bh_h[:, 0:BEAT], ptA[0:128, 0:BEAT],
                                     AF.Copy, scale=2.0)
                ptB = bp2.tile([128, 128], F16, tag="ptA", name="ptA")
                nc.tensor.transpose(
                    ptB[0:128, 0:BEAT],
                    Hbb[:, (1 - half) * 128:(2 - half) * 128],
                    ident16[0:BEAT, 0:BEAT])
                C1 = dynp.tile([128, 128], F16, tag="C1", name="C1")
                nc.vector.tensor_copy(C1[0:128, 0:BEAT],
                                      ptB[0:128, 0:BEAT])
                pf = bp2.tile([128, 128], F32, tag="ptF", name="ptF")
                MM(pf[0:128, 0:BEAT], cst["J128_2"][:, :],
                   C1[0:128, 0:BEAT], start=True, stop=True)
                nc.vector.tensor_copy(bh_h[:, BEAT:2 * BEAT],
                                      pf[0:128, 0:BEAT])
                nc.vector.tensor_copy(bh16[half][:, :], bh_h[:, :])
                transpose_to(bhT[:, half * 128:(half + 1) * 128],
                             bh_h[:, :], 128)

    def build_meas():
        with tc.tile_pool(name="mhp", bufs=2, space="PSUM") as mp2:
            ptA = mp2.tile([64, 64], F16, tag="ptA", name="ptA")
            nc.tensor.transpose(ptA[0:NM, 0:MEAS], Hfm[:, :],
                                ident16[0:MEAS, 0:MEAS])
            nc.scalar.activation(mh[:, 0:MEAS], ptA[0:NM, 0:MEAS],
                                 AF.Copy, scale=2.0)
            ptB = mp2.tile([64, 64], F16, tag="ptA", name="ptA")
            nc.tensor.transpose(ptB[0:NM, 0:MEAS], Hbm[:, :],
                                ident16[0:MEAS, 0:MEAS])
            C1 = dynp.tile([128, 128], F16, tag="C1", name="C1")
            nc.vector.tensor_copy(C1[0:NM, 0:MEAS], ptB[0:NM, 0:MEAS])
            pf = mp2.tile([64, 64], F32, tag="ptF", name="ptF")
            MM(pf[0:NM, 0:MEAS], cst["J64_2"][:, :], C1[0:NM, 0:MEAS],
               start=True, stop=True)
            nc.vector.tensor_copy(mh[:, MEAS:2 * MEAS], pf[0:NM, 0:MEAS])
            nc.vector.tensor_copy(mh16[:, :], mh[:, :])

    # ---------------- main sequence ----------------
    dbgflag = [True]
    for s in range(SEQ_ITER):
        dbgflag[0] = s == 0
        with nc.named_scope(f"g1_{s}"):
            gated_graph("g1", "none" if s == 0 else "bh",
                        [ag[f"sec_in{s * 4 + i}"] for i in range(2)],
                        [ag[f"sec_out{s * 4 + i}"] for i in range(2)],
                        nh16, nsl)
        dump(1 if s == 0 else 8, nh16[0][:, :], 128, 128)
        if s == 1:
            dump(9, mstat[:, :], 128, 384)
        with nc.named_scope(f"nb_{s}"):
            nb_phase(s)
        if s == 0:
            dump(2, xb[0][:, 0:S], 128, S)
        with nc.named_scope(f"g2_{s}"):
            gated_graph("g2", "generic",
                        [ag[f"sec_in{s * 4 + 2 + i}"] for i in range(2)],
                        [ag[f"sec_out{s * 4 + 2 + i}"] for i in range(2)],
                        nh216, None)
        if s == 0:
            dump(3, nh216[0][:, :], 128, 128)
        with nc.named_scope(f"batt_{s}"):
            beat_attention()
        if s == 0:
            dump(4, bnT[0][:, :], 128, 256)
        with nc.named_scope(f"blstm_{s}"):
            run_lstm2(BEAT, NB, [bnT[0][:, :], bnT[1][:, :]],
                      [bnTr[0][:, :], bnTr[1][:, :]], "bwhp", "bwip",
                      "bbp", 2, Hfb, Hbb)
            build_beat()
            if s == 0:
                dump(5, bh0[:, :], 128, 128)
        with nc.named_scope(f"matt_{s}"):
            measure_attention()
        with nc.named_scope(f"mlstm_{s}"):
            run_lstm2(MEAS, NM, [mnT[:, :]], [mnTr[:, :]], "mwhp",
                      "mwip", "mbp", 1, Hfm, Hbm)
            build_meas()
            if s == 0:
                dump(6, mh[:, :], NM, 2 * MEAS)
        # rebuild x tiles for next iteration / final output
        with nc.named_scope(f"rebuild_{s}"):
            for k in range(8):
                pbs = ps_m.tile([128, 512], F32, tag="pm", name="pm")
                for half in range(2):
                    MM(pbs[:, 0:128],
                       cst[f"S_bs{k}"][:, half * 128:(half + 1) * 128],
                       bh16[half][:, :], start=(half == 0), stop=(half == 1))
                MM(pbs[:, 128:192], cst[f"S_ms{k}"][:, :], mh16[:, :],
                   start=True, stop=True)
                if s + 1 < SEQ_ITER:
                    nc.vector.tensor_copy(xb[k][:, 0:192], pbs[:, 0:192])
                    nc.vector.tensor_copy(xb[k][:, 192:S], nh16[k][:, :])
                else:
                    outst = acts.tile([128, S + SEC], F32, tag="outst",
                                      name="outst")
                    nc.vector.tensor_copy(outst[:, 0:192], pbs[:, 0:192])
                    nc.vector.tensor_copy(outst[:, 192:S], nh16[k][:, :])
                    nc.vector.tensor_copy(outst[:, S:S + SEC], nh216[k][:, :])
                    dma(out_dram[0, k * 128:(k + 1) * 128, :], outst[:, :])
            if s + 1 < SEQ_ITER:
                pbs = ps_m.tile([128, 512], F32, tag="pm", name="pm")
                for half in range(2):
                    MM(pbs[:, 0:128],
                       cst["S_bs_loc"][:, half * 128:(half + 1) * 128],
                       bh16[half][:, :], start=(half == 0), stop=(half == 1))
                MM(pbs[:, 128:192], cst["S_ms_loc"][:, :], mh16[:, :],
                   start=True, stop=True)
                nc.vector.tensor_copy(xl[:, 0:192], pbs[:, 0:192])
                nc.vector.tensor_copy(xl[:, 192:S], nsl[:, :])
                dump(7, xb[0][:, 0:S], 128, S)
    stack.close()


# ================= host side =================

def _host_inputs(inputs):
    f32, f16 = np.float32, np.float16
    nodes = np.asarray(inputs["nodes"], f32)[0]
    adjacency = np.asarray(inputs["adjacency"], f32)
    beat = np.asarray(inputs["beat_numbers"], np.int64)
    meas = np.asarray(inputs["measure_numbers"], np.int64)
    rep = {}
    rep["nodes_T16"] = nodes.T.astype(f16)
    fw = np.asarray(inputs["note_fc_w"], f32)
    fb = np.asarray(inputs["note_fc_b"], f32)[None, :]
    rep["note_fc_w16"] = fw.astype(f16)
    rep["note_fc_b16"] = fb.astype(f16)
    rep["note_fc_w32"] = fw
    rep["note_fc_b32"] = fb
    gbw = np.zeros((128, 3, S), f32)
    gw = np.asarray(inputs["gb_w"], f32)
    for fc, (st, w) in enumerate(FCS):
        gbw[0:w, fc, :] = gw[st:st + w, :]
    rep["gb_w32"] = gbw.reshape(128, 3 * S)
    rep["gb_b32"] = np.asarray(inputs["gb_b"], f32)[None, :]
    rep["gb_w16"] = rep["gb_w32"].astype(f16)
    rep["gb_b16"] = rep["gb_b32"].astype(f16)
    rep["batt_w"] = np.asarray(inputs["batt_w"], f32).reshape(
        2, 128, 2 * NOTE).astype(f16)
    rep["batt_b"] = np.asarray(inputs["batt_b"],
                               f32).reshape(2, 128).T.copy()
    rep["matt_w"] = np.asarray(inputs["matt_w"], f32)
    rep["matt_b"] = np.asarray(inputs["matt_b"], f32)[:, None]
    bc = np.asarray(inputs["batt_c"], f32)
    Cb = np.zeros((2 * NOTE, HEADS), f32)
    for h in range(HEADS):
        Cb[h * 32:(h + 1) * 32, h] = bc[h]
    rep["Cb"] = Cb.reshape(2, 128, HEADS).astype(f16)
    mcc = np.asarray(inputs["matt_c"], f32)
    Cm = np.zeros((2 * BEAT, HEADS), f32)
    for h in range(HEADS):
        Cm[h * 16:(h + 1) * 16, h] = mcc[h]
    rep["Cm"] = Cm
    Bf = np.zeros((HEADS, 2 * NOTE), f32)
    for h in range(HEADS):
        Bf[h, h * 32:(h + 1) * 32] = 1.0
    rep["Bfree_b"] = Bf
    Bm = np.zeros((HEADS, 2 * BEAT), f32)
    for h in range(HEADS):
        Bm[h, h * 16:(h + 1) * 16] = 1.0
    rep["Bfree_m"] = Bm
    Ppool = np.zeros((8, 128, 32), f32)
    for k in range(8):
        for p in range(128):
            b = beat[k * 128 + p] - 32 * k
            assert 0 <= b < 32, "beats not aligned to 128-node chunks"
            Ppool[k, p, b] = 1.0
    rep["Ppool"] = Ppool
    b2m = np.full(NB, 1 << 40, np.int64)
    np.minimum.at(b2m, beat, meas)
    Ppoolm = np.zeros((2, 128, 32), f32)
    for half in range(2):
        for p in range(128):
            m_ = b2m[half * 128 + p] - 32 * half
            assert 0 <= m_ < 32
            Ppoolm[half, p, m_] = 1.0
    rep["Ppoolm"] = np.concatenate([Ppoolm[0], Ppoolm[1]], axis=1)
    S_bs = np.zeros((8, NB, 128), f32)
    S_ms = np.zeros((8, NM, 128), f32)
    for k in range(8):
        for p in range(128):
            S_bs[k, beat[k * 128 + p], p] = 1.0
            S_ms[k, meas[k * 128 + p], p] = 1.0
    S_bs_hc = np.zeros((8, 128, 2, 128), f32)
    for k in range(8):
        S_bs_hc[k, :, 0, :] = S_bs[k, 0:128, :]
        S_bs_hc[k, :, 1, :] = S_bs[k, 128:256, :]
    rep["S_bs"] = S_bs_hc.reshape(8, 128, 256).astype(f16)
    rep["S_ms"] = S_ms.astype(f16)
    rep["ident"] = np.eye(128, dtype=f32)
    rep["ident16"] = np.eye(128, dtype=f32).astype(f16)

    def lstm_pack2(wi_f, wh_f, b_f, wi_b, wh_b, b_b, H):
        # pair0 = (f; i), pair1 = (o; 2*g); reference gate order i,f,g,o
        KIN = wi_f.shape[1]
        nkc = max(1, KIN // 128)
        whp = np.zeros((2, 2, H, 2 * H), f32)
        wip = np.zeros((2, nkc, 128, 4 * H), f32)
        bp = np.zeros((2, 1, 4 * H), f32)
        for d, (wi, wh, b) in enumerate(((wi_f, wh_f, b_f),
                                         (wi_b, wh_b, b_b))):
            blk = [wh[i * H:(i + 1) * H] for i in range(4)]  # i,f,g,o
            whp[d, 0, :, 0:H] = blk[1].T
            whp[d, 0, :, H:2 * H] = blk[0].T
            whp[d, 1, :, 0:H] = blk[3].T
            whp[d, 1, :, H:2 * H] = 2.0 * blk[2].T
            wt = wi.T  # (KIN, 4H) col blocks i,f,g,o
            for kc in range(nkc):
                w = wt[kc * 128:(kc + 1) * 128]
                r = w.shape[0]
                wip[d, kc, 0:r, 0:H] = w[:, H:2 * H]
                wip[d, kc, 0:r, H:2 * H] = w[:, 0:H]
                wip[d, kc, 0:r, 2 * H:3 * H] = w[:, 3 * H:4 * H]
                wip[d, kc, 0:r, 3 * H:4 * H] = 2.0 * w[:, 2 * H:3 * H]
            bp[d, 0, 0:H] = b[H:2 * H]
            bp[d, 0, H:2 * H] = b[0:H]
            bp[d, 0, 2 * H:3 * H] = b[3 * H:4 * H]
            bp[d, 0, 3 * H:4 * H] = 2.0 * b[2 * H:3 * H]
        return whp.astype(f16), wip.astype(f16), bp.astype(f16)

    g = lambda n: np.asarray(inputs[n], f32)
    rep["bwhp"], rep["bwip"], rep["bbp"] = lstm_pack2(
        g("blstm_wi_f"), g("blstm_wh_f"), g("blstm_b_f"),
        g("blstm_wi_b"), g("blstm_wh_b"), g("blstm_b_b"), BEAT)
    rep["mwhp"], rep["mwip"], rep["mbp"] = lstm_pack2(
        g("mlstm_wi_f"), g("mlstm_wh_f"), g("mlstm_b_f"),
        g("mlstm_wi_b"), g("mlstm_wh_b"), g("mlstm_b_b"), MEAS)
    idmv64 = np.zeros((128, 64), f32)
    idmv64[64:128] = np.eye(64) * 2.0
    rep["idmv64"] = idmv64.astype(f16)
    idmv32 = np.zeros((64, 32), f32)
    idmv32[32:64] = np.eye(32) * 2.0
    rep["idmv32"] = idmv32.astype(f16)
    rep["bwhp"] = (rep["bwhp"].astype(f32) * 2.0).astype(f16)
    rep["mwhp"] = (rep["mwhp"].astype(f32) * 2.0).astype(f16)
    rep["ident2_16"] = (np.eye(128, dtype=f32) * 2.0).astype(f16)
    rep["J128_2"] = (np.eye(128, dtype=f32)[::-1] * 2.0).astype(f16)
    rep["J64_2"] = (np.eye(64, dtype=f32)[::-1] * 2.0).astype(f16)
    rep["J32"] = np.eye(32, dtype=f32)[::-1].copy()
    for gg in ("g1", "g2"):
        for gate in ("z", "r", "h"):
            w = np.asarray(inputs[f"{gg}_w{gate}"], f32)  # (E, S, SEC)
            dyn = w[:, 192:320, :]                        # (E, 128, SEC)
            dhi = dyn.astype(f16)
            dlo = (dyn - dhi.astype(f32)).astype(f16)
            rep[f"{gg}_w{gate}_dhi"] = \
                dhi.transpose(1, 0, 2).reshape(128, E * SEC).copy()
            rep[f"{gg}_w{gate}_dlo"] = \
                dlo.transpose(1, 0, 2).reshape(128, E * SEC).copy()
            ws = np.zeros((128, E, 2 * SEC), f16)
            ws[0:128, :, 0:SEC] = w[:, 0:128, :].transpose(1, 0, 2)
            ws[0:64, :, SEC:2 * SEC] = w[:, 128:192, :].transpose(1, 0, 2)
            rep[f"{gg}_w{gate}_st"] = ws.reshape(128, E * 2 * SEC)
            rep[f"{gg}_u{gate}16"] = \
                np.asarray(inputs[f"{gg}_u{gate}"], f32).astype(f16)
            rep[f"{gg}_b{gate}16"] = \
                np.asarray(inputs[f"{gg}_b{gate}"], f32)[None, :].astype(f16)

    # beat/measure aggregated adjacency (static message terms for g1)
    if np.array_equal(beat, np.arange(N) // 4):
        adjB_full = adjacency.reshape(E, NB, 4, N).sum(2)
    else:
        Sb = np.zeros((NB, N), f32)
        Sb[beat, np.arange(N)] = 1.0
        adjB_full = np.einsum("bm,emn->ebn", Sb, adjacency)
    if np.array_equal(b2m, np.arange(NB) // 4):
        adjM_full = adjB_full.reshape(E, NM, 4, N).sum(2)
    else:
        Sm = np.zeros((NM, NB), f32)
        Sm[b2m, np.arange(NB)] = 1.0
        adjM_full = np.einsum("mb,ebn->emn", Sm, adjB_full)

    specs = _input_specs()
    # convert replicated entries once
    for kk in list(rep.keys()):
        shape, dt = specs[kk]
        npdt = np.float16 if dt == F16 else np.float32
        rep[kk] = np.ascontiguousarray(
            np.asarray(rep[kk]).reshape(shape).astype(npdt))
    in_maps = []
    for c in range(NCORES):
        sl = slice(c * LOC, (c + 1) * LOC)
        m = dict(rep)
        m["nodes_T_loc"] = np.ascontiguousarray(nodes[sl].T)
        adjc = adjacency[:, :, sl]
        m["adj_sl"] = np.ascontiguousarray(
            adjc.reshape(E, 8, 128, LOC).transpose(0, 2, 1, 3)
            .reshape(E, 128, N).astype(f16))
        m["adjB"] = np.ascontiguousarray(
            adjB_full[:, :, sl].reshape(E, 2, 128, LOC)
            .transpose(0, 2, 1, 3).reshape(E, 128, 2 * LOC).astype(f16))
        m["adjM"] = np.ascontiguousarray(adjM_full[:, :, sl].astype(f16))
        m["S_bs_loc"] = rep["S_bs"][c]
        m["S_ms_loc"] = rep["S_ms"][c]
        in_maps.append(m)
    return in_maps


def kernel(**inputs):
    if "nc" not in _CACHE:
        _CACHE["nc"] = _build_program()
    nc = _CACHE["nc"]
    in_maps = _host_inputs(inputs)
    res = bass_utils.run_bass_kernel_spmd(nc, in_maps,
                                          core_ids=list(range(NCORES)))
    _CACHE["last_res"] = res
    out = res.results[0]["out"]
    return np.asarray(out, np.float32)
